# revision 40
# baseline (speedup 1.0000x reference)
"""DiffAttention TRN2 kernel v2: 8-core SPMD (batch x head-group sharding).

Sharding: core c -> batch b = c//4, head-group g = c%4 (4 raw heads = 2
effective pairs per core). Each core computes q/k/v projections for its
head slice, diff-attention, headwise RMSNorm, and a partial output
projection (its 256 input-feature slice of Wo). Host sums the 4 partials
per batch and adds bo.

v2 design notes (cost model: matmul time = out-free-size * cycles/row,
independent of K and M):
  - x shipped bf16 from host; xT produced by DMA XBAR transpose (no PE
    transposes, no PSUM->SBUF copies for xT).
  - all weights bf16 host-side; q-scale folded into Wq/bq; norm_w and
    (1-lambda_init) folded into Wo columns host-side.
  - softmax denominators via tiny matmuls d[128q,1] = e[128k,128q]^T @
    ones (1 cycle each) instead of M=1 ones-matmuls (512 cycles each).
    PSUM pitfall: a matmul with start=True resets a whole accumulation
    granule (wipes neighboring columns mid-accumulation) -- only the
    FIRST matmul of the interleaved group-set carries start=True.
  - diff-combine uses RMSNorm scale-invariance: ou = A0 - c*A1 with
    c = lam*d0/d1 (one per-q broadcast for the diff + one for rstd);
    the reference's +EPS becomes +EPS*d0^2 since ou is d0-scaled.
  - per-q row vectors are assembled via four M=1 PE transposes into a
    [1,512] PSUM row (partition_broadcast from nonzero partition bases
    is broken on HW), then one partition_broadcast.
  - exps widened to 2 PSUM banks ([128,1024]) to amortize Act access.
  - software pipelining: attention matmuls lag their scores/exp by one
    k-group; v chunk 3 / q-proj chunks 1-3 / the output projection are
    deferred into the attention loop to fill PE stall slots; boundary
    (combine + RMSNorm) work for iteration i runs inside iteration i+1.
  - PSUM budget (8 banks): scores 2x[128,1024]=4, A0/A1=2, dacc=1 (both
    parities share one bank), deferred-work/transposes=1.
"""

import sys

sys.path.insert(0, "/opt/trn_rl_repo")

import numpy as np
import ml_dtypes

import concourse.bass as bass  # noqa: F401
import concourse.mybir as mybir
import concourse.tile as tile
from concourse import bacc
from concourse.bass_utils import run_bass_kernel_spmd
from concourse.masks import make_identity

# Steer every activation to the one table set containing both Exp and Ln
# (no ACT table thrash). See v1 notes: hide used funcs from other sets so
# the chooser can't pick them.
_orig_gat = bacc.get_activation_tables
_PREF_SET = "natural_log_exp_and_others"
def _gat_pref(arch):
    tabs = _orig_gat(arch)
    if _PREF_SET not in tabs:
        return tabs
    used = {mybir.ActivationFunctionType.Exp, mybir.ActivationFunctionType.Ln,
            mybir.ActivationFunctionType.Identity, mybir.ActivationFunctionType.Copy,
            mybir.ActivationFunctionType.Square}
    out = {}
    for name, fns in tabs.items():
        if name == _PREF_SET:
            out[name] = fns
        else:
            out[name] = {f for f in fns if f not in used}
    return out
bacc.get_activation_tables = _gat_pref

F32 = mybir.dt.float32
F32R = mybir.dt.float32r
BF16 = mybir.dt.bfloat16
AL = mybir.AluOpType
AF = mybir.ActivationFunctionType
BF = ml_dtypes.bfloat16

B, N, DIM = 2, 2048, 1024
NUM_HEADS = 16
EFF = NUM_HEADS // 2
HD = DIM // NUM_HEADS          # 64
HPC = 4                        # raw heads per core
PPC = 2                        # head pairs per core
DLOC = HPC * HD                # 256 local feature dims
LAMBDA_INIT = 0.8
EPS = 1e-5
SCALE = HD ** -0.5

NT = N // 128                  # 16 token tiles of 128
NC4 = N // 512                 # 4 chunks of 512 tokens
KB = N // 128                  # 16 k-blocks
NG = KB // 2                   # 8 k-block pair groups

_CACHE = {}


def _build_nc():
    nc = bacc.Bacc()

    x_ = nc.declare_dram_parameter("x", [N, DIM], BF16, isOutput=False)
    # weights shipped pre-tiled: [128, 8*DLOC] where slice i*DLOC:(i+1)*DLOC
    # is the [128, DLOC] tile for input-feature block i (host reshapes).
    wqT = nc.declare_dram_parameter("wqT", [128, 8 * DLOC], BF16, isOutput=False)
    wkT = nc.declare_dram_parameter("wkT", [128, 8 * DLOC], BF16, isOutput=False)
    wvT = nc.declare_dram_parameter("wvT", [128, 8 * DLOC], BF16, isOutput=False)
    woT = nc.declare_dram_parameter("woT", [128, 2 * DIM], BF16, isOutput=False)
    # bpack cols: 0-1 bq (scale-folded) fc halves, 2-3 bk fc halves
    bpack_ = nc.declare_dram_parameter("bpack", [128, 4], F32, isOutput=False)
    bv_ = nc.declare_dram_parameter("bv", [DLOC], F32, isOutput=False)
    # lrow: [lq1 | lk1 | lq2 | lk2]
    lrow_ = nc.declare_dram_parameter("lrow", [4 * HD], F32, isOutput=False)
    out_ = nc.declare_dram_parameter("out", [N, DIM], F32, isOutput=True)

    with tile.TileContext(nc) as tc:
        with tc.tile_pool(name="persist", bufs=1) as pp:
            # ---- constants ----
            identf = pp.tile([128, 128], F32, tag="identf")
            make_identity(nc, identf[:])
            ones_bf = pp.tile([128, 1], BF16, tag="ones")
            nc.vector.memset(ones_bf[:], 1.0)
            ones_f = pp.tile([128, 1], F32, tag="onesf")
            nc.vector.memset(ones_f[:], 1.0)

            # ---- DMA emission order is the DMA service order: feed the
            # first k/v projections ASAP (wk, xT chunk 0, wv), then smalls,
            # then the rest of x interleaved with remaining weights. ----
            xT = [pp.tile([128, N], BF16, tag=f"xT{i}", name=f"xT{i}") for i in range(8)]

            def emit_xt_chunk(tc4):
                for i in range(8):
                    nc.sync.dma_start_transpose(
                        xT[i][:, tc4 * 512:(tc4 + 1) * 512],
                        x_[tc4 * 512:(tc4 + 1) * 512, i * 128:(i + 1) * 128],
                    )

            wk_all = pp.tile([128, 8 * DLOC], BF16, tag="wk")
            nc.sync.dma_start(wk_all[:], wkT[:])
            bpack_t = pp.tile([128, 4], F32, tag="bpack")
            nc.sync.dma_start(bpack_t[:], bpack_[:])
            bq_t = [bpack_t[:, fc:fc + 1] for fc in range(2)]
            bk_t = [bpack_t[:, 2 + fc:3 + fc] for fc in range(2)]
            emit_xt_chunk(0)
            emit_xt_chunk(1)
            wv_all = pp.tile([128, 8 * DLOC], BF16, tag="wv")
            nc.sync.dma_start(wv_all[:], wvT[:])

            # ---- small DMAs (v bias, lambda) ----
            bv_row = pp.tile([1, DLOC], F32, tag="bvrow")
            nc.sync.dma_start(bv_row[:], bv_[:].rearrange("(one f) -> one f", one=1))
            bv_bc = pp.tile([128, DLOC], F32, tag="bvbc")
            nc.gpsimd.partition_broadcast(bv_bc[:], bv_row[:])

            lrow = pp.tile([1, 4 * HD], F32, tag="lrow")
            nc.sync.dma_start(lrow[:], lrow_[:].rearrange("(one f) -> one f", one=1))
            lprod = pp.tile([1, 2 * HD], F32, tag="lprod")
            nc.vector.tensor_mul(lprod[:, 0:HD], lrow[:, 0:HD], lrow[:, HD:2 * HD])
            nc.vector.tensor_mul(lprod[:, HD:2 * HD], lrow[:, 2 * HD:3 * HD], lrow[:, 3 * HD:4 * HD])
            lsum = pp.tile([1, 2], F32, tag="lsum")
            nc.vector.tensor_reduce(lsum[:, 0:1], lprod[:, 0:HD], mybir.AxisListType.X, AL.add)
            nc.vector.tensor_reduce(lsum[:, 1:2], lprod[:, HD:2 * HD], mybir.AxisListType.X, AL.add)
            lexp = pp.tile([1, 2], F32, tag="lexp")
            nc.scalar.activation(lexp[:], lsum[:], AF.Exp)
            lam_t = pp.tile([1, 1], F32, tag="lam")
            nc.vector.tensor_sub(lam_t[:], lexp[:, 0:1], lexp[:, 1:2])
            nc.vector.tensor_scalar_add(lam_t[:], lam_t[:], LAMBDA_INIT)
            lam_col = pp.tile([128, 1], F32, tag="lamc")
            nc.gpsimd.partition_broadcast(lam_col[:], lam_t[:])

            # ---- remaining bulk loads ----
            emit_xt_chunk(2)
            wq_all = pp.tile([128, 8 * DLOC], BF16, tag="wq")
            nc.sync.dma_start(wq_all[:], wqT[:])
            emit_xt_chunk(3)
            wo_all = pp.tile([128, 2 * DIM], BF16, tag="wo")
            nc.sync.dma_start(wo_all[:], woT[:])

            def wk_i(i):
                return wk_all[:, i * DLOC:(i + 1) * DLOC]

            def wv_i(i):
                return wv_all[:, i * DLOC:(i + 1) * DLOC]

            def wq_i(i):
                return wq_all[:, i * DLOC:(i + 1) * DLOC]

            def wo_p(p):
                return wo_all[:, p * DIM:(p + 1) * DIM]

            # ---- persistent activations ----
            qT_sb = [pp.tile([128, N], BF16, tag=f"qT{fc}", name=f"qT{fc}") for fc in range(2)]
            kT_sb = [pp.tile([128, N], BF16, tag=f"kT{fc}", name=f"kT{fc}") for fc in range(2)]
            v_sb = [pp.tile([128, DLOC], BF16, tag=f"v{tt}", name=f"v{tt}") for tt in range(NT)]
            oTn = [pp.tile([128, N], BF16, tag=f"oTn{p}", name=f"oTn{p}") for p in range(PPC)]

            # ================= PHASE A: k/v projections + qT chunk 0 ========
            with tc.tile_pool(name="pja", bufs=3, space="PSUM") as pj_ps:
                def emit_kproj(fc, tc4):
                    tsl = slice(tc4 * 512, (tc4 + 1) * 512)
                    ps = pj_ps.tile([128, 512], F32, tag="pj", name="kps")
                    for i in range(8):
                        nc.tensor.matmul(ps[:], wk_i(i)[:, fc * 128:(fc + 1) * 128],
                                         xT[i][:, tsl], start=(i == 0), stop=(i == 7))
                    nc.vector.tensor_scalar(kT_sb[fc][:, tsl], ps[:], 1.0, bk_t[fc],
                                            AL.mult, AL.add)

                def emit_vproj(tc4, tt, pool=None):
                    t = tc4 * 4 + tt
                    pool = pool if pool is not None else pj_ps
                    ps = pool.tile([128, DLOC], F32,
                                   tag="pj" if pool is pj_ps else "t", name="vps")
                    for i in range(8):
                        nc.tensor.matmul(ps[:], xT[i][:, t * 128:(t + 1) * 128],
                                         wv_i(i), start=(i == 0), stop=(i == 7))
                    nc.vector.scalar_tensor_tensor(v_sb[t][:], ps[:], 1.0, bv_bc[:],
                                                   AL.mult, AL.add)

                def emit_qproj(fc, tc4, pool):
                    tsl = slice(tc4 * 512, (tc4 + 1) * 512)
                    ps = pool.tile([128, 512], F32, tag="t", name="qps")
                    for i in range(8):
                        nc.tensor.matmul(ps[:], wq_i(i)[:, fc * 128:(fc + 1) * 128],
                                         xT[i][:, tsl], start=(i == 0), stop=(i == 7))
                    nc.vector.tensor_scalar(qT_sb[fc][:, tsl], ps[:], 1.0, bq_t[fc],
                                            AL.mult, AL.add)

                emit_kproj(0, 0)
                emit_kproj(1, 0)
                emit_kproj(0, 1)
                emit_kproj(1, 1)
                for tt in range(4):
                    emit_vproj(0, tt)
                emit_kproj(0, 2)
                emit_kproj(1, 2)
                for tt in range(4):
                    emit_vproj(1, tt)
                emit_kproj(0, 3)
                emit_kproj(1, 3)
                for tt in range(4):
                    emit_vproj(2, tt)
                for fc in range(2):
                    emit_qproj(fc, 0, pj_ps)

            # ========== PHASE B: attention + norm + output projection ======
            with (
                tc.tile_pool(name="s_ps", bufs=2, space="PSUM") as s_ps,
                tc.tile_pool(name="a_ps", bufs=2, space="PSUM") as a_ps,
                tc.tile_pool(name="d_ps", bufs=1, space="PSUM") as d_ps,
                tc.tile_pool(name="t_ps", bufs=1, space="PSUM") as t_ps,
                tc.tile_pool(name="e_sb", bufs=6) as e_pool,
                tc.tile_pool(name="cmb", bufs=8) as cmb_pool,
                tc.tile_pool(name="sqp", bufs=2) as sq_pool,
                tc.tile_pool(name="row", bufs=14) as row_pool,
                tc.tile_pool(name="bc", bufs=6) as bc_pool,
                tc.tile_pool(name="o_sb", bufs=4) as o_sb,
            ):
                # one persistent PSUM bank for all denominator accumulators;
                # (qc,p) parity picks column block 0:8 or 8:16.
                dacc_all = d_ps.tile([128, 16], F32, tag="dacc", name="dacc_all")

                def emit_scores(qc, p, g, A0, A1, dcols):
                    """Scores + exp for k-blocks 2g, 2g+1; returns pending
                    attn work (consumed one group later to keep PE fed)."""
                    qsl = slice(qc * 512, (qc + 1) * 512)
                    kb0 = 2 * g
                    s0 = s_ps.tile([128, 1024], F32, tag="s", name="s0")
                    s1 = s_ps.tile([128, 1024], F32, tag="s", name="s1")
                    for j in range(2):
                        ksl = slice((kb0 + j) * 128, (kb0 + j + 1) * 128)
                        nc.tensor.matmul(s0[:, j * 512:(j + 1) * 512],
                                         kT_sb[p][0:64, ksl],
                                         qT_sb[p][0:64, qsl],
                                         start=True, stop=True, skip_group_check=True)
                        nc.tensor.matmul(s1[:, j * 512:(j + 1) * 512],
                                         kT_sb[p][64:128, ksl],
                                         qT_sb[p][64:128, qsl],
                                         start=True, stop=True, skip_group_check=True)
                    e0 = e_pool.tile([128, 1024], BF16, tag="e", name="e0")
                    nc.scalar.activation(e0[:], s0[:], AF.Exp)
                    e1 = e_pool.tile([128, 1024], BF16, tag="e", name="e1")
                    nc.scalar.activation(e1[:], s1[:], AF.Exp)
                    return (p, g, e0, e1, A0, A1, dcols)

                def emit_attn(w):
                    """v-contraction + tiny denominator matmuls for a group."""
                    p, g, e0, e1, A0, A1, dcols = w
                    for j in range(2):
                        kb = 2 * g + j
                        st, sp = (kb == 0), (kb == KB - 1)
                        vt = v_sb[kb][:, p * 128:(p + 1) * 128]
                        jsl = slice(j * 512, (j + 1) * 512)
                        nc.tensor.matmul(A0[:], vt, e0[:, jsl], start=st, stop=sp)
                        nc.tensor.matmul(A1[:], vt, e1[:, jsl], start=st, stop=sp)
                        for s4 in range(4):
                            esl = slice(j * 512 + s4 * 128, j * 512 + (s4 + 1) * 128)
                            # PSUM start=True resets a whole granule (wipes
                            # neighboring columns): reset only on the FIRST
                            # matmul of the group-set; the rest accumulate
                            # onto the zeroed granule.
                            st_d = st and s4 == 0
                            nc.tensor.matmul(dacc_all[:, dcols + s4:dcols + s4 + 1],
                                             e0[:, esl], ones_bf[:],
                                             start=st_d, stop=sp, skip_group_check=True)
                            nc.tensor.matmul(dacc_all[:, dcols + 4 + s4:dcols + 5 + s4],
                                             e1[:, esl], ones_bf[:],
                                             start=False, stop=sp, skip_group_check=True)

                def boundary0(qc, p, A0, A1, dcols, copy=True):
                    """Right after kb15: free A banks via SBUF copies; start
                    the c = lam*d0/d1 chain on DVE."""
                    if copy:
                        A0c = cmb_pool.tile([128, 512], F32, tag="m", name="A0c")
                        nc.vector.tensor_copy(A0c[:], A0[:])
                        A1c = cmb_pool.tile([128, 512], F32, tag="m", name="A1c")
                        nc.vector.tensor_copy(A1c[:], A1[:])
                    else:
                        A0c, A1c = A0, A1
                    cinv = row_pool.tile([128, 4], F32, tag="row", name="cinv")
                    nc.vector.reciprocal(cinv[:], dacc_all[:, dcols + 4:dcols + 8])
                    cl = row_pool.tile([128, 4], F32, tag="row", name="cl")
                    nc.vector.tensor_mul(cl[:], cinv[:], dacc_all[:, dcols:dcols + 4])
                    nc.vector.tensor_scalar(cl[:], cl[:], lam_col[:, 0:1], None,
                                            AL.mult, AL.bypass)
                    # ou is d0-scaled, so the reference's +EPS inside the
                    # RMSNorm becomes +EPS*d0^2. Capture it now -- this dacc
                    # parity slot is re-accumulated two iterations later.
                    d0c = row_pool.tile([128, 4], F32, tag="row", name="d0c")
                    nc.vector.tensor_copy(d0c[:], dacc_all[:, dcols:dcols + 4])
                    epsd = row_pool.tile([128, 4], F32, tag="row", name="epsd")
                    nc.vector.tensor_mul(epsd[:], d0c[:], d0c[:])
                    nc.vector.tensor_scalar(epsd[:], epsd[:], EPS, None,
                                            AL.mult, AL.bypass)
                    return (qc, p, A0c, A1c, cl, epsd)

                def bcast_cols(vec4, name):
                    """[128,4] per-(qsub) column vector -> [128,512] broadcast.
                    Four M=1 PE transposes assemble a [1,512] row (partition 0
                    only -- partition_broadcast from nonzero bases is broken on
                    HW), then one broadcast."""
                    vT5 = t_ps.tile([1, 512], F32, tag="t", name=f"{name}T")
                    for s4 in range(4):
                        nc.tensor.matmul(vT5[0:1, s4 * 128:(s4 + 1) * 128],
                                         vec4[:, s4:s4 + 1], identf[:],
                                         is_transpose=True, start=True, stop=True,
                                         skip_group_check=True)
                    row = row_pool.tile([1, 512], F32, tag="rowr", name=f"{name}row")
                    nc.vector.tensor_copy(row[:], vT5[:])
                    bc = bc_pool.tile([128, 512], F32, tag="bc", name=f"{name}bc")
                    nc.gpsimd.partition_broadcast(bc[:], row[:])
                    return bc

                def boundary1(st1):
                    """Diff-combine: ou = A0 - c*A1 (c broadcast per qsub)."""
                    qc, p, A0c, A1c, cl, epsd = st1
                    bcc = bcast_cols(cl, "c")
                    m1 = cmb_pool.tile([128, 512], F32, tag="m", name="m1")
                    nc.vector.tensor_mul(m1[:], A1c[:], bcc[:])
                    ou = cmb_pool.tile([128, 512], F32, tag="m", name="ou")
                    nc.vector.tensor_sub(ou[:], A0c[:], m1[:])
                    return (qc, p, ou, epsd)

                def boundary2(st2, tail=False):
                    """RMSNorm (scale-invariant: ou is d0-scaled, harmless)."""
                    qc, p, ou, epsd = st2
                    qsl = slice(qc * 512, (qc + 1) * 512)
                    sq = sq_pool.tile([128, 512], BF16, tag="sq", name="sq")
                    if tail:
                        nc.scalar.activation(sq[:], ou[:], AF.Square)
                    else:
                        nc.vector.tensor_mul(sq[:], ou[:], ou[:])
                    rms = t_ps.tile([128, 4], F32, tag="t", name="rms")
                    for s4 in range(4):
                        nc.tensor.matmul(rms[:, s4:s4 + 1],
                                         sq[:, s4 * 128:(s4 + 1) * 128], ones_bf[:],
                                         start=True, stop=True, skip_group_check=True)
                    msn = row_pool.tile([128, 4], F32, tag="row", name="msn")
                    nc.vector.scalar_tensor_tensor(msn[:], rms[:], 1.0 / (2 * HD),
                                                   epsd[:], AL.mult, AL.add)
                    lnm = row_pool.tile([128, 4], F32, tag="row", name="lnm")
                    nc.scalar.activation(lnm[:], msn[:], AF.Ln)
                    rstd = row_pool.tile([128, 4], F32, tag="row", name="rstd")
                    nc.scalar.activation(rstd[:], lnm[:], AF.Exp, scale=-0.5)
                    bcr = bcast_cols(rstd, "r")
                    nc.vector.tensor_mul(oTn[p][:, qsl], ou[:], bcr[:])

                def emit_outproj_tile(qc, tt4, pool, split_dma=False):
                    tt = qc * 4 + tt4
                    tsl = slice(tt * 128, (tt + 1) * 128)
                    ot = o_sb.tile([128, DIM], F32, tag="ot", name="ot")
                    for oc in range(2):
                        osl = slice(oc * 512, (oc + 1) * 512)
                        ps = pool.tile([128, 512], F32, tag="t" if pool is t_ps else "A",
                                       name="ops")
                        for p in range(PPC):
                            nc.tensor.matmul(ps[:], oTn[p][:, tsl],
                                             wo_p(p)[:, osl],
                                             start=(p == 0), stop=(p == PPC - 1))
                        if split_dma:
                            nc.scalar.activation(ot[:, osl], ps[:], AF.Copy)
                        else:
                            nc.vector.tensor_copy(ot[:, osl], ps[:])
                        if split_dma:
                            nc.sync.dma_start(out_[tsl, osl], ot[:, osl])
                    if not split_dma:
                        nc.sync.dma_start(out_[tsl, :], ot[:])

                pend_attn = None   # attn work lagging one group behind scores
                prev_acc = None    # (qc, p, A0, A1, dcols) awaiting boundary0
                pend1 = None       # awaiting boundary1 (diff-combine)
                pend2 = None       # awaiting boundary2 (RMSNorm)
                parity = 0
                for qc in range(NC4):
                    for p in range(PPC):
                        A0 = a_ps.tile([128, 512], F32, tag="A", name="A0")
                        A1 = a_ps.tile([128, 512], F32, tag="A", name="A1")
                        dcols = parity * 8
                        for g in range(NG):
                            w = emit_scores(qc, p, g, A0, A1, dcols)
                            if pend_attn is not None:
                                emit_attn(pend_attn)
                            pend_attn = w
                            if qc == 0 and p == 0 and g < 4:
                                emit_vproj(3, g, t_ps)
                            if g == 0 and prev_acc is not None:
                                pend1 = boundary0(*prev_acc)
                                prev_acc = None
                            if g == 2 and pend1 is not None:
                                pend2 = boundary1(pend1)
                                pend1 = None
                            if g == 4 and pend2 is not None:
                                boundary2(pend2)
                                pend2 = None
                            if g == 5 and p == 0 and qc < NC4 - 1:
                                for fc in range(2):
                                    emit_qproj(fc, qc + 1, t_ps)
                            if p == 1 and qc > 0 and g in (2, 3, 4, 5):
                                emit_outproj_tile(qc - 1, g - 2, t_ps)
                        prev_acc = (qc, p, A0, A1, dcols)
                        parity ^= 1
                # tail: finish last (qc=3, p=1); out-projection tiles alternate
                # between the freed A banks and the t bank for 2-wide pipelining
                emit_attn(pend_attn)
                boundary2(boundary1(boundary0(*prev_acc, copy=False)), tail=True)
                for tt4 in range(4):
                    emit_outproj_tile(NC4 - 1, tt4, a_ps if tt4 % 2 == 0 else t_ps,
                                      split_dma=True)

    nc.finalize()
    return nc


def kernel(x, Wq, bq, Wk, bk, Wv, bv, Wo, bo, norm_w, lq1, lk1, lq2, lk2):
    x = np.asarray(x, dtype=np.float32)
    Wq = np.asarray(Wq, dtype=np.float32)
    Wk = np.asarray(Wk, dtype=np.float32)
    Wv = np.asarray(Wv, dtype=np.float32)
    Wo = np.asarray(Wo, dtype=np.float32)
    nw = np.asarray(norm_w, np.float32)

    if "nc" not in _CACHE:
        _CACHE["nc"] = _build_nc()
    nc = _CACHE["nc"]

    in_maps = []
    for c in range(8):
        b, g = c // 4, c % 4
        sl = slice(g * DLOC, (g + 1) * DLOC)
        # fold q-scale into Wq/bq; fold norm_w * (1-lambda_init) into Wo.
        # pre-tile [DIM, DLOC] -> [128, 8*DLOC] (block i at cols i*DLOC:)
        def tile8(w):  # w: [1024, 256] -> [128, 2048]
            return np.ascontiguousarray(
                w.reshape(8, 128, DLOC).transpose(1, 0, 2).reshape(128, 8 * DLOC))

        wo_scale = (nw[np.arange(DLOC) % (2 * HD)] * (1.0 - LAMBDA_INIT)).astype(np.float32)
        woT_full = (Wo[:, sl] * wo_scale[None, :]).T  # [256, 1024]
        in_maps.append({
            "x": np.ascontiguousarray(x[b]).astype(BF),
            "wqT": tile8((Wq[sl, :] * SCALE).T).astype(BF),
            "wkT": tile8(Wk[sl, :].T).astype(BF),
            "wvT": tile8(Wv[sl, :].T).astype(BF),
            "woT": np.ascontiguousarray(
                woT_full.reshape(2, 128, DIM).transpose(1, 0, 2).reshape(128, 2 * DIM)).astype(BF),
            "bpack": np.ascontiguousarray(np.stack([
                np.asarray(bq, np.float32)[sl][0:128] * SCALE,
                np.asarray(bq, np.float32)[sl][128:256] * SCALE,
                np.asarray(bk, np.float32)[sl][0:128],
                np.asarray(bk, np.float32)[sl][128:256],
            ], axis=1)),
            "bv": np.ascontiguousarray(np.asarray(bv, np.float32)[sl]),
            "lrow": np.ascontiguousarray(np.concatenate([
                np.asarray(lq1, np.float32), np.asarray(lk1, np.float32),
                np.asarray(lq2, np.float32), np.asarray(lk2, np.float32)])),
        })

    res = None
    for attempt in range(3):
        try:
            res = run_bass_kernel_spmd(nc, in_maps, list(range(8))).results
            break
        except Exception:
            if attempt == 2:
                raise
            import time as _time

            _time.sleep(5 + 10 * attempt)
            try:
                import jax

                jax.clear_caches()
                jax.extend.backend.clear_backends()
            except Exception:
                pass
    bo_f = np.asarray(bo, np.float32)
    out = np.empty((B, N, DIM), np.float32)
    for b in range(B):
        acc = res[4 * b]["out"].astype(np.float32)
        for g in range(1, 4):
            acc = acc + res[4 * b + g]["out"]
        out[b] = acc + bo_f[None, :]
    return out


# revision 44
# speedup vs baseline: 1.0027x; 1.0027x over previous
"""DiffAttention TRN2 kernel v2: 8-core SPMD (batch x head-group sharding).

Sharding: core c -> batch b = c//4, head-group g = c%4 (4 raw heads = 2
effective pairs per core). Each core computes q/k/v projections for its
head slice, diff-attention, headwise RMSNorm, and a partial output
projection (its 256 input-feature slice of Wo). Host sums the 4 partials
per batch and adds bo.

v2 design notes (cost model: matmul time = out-free-size * cycles/row,
independent of K and M):
  - x shipped bf16 from host; xT produced by DMA XBAR transpose (no PE
    transposes, no PSUM->SBUF copies for xT).
  - all weights bf16 host-side; q-scale folded into Wq/bq; norm_w and
    (1-lambda_init) folded into Wo columns host-side.
  - softmax denominators via tiny matmuls d[128q,1] = e[128k,128q]^T @
    ones (1 cycle each) instead of M=1 ones-matmuls (512 cycles each).
    PSUM pitfall: a matmul with start=True resets a whole accumulation
    granule (wipes neighboring columns mid-accumulation) -- only the
    FIRST matmul of the interleaved group-set carries start=True.
  - diff-combine uses RMSNorm scale-invariance: ou = A0 - c*A1 with
    c = lam*d0/d1 (one per-q broadcast for the diff + one for rstd);
    the reference's +EPS becomes +EPS*d0^2 since ou is d0-scaled.
  - per-q row vectors are assembled via four M=1 PE transposes into a
    [1,512] PSUM row (partition_broadcast from nonzero partition bases
    is broken on HW), then one partition_broadcast.
  - exps widened to 2 PSUM banks ([128,1024]) to amortize Act access.
  - software pipelining: attention matmuls lag their scores/exp by one
    k-group; v chunk 3 / q-proj chunks 1-3 / the output projection are
    deferred into the attention loop to fill PE stall slots; boundary
    (combine + RMSNorm) work for iteration i runs inside iteration i+1.
  - PSUM budget (8 banks): scores 2x[128,1024]=4, A0/A1=2, dacc=1 (both
    parities share one bank), deferred-work/transposes=1.
"""

import sys

sys.path.insert(0, "/opt/trn_rl_repo")

import numpy as np
import ml_dtypes

import concourse.bass as bass  # noqa: F401
import concourse.mybir as mybir
import concourse.tile as tile
from concourse import bacc
from concourse.bass_utils import run_bass_kernel_spmd
from concourse.masks import make_identity

# Steer every activation to the one table set containing both Exp and Ln
# (no ACT table thrash). See v1 notes: hide used funcs from other sets so
# the chooser can't pick them.
_orig_gat = bacc.get_activation_tables
_PREF_SET = "natural_log_exp_and_others"
def _gat_pref(arch):
    tabs = _orig_gat(arch)
    if _PREF_SET not in tabs:
        return tabs
    used = {mybir.ActivationFunctionType.Exp, mybir.ActivationFunctionType.Ln,
            mybir.ActivationFunctionType.Identity, mybir.ActivationFunctionType.Copy,
            mybir.ActivationFunctionType.Square}
    out = {}
    for name, fns in tabs.items():
        if name == _PREF_SET:
            out[name] = fns
        else:
            out[name] = {f for f in fns if f not in used}
    return out
bacc.get_activation_tables = _gat_pref

F32 = mybir.dt.float32
F32R = mybir.dt.float32r
BF16 = mybir.dt.bfloat16
AL = mybir.AluOpType
AF = mybir.ActivationFunctionType
BF = ml_dtypes.bfloat16

B, N, DIM = 2, 2048, 1024
NUM_HEADS = 16
EFF = NUM_HEADS // 2
HD = DIM // NUM_HEADS          # 64
HPC = 4                        # raw heads per core
PPC = 2                        # head pairs per core
DLOC = HPC * HD                # 256 local feature dims
LAMBDA_INIT = 0.8
EPS = 1e-5
SCALE = HD ** -0.5

NT = N // 128                  # 16 token tiles of 128
NC4 = N // 512                 # 4 chunks of 512 tokens
KB = N // 128                  # 16 k-blocks
NG = KB // 2                   # 8 k-block pair groups

_CACHE = {}


def _build_nc():
    nc = bacc.Bacc()

    x_ = nc.declare_dram_parameter("x", [N, DIM], BF16, isOutput=False)
    # weights shipped pre-tiled: [128, 8*DLOC] where slice i*DLOC:(i+1)*DLOC
    # is the [128, DLOC] tile for input-feature block i (host reshapes).
    wqT = nc.declare_dram_parameter("wqT", [128, 8 * DLOC], BF16, isOutput=False)
    wkT = nc.declare_dram_parameter("wkT", [128, 8 * DLOC], BF16, isOutput=False)
    wvT = nc.declare_dram_parameter("wvT", [128, 8 * DLOC], BF16, isOutput=False)
    woT = nc.declare_dram_parameter("woT", [128, 2 * DIM], BF16, isOutput=False)
    # bpack cols: 0-1 bq (scale-folded) fc halves, 2-3 bk fc halves
    bpack_ = nc.declare_dram_parameter("bpack", [128, 4], F32, isOutput=False)
    bv_ = nc.declare_dram_parameter("bv", [DLOC], F32, isOutput=False)
    # lrow: [lq1 | lk1 | lq2 | lk2]
    lrow_ = nc.declare_dram_parameter("lrow", [4 * HD], F32, isOutput=False)
    out_ = nc.declare_dram_parameter("out", [N, DIM], F32, isOutput=True)

    with tile.TileContext(nc) as tc:
        with tc.tile_pool(name="persist", bufs=1) as pp:
            # ---- constants ----
            identf = pp.tile([128, 128], F32, tag="identf")
            make_identity(nc, identf[:])
            ones_bf = pp.tile([128, 1], BF16, tag="ones")
            nc.vector.memset(ones_bf[:], 1.0)
            ones_f = pp.tile([128, 1], F32, tag="onesf")
            nc.vector.memset(ones_f[:], 1.0)

            # ---- DMA emission order is the DMA service order: feed the
            # first k/v projections ASAP (wk, xT chunk 0, wv), then smalls,
            # then the rest of x interleaved with remaining weights. ----
            xT = [pp.tile([128, N], BF16, tag=f"xT{i}", name=f"xT{i}") for i in range(8)]

            def emit_xt_chunk(tc4):
                for i in range(8):
                    nc.sync.dma_start_transpose(
                        xT[i][:, tc4 * 512:(tc4 + 1) * 512],
                        x_[tc4 * 512:(tc4 + 1) * 512, i * 128:(i + 1) * 128],
                    )

            wk_all = pp.tile([128, 8 * DLOC], BF16, tag="wk")
            nc.sync.dma_start(wk_all[:], wkT[:])
            bpack_t = pp.tile([128, 4], F32, tag="bpack")
            nc.sync.dma_start(bpack_t[:], bpack_[:])
            bq_t = [bpack_t[:, fc:fc + 1] for fc in range(2)]
            bk_t = [bpack_t[:, 2 + fc:3 + fc] for fc in range(2)]
            emit_xt_chunk(0)
            emit_xt_chunk(1)
            wv_all = pp.tile([128, 8 * DLOC], BF16, tag="wv")
            nc.sync.dma_start(wv_all[:], wvT[:])

            # ---- small DMAs (v bias, lambda) ----
            bv_row = pp.tile([1, DLOC], F32, tag="bvrow")
            nc.sync.dma_start(bv_row[:], bv_[:].rearrange("(one f) -> one f", one=1))
            bv_bc = pp.tile([128, DLOC], F32, tag="bvbc")
            nc.gpsimd.partition_broadcast(bv_bc[:], bv_row[:])

            lrow = pp.tile([1, 4 * HD], F32, tag="lrow")
            nc.sync.dma_start(lrow[:], lrow_[:].rearrange("(one f) -> one f", one=1))
            lprod = pp.tile([1, 2 * HD], F32, tag="lprod")
            nc.vector.tensor_mul(lprod[:, 0:HD], lrow[:, 0:HD], lrow[:, HD:2 * HD])
            nc.vector.tensor_mul(lprod[:, HD:2 * HD], lrow[:, 2 * HD:3 * HD], lrow[:, 3 * HD:4 * HD])
            lsum = pp.tile([1, 2], F32, tag="lsum")
            nc.vector.tensor_reduce(lsum[:, 0:1], lprod[:, 0:HD], mybir.AxisListType.X, AL.add)
            nc.vector.tensor_reduce(lsum[:, 1:2], lprod[:, HD:2 * HD], mybir.AxisListType.X, AL.add)
            lexp = pp.tile([1, 2], F32, tag="lexp")
            nc.scalar.activation(lexp[:], lsum[:], AF.Exp)
            lam_t = pp.tile([1, 1], F32, tag="lam")
            nc.vector.tensor_sub(lam_t[:], lexp[:, 0:1], lexp[:, 1:2])
            nc.vector.tensor_scalar_add(lam_t[:], lam_t[:], LAMBDA_INIT)
            lam_col = pp.tile([128, 1], F32, tag="lamc")
            nc.gpsimd.partition_broadcast(lam_col[:], lam_t[:])

            # ---- remaining bulk loads ----
            emit_xt_chunk(2)
            wq_all = pp.tile([128, 8 * DLOC], BF16, tag="wq")
            nc.sync.dma_start(wq_all[:], wqT[:])
            emit_xt_chunk(3)
            wo_all = pp.tile([128, 2 * DIM], BF16, tag="wo")
            nc.sync.dma_start(wo_all[:], woT[:])

            def wk_i(i):
                return wk_all[:, i * DLOC:(i + 1) * DLOC]

            def wv_i(i):
                return wv_all[:, i * DLOC:(i + 1) * DLOC]

            def wq_i(i):
                return wq_all[:, i * DLOC:(i + 1) * DLOC]

            def wo_p(p):
                return wo_all[:, p * DIM:(p + 1) * DIM]

            # ---- persistent activations ----
            qT_sb = [pp.tile([128, N], BF16, tag=f"qT{fc}", name=f"qT{fc}") for fc in range(2)]
            kT_sb = [pp.tile([128, N], BF16, tag=f"kT{fc}", name=f"kT{fc}") for fc in range(2)]
            v_sb = [pp.tile([128, DLOC], BF16, tag=f"v{tt}", name=f"v{tt}") for tt in range(NT)]
            oTn = [pp.tile([128, N], BF16, tag=f"oTn{p}", name=f"oTn{p}") for p in range(PPC)]

            # ================= PHASE A: k/v projections + qT chunk 0 ========
            with tc.tile_pool(name="pja", bufs=3, space="PSUM") as pj_ps:
                def emit_kproj(fc, tc4):
                    tsl = slice(tc4 * 512, (tc4 + 1) * 512)
                    ps = pj_ps.tile([128, 512], F32, tag="pj", name="kps")
                    for i in range(8):
                        nc.tensor.matmul(ps[:], wk_i(i)[:, fc * 128:(fc + 1) * 128],
                                         xT[i][:, tsl], start=(i == 0), stop=(i == 7))
                    nc.vector.tensor_scalar(kT_sb[fc][:, tsl], ps[:], 1.0, bk_t[fc],
                                            AL.mult, AL.add)

                def emit_vproj(tc4, tt, pool=None):
                    t = tc4 * 4 + tt
                    pool = pool if pool is not None else pj_ps
                    ps = pool.tile([128, DLOC], F32,
                                   tag="pj" if pool is pj_ps else "t", name="vps")
                    for i in range(8):
                        nc.tensor.matmul(ps[:], xT[i][:, t * 128:(t + 1) * 128],
                                         wv_i(i), start=(i == 0), stop=(i == 7))
                    nc.vector.scalar_tensor_tensor(v_sb[t][:], ps[:], 1.0, bv_bc[:],
                                                   AL.mult, AL.add)

                def emit_qproj(fc, tc4, pool):
                    tsl = slice(tc4 * 512, (tc4 + 1) * 512)
                    ps = pool.tile([128, 512], F32, tag="t", name="qps")
                    for i in range(8):
                        nc.tensor.matmul(ps[:], wq_i(i)[:, fc * 128:(fc + 1) * 128],
                                         xT[i][:, tsl], start=(i == 0), stop=(i == 7))
                    nc.vector.tensor_scalar(qT_sb[fc][:, tsl], ps[:], 1.0, bq_t[fc],
                                            AL.mult, AL.add)

                emit_kproj(0, 0)
                emit_kproj(1, 0)
                emit_kproj(0, 1)
                emit_kproj(1, 1)
                for tt in range(4):
                    emit_vproj(0, tt)
                emit_kproj(0, 2)
                emit_kproj(1, 2)
                for tt in range(4):
                    emit_vproj(1, tt)
                emit_kproj(0, 3)
                emit_kproj(1, 3)
                for tt in range(4):
                    emit_vproj(2, tt)
                for fc in range(2):
                    emit_qproj(fc, 0, pj_ps)

            # ========== PHASE B: attention + norm + output projection ======
            with (
                tc.tile_pool(name="s_ps", bufs=2, space="PSUM") as s_ps,
                tc.tile_pool(name="a_ps", bufs=2, space="PSUM") as a_ps,
                tc.tile_pool(name="d_ps", bufs=1, space="PSUM") as d_ps,
                tc.tile_pool(name="t_ps", bufs=1, space="PSUM") as t_ps,
                tc.tile_pool(name="e_sb", bufs=6) as e_pool,
                tc.tile_pool(name="cmb", bufs=8) as cmb_pool,
                tc.tile_pool(name="sqp", bufs=2) as sq_pool,
                tc.tile_pool(name="row", bufs=14) as row_pool,
                tc.tile_pool(name="bc", bufs=6) as bc_pool,
                tc.tile_pool(name="o_sb", bufs=4) as o_sb,
            ):
                # one persistent PSUM bank for all denominator accumulators;
                # (qc,p) parity picks column block 0:8 or 8:16.
                dacc_all = d_ps.tile([128, 16], F32, tag="dacc", name="dacc_all")

                def emit_scores(qc, p, g, A0, A1, dcols):
                    """Scores + exp for k-blocks 2g, 2g+1; returns pending
                    attn work (consumed one group later to keep PE fed)."""
                    qsl = slice(qc * 512, (qc + 1) * 512)
                    kb0 = 2 * g
                    s0 = s_ps.tile([128, 1024], F32, tag="s", name="s0")
                    s1 = s_ps.tile([128, 1024], F32, tag="s", name="s1")
                    for j in range(2):
                        ksl = slice((kb0 + j) * 128, (kb0 + j + 1) * 128)
                        nc.tensor.matmul(s0[:, j * 512:(j + 1) * 512],
                                         kT_sb[p][0:64, ksl],
                                         qT_sb[p][0:64, qsl],
                                         start=True, stop=True, skip_group_check=True)
                        nc.tensor.matmul(s1[:, j * 512:(j + 1) * 512],
                                         kT_sb[p][64:128, ksl],
                                         qT_sb[p][64:128, qsl],
                                         start=True, stop=True, skip_group_check=True)
                    e0 = e_pool.tile([128, 1024], BF16, tag="e", name="e0")
                    nc.scalar.activation(e0[:], s0[:], AF.Exp)
                    e1 = e_pool.tile([128, 1024], BF16, tag="e", name="e1")
                    nc.scalar.activation(e1[:], s1[:], AF.Exp)
                    return (p, g, e0, e1, A0, A1, dcols)

                def emit_attn(w):
                    """v-contraction + tiny denominator matmuls for a group."""
                    p, g, e0, e1, A0, A1, dcols = w
                    for j in range(2):
                        kb = 2 * g + j
                        st, sp = (kb == 0), (kb == KB - 1)
                        vt = v_sb[kb][:, p * 128:(p + 1) * 128]
                        jsl = slice(j * 512, (j + 1) * 512)
                        nc.tensor.matmul(A0[:], vt, e0[:, jsl], start=st, stop=sp)
                        nc.tensor.matmul(A1[:], vt, e1[:, jsl], start=st, stop=sp)
                        for s4 in range(4):
                            esl = slice(j * 512 + s4 * 128, j * 512 + (s4 + 1) * 128)
                            # PSUM start=True resets a whole granule (wipes
                            # neighboring columns): reset only on the FIRST
                            # matmul of the group-set; the rest accumulate
                            # onto the zeroed granule.
                            st_d = st and s4 == 0
                            nc.tensor.matmul(dacc_all[:, dcols + s4:dcols + s4 + 1],
                                             e0[:, esl], ones_bf[:],
                                             start=st_d, stop=sp, skip_group_check=True)
                            nc.tensor.matmul(dacc_all[:, dcols + 4 + s4:dcols + 5 + s4],
                                             e1[:, esl], ones_bf[:],
                                             start=False, stop=sp, skip_group_check=True)

                def boundary0(qc, p, A0, A1, dcols, copy=True):
                    """Right after kb15: free A banks via SBUF copies; start
                    the c = lam*d0/d1 chain on DVE."""
                    if copy:
                        A0c = cmb_pool.tile([128, 512], F32, tag="m", name="A0c")
                        nc.vector.tensor_copy(A0c[:], A0[:])
                        A1c = cmb_pool.tile([128, 512], F32, tag="m", name="A1c")
                        nc.vector.tensor_copy(A1c[:], A1[:])
                    else:
                        A0c, A1c = A0, A1
                    cinv = row_pool.tile([128, 4], F32, tag="row", name="cinv")
                    nc.vector.reciprocal(cinv[:], dacc_all[:, dcols + 4:dcols + 8])
                    cl = row_pool.tile([128, 4], F32, tag="row", name="cl")
                    nc.vector.tensor_mul(cl[:], cinv[:], dacc_all[:, dcols:dcols + 4])
                    nc.vector.tensor_scalar(cl[:], cl[:], lam_col[:, 0:1], None,
                                            AL.mult, AL.bypass)
                    # ou is d0-scaled, so the reference's +EPS inside the
                    # RMSNorm becomes +EPS*d0^2. Capture it now -- this dacc
                    # parity slot is re-accumulated two iterations later.
                    d0c = row_pool.tile([128, 4], F32, tag="row", name="d0c")
                    nc.vector.tensor_copy(d0c[:], dacc_all[:, dcols:dcols + 4])
                    epsd = row_pool.tile([128, 4], F32, tag="row", name="epsd")
                    nc.vector.tensor_mul(epsd[:], d0c[:], d0c[:])
                    nc.vector.tensor_scalar(epsd[:], epsd[:], EPS, None,
                                            AL.mult, AL.bypass)
                    return (qc, p, A0c, A1c, cl, epsd)

                def bcast_cols(vec4, name):
                    """[128,4] per-(qsub) column vector -> [128,512] broadcast.
                    Four M=1 PE transposes assemble a [1,512] row (partition 0
                    only -- partition_broadcast from nonzero bases is broken on
                    HW), then one broadcast."""
                    vT5 = t_ps.tile([1, 512], F32, tag="t", name=f"{name}T")
                    for s4 in range(4):
                        nc.tensor.matmul(vT5[0:1, s4 * 128:(s4 + 1) * 128],
                                         vec4[:, s4:s4 + 1], identf[:],
                                         is_transpose=True, start=True, stop=True,
                                         skip_group_check=True)
                    row = row_pool.tile([1, 512], F32, tag="rowr", name=f"{name}row")
                    nc.vector.tensor_copy(row[:], vT5[:])
                    bc = bc_pool.tile([128, 512], F32, tag="bc", name=f"{name}bc")
                    nc.gpsimd.partition_broadcast(bc[:], row[:])
                    return bc

                def boundary1(st1):
                    """Diff-combine: ou = A0 - c*A1 (c broadcast per qsub)."""
                    qc, p, A0c, A1c, cl, epsd = st1
                    bcc = bcast_cols(cl, "c")
                    m1 = cmb_pool.tile([128, 512], F32, tag="m", name="m1")
                    nc.vector.tensor_mul(m1[:], A1c[:], bcc[:])
                    ou = cmb_pool.tile([128, 512], F32, tag="m", name="ou")
                    nc.vector.tensor_sub(ou[:], A0c[:], m1[:])
                    return (qc, p, ou, epsd)

                def boundary2(st2, tail=False):
                    """RMSNorm (scale-invariant: ou is d0-scaled, harmless)."""
                    qc, p, ou, epsd = st2
                    qsl = slice(qc * 512, (qc + 1) * 512)
                    sq = sq_pool.tile([128, 512], BF16, tag="sq", name="sq")
                    if tail:
                        nc.scalar.activation(sq[:], ou[:], AF.Square)
                    else:
                        nc.vector.tensor_mul(sq[:], ou[:], ou[:])
                    rms = t_ps.tile([128, 4], F32, tag="t", name="rms")
                    for s4 in range(4):
                        nc.tensor.matmul(rms[:, s4:s4 + 1],
                                         sq[:, s4 * 128:(s4 + 1) * 128], ones_bf[:],
                                         start=True, stop=True, skip_group_check=True)
                    msn = row_pool.tile([128, 4], F32, tag="row", name="msn")
                    nc.vector.scalar_tensor_tensor(msn[:], rms[:], 1.0 / (2 * HD),
                                                   epsd[:], AL.mult, AL.add)
                    lnm = row_pool.tile([128, 4], F32, tag="row", name="lnm")
                    nc.scalar.activation(lnm[:], msn[:], AF.Ln)
                    rstd = row_pool.tile([128, 4], F32, tag="row", name="rstd")
                    nc.scalar.activation(rstd[:], lnm[:], AF.Exp, scale=-0.5)
                    bcr = bcast_cols(rstd, "r")
                    nc.vector.tensor_mul(oTn[p][:, qsl], ou[:], bcr[:])

                def emit_outproj_tile(qc, tt4, pool, split_dma=False):
                    tt = qc * 4 + tt4
                    tsl = slice(tt * 128, (tt + 1) * 128)
                    ot = o_sb.tile([128, DIM], F32, tag="ot", name="ot")
                    for oc in range(2):
                        osl = slice(oc * 512, (oc + 1) * 512)
                        ps = pool.tile([128, 512], F32, tag="t" if pool is t_ps else "A",
                                       name="ops")
                        for p in range(PPC):
                            nc.tensor.matmul(ps[:], oTn[p][:, tsl],
                                             wo_p(p)[:, osl],
                                             start=(p == 0), stop=(p == PPC - 1))
                        if split_dma:
                            nc.scalar.activation(ot[:, osl], ps[:], AF.Copy)
                        else:
                            nc.vector.tensor_copy(ot[:, osl], ps[:])
                        if split_dma:
                            nc.sync.dma_start(out_[tsl, osl], ot[:, osl])
                    if not split_dma:
                        nc.sync.dma_start(out_[tsl, :], ot[:])

                pend_attn = None   # attn work lagging one group behind scores
                prev_acc = None    # (qc, p, A0, A1, dcols) awaiting boundary0
                pend1 = None       # awaiting boundary1 (diff-combine)
                pend2 = None       # awaiting boundary2 (RMSNorm)
                parity = 0
                for qc in range(NC4):
                    for p in range(PPC):
                        A0 = a_ps.tile([128, 512], F32, tag="A", name="A0")
                        A1 = a_ps.tile([128, 512], F32, tag="A", name="A1")
                        dcols = parity * 8
                        for g in range(NG):
                            w = emit_scores(qc, p, g, A0, A1, dcols)
                            if pend_attn is not None:
                                emit_attn(pend_attn)
                            pend_attn = w
                            if qc == 0 and p == 0 and g < 4:
                                emit_vproj(3, g, t_ps)
                            if g == 0 and prev_acc is not None:
                                pend1 = boundary0(*prev_acc)
                                prev_acc = None
                            if g == 2 and pend1 is not None:
                                pend2 = boundary1(pend1)
                                pend1 = None
                            if g == 4 and pend2 is not None:
                                boundary2(pend2)
                                pend2 = None
                            if g == 5 and p == 0 and qc < NC4 - 1:
                                for fc in range(2):
                                    emit_qproj(fc, qc + 1, t_ps)
                            if p == 1 and qc > 0 and g in (2, 3, 4, 5):
                                emit_outproj_tile(qc - 1, g - 2, t_ps)
                        prev_acc = (qc, p, A0, A1, dcols)
                        parity ^= 1
                # tail: finish last (qc=3, p=1). The p=0 halves of the first
                # two out-projection tiles are issued into the freed score
                # banks while the DVE c-chain runs (PE would otherwise idle),
                # then completed with the p=1 halves after the final RMSNorm.
                emit_attn(pend_attn)
                st0 = boundary0(*prev_acc, copy=False)
                sA = s_ps.tile([128, 1024], F32, tag="s", name="opA")
                sB = s_ps.tile([128, 1024], F32, tag="s", name="opB")
                for tt4 in range(2):
                    tt = (NC4 - 1) * 4 + tt4
                    tsl = slice(tt * 128, (tt + 1) * 128)
                    hold = sA if tt4 == 0 else sB
                    for oc in range(2):
                        osl = slice(oc * 512, (oc + 1) * 512)
                        nc.tensor.matmul(hold[:, oc * 512:(oc + 1) * 512],
                                         oTn[0][:, tsl], wo_p(0)[:, osl],
                                         start=True, stop=False,
                                         skip_group_check=True)
                boundary2(boundary1(st0), tail=True)
                for tt4 in range(2):
                    tt = (NC4 - 1) * 4 + tt4
                    tsl = slice(tt * 128, (tt + 1) * 128)
                    hold = sA if tt4 == 0 else sB
                    ot = o_sb.tile([128, DIM], F32, tag="ot", name="ot")
                    for oc in range(2):
                        osl = slice(oc * 512, (oc + 1) * 512)
                        nc.tensor.matmul(hold[:, oc * 512:(oc + 1) * 512],
                                         oTn[1][:, tsl], wo_p(1)[:, osl],
                                         start=False, stop=True,
                                         skip_group_check=True)
                        nc.scalar.activation(ot[:, osl],
                                             hold[:, oc * 512:(oc + 1) * 512],
                                             AF.Copy)
                        nc.sync.dma_start(out_[tsl, osl], ot[:, osl])
                for tt4 in (2, 3):
                    emit_outproj_tile(NC4 - 1, tt4, a_ps if tt4 == 2 else t_ps,
                                      split_dma=True)

    nc.finalize()
    return nc


def kernel(x, Wq, bq, Wk, bk, Wv, bv, Wo, bo, norm_w, lq1, lk1, lq2, lk2):
    x = np.asarray(x, dtype=np.float32)
    Wq = np.asarray(Wq, dtype=np.float32)
    Wk = np.asarray(Wk, dtype=np.float32)
    Wv = np.asarray(Wv, dtype=np.float32)
    Wo = np.asarray(Wo, dtype=np.float32)
    nw = np.asarray(norm_w, np.float32)

    if "nc" not in _CACHE:
        _CACHE["nc"] = _build_nc()
    nc = _CACHE["nc"]

    in_maps = []
    for c in range(8):
        b, g = c // 4, c % 4
        sl = slice(g * DLOC, (g + 1) * DLOC)
        # fold q-scale into Wq/bq; fold norm_w * (1-lambda_init) into Wo.
        # pre-tile [DIM, DLOC] -> [128, 8*DLOC] (block i at cols i*DLOC:)
        def tile8(w):  # w: [1024, 256] -> [128, 2048]
            return np.ascontiguousarray(
                w.reshape(8, 128, DLOC).transpose(1, 0, 2).reshape(128, 8 * DLOC))

        wo_scale = (nw[np.arange(DLOC) % (2 * HD)] * (1.0 - LAMBDA_INIT)).astype(np.float32)
        woT_full = (Wo[:, sl] * wo_scale[None, :]).T  # [256, 1024]
        in_maps.append({
            "x": np.ascontiguousarray(x[b]).astype(BF),
            "wqT": tile8((Wq[sl, :] * SCALE).T).astype(BF),
            "wkT": tile8(Wk[sl, :].T).astype(BF),
            "wvT": tile8(Wv[sl, :].T).astype(BF),
            "woT": np.ascontiguousarray(
                woT_full.reshape(2, 128, DIM).transpose(1, 0, 2).reshape(128, 2 * DIM)).astype(BF),
            "bpack": np.ascontiguousarray(np.stack([
                np.asarray(bq, np.float32)[sl][0:128] * SCALE,
                np.asarray(bq, np.float32)[sl][128:256] * SCALE,
                np.asarray(bk, np.float32)[sl][0:128],
                np.asarray(bk, np.float32)[sl][128:256],
            ], axis=1)),
            "bv": np.ascontiguousarray(np.asarray(bv, np.float32)[sl]),
            "lrow": np.ascontiguousarray(np.concatenate([
                np.asarray(lq1, np.float32), np.asarray(lk1, np.float32),
                np.asarray(lq2, np.float32), np.asarray(lk2, np.float32)])),
        })

    res = None
    for attempt in range(3):
        try:
            res = run_bass_kernel_spmd(nc, in_maps, list(range(8))).results
            break
        except Exception:
            if attempt == 2:
                raise
            import time as _time

            _time.sleep(5 + 10 * attempt)
            try:
                import jax

                jax.clear_caches()
                jax.extend.backend.clear_backends()
            except Exception:
                pass
    bo_f = np.asarray(bo, np.float32)
    out = np.empty((B, N, DIM), np.float32)
    for b in range(B):
        acc = res[4 * b]["out"].astype(np.float32)
        for g in range(1, 4):
            acc = acc + res[4 * b + g]["out"]
        out[b] = acc + bo_f[None, :]
    return out


# revision 47
# speedup vs baseline: 1.0222x; 1.0195x over previous
"""DiffAttention TRN2 kernel v2: 8-core SPMD (batch x head-group sharding).

Sharding: core c -> batch b = c//4, head-group g = c%4 (4 raw heads = 2
effective pairs per core). Each core computes q/k/v projections for its
head slice, diff-attention, headwise RMSNorm, and a partial output
projection (its 256 input-feature slice of Wo). Host sums the 4 partials
per batch and adds bo.

v2 design notes (cost model: matmul time = out-free-size * cycles/row,
independent of K and M):
  - x shipped bf16 from host; xT produced by DMA XBAR transpose (no PE
    transposes, no PSUM->SBUF copies for xT).
  - all weights bf16 host-side; q-scale folded into Wq/bq; norm_w and
    (1-lambda_init) folded into Wo columns host-side.
  - softmax denominators via tiny matmuls d[128q,1] = e[128k,128q]^T @
    ones (1 cycle each) instead of M=1 ones-matmuls (512 cycles each).
    PSUM pitfall: a matmul with start=True resets a whole accumulation
    granule (wipes neighboring columns mid-accumulation) -- only the
    FIRST matmul of the interleaved group-set carries start=True.
  - diff-combine uses RMSNorm scale-invariance: ou = A0 - c*A1 with
    c = lam*d0/d1 (one per-q broadcast for the diff + one for rstd);
    the reference's +EPS becomes +EPS*d0^2 since ou is d0-scaled.
  - per-q row vectors are assembled via four M=1 PE transposes into a
    [1,512] PSUM row (partition_broadcast from nonzero partition bases
    is broken on HW), then one partition_broadcast.
  - exps widened to 2 PSUM banks ([128,1024]) to amortize Act access.
  - software pipelining: attention matmuls lag their scores/exp by one
    k-group; v chunk 3 / q-proj chunks 1-3 / the output projection are
    deferred into the attention loop to fill PE stall slots; boundary
    (combine + RMSNorm) work for iteration i runs inside iteration i+1.
  - PSUM budget (8 banks): scores 2x[128,1024]=4, A0/A1=2, dacc=1 (both
    parities share one bank), deferred-work/transposes=1.
"""

import sys

sys.path.insert(0, "/opt/trn_rl_repo")

import numpy as np
import ml_dtypes

import concourse.bass as bass  # noqa: F401
import concourse.mybir as mybir
import concourse.tile as tile
from concourse import bacc
from concourse.bass_utils import run_bass_kernel_spmd
from concourse.masks import make_identity

# Steer every activation to the one table set containing both Exp and Ln
# (no ACT table thrash). See v1 notes: hide used funcs from other sets so
# the chooser can't pick them.
_orig_gat = bacc.get_activation_tables
_PREF_SET = "natural_log_exp_and_others"
def _gat_pref(arch):
    tabs = _orig_gat(arch)
    if _PREF_SET not in tabs:
        return tabs
    used = {mybir.ActivationFunctionType.Exp, mybir.ActivationFunctionType.Ln,
            mybir.ActivationFunctionType.Identity, mybir.ActivationFunctionType.Copy,
            mybir.ActivationFunctionType.Square}
    out = {}
    for name, fns in tabs.items():
        if name == _PREF_SET:
            out[name] = fns
        else:
            out[name] = {f for f in fns if f not in used}
    return out
bacc.get_activation_tables = _gat_pref

F32 = mybir.dt.float32
F32R = mybir.dt.float32r
BF16 = mybir.dt.bfloat16
AL = mybir.AluOpType
AF = mybir.ActivationFunctionType
BF = ml_dtypes.bfloat16

B, N, DIM = 2, 2048, 1024
NUM_HEADS = 16
EFF = NUM_HEADS // 2
HD = DIM // NUM_HEADS          # 64
HPC = 4                        # raw heads per core
PPC = 2                        # head pairs per core
DLOC = HPC * HD                # 256 local feature dims
LAMBDA_INIT = 0.8
EPS = 1e-5
SCALE = HD ** -0.5

NT = N // 128                  # 16 token tiles of 128
NC4 = N // 512                 # 4 chunks of 512 tokens
KB = N // 128                  # 16 k-blocks
NG = KB // 2                   # 8 k-block pair groups

_CACHE = {}


def _build_nc():
    nc = bacc.Bacc()

    x_ = nc.declare_dram_parameter("x", [N, DIM], BF16, isOutput=False)
    # weights shipped pre-tiled: [128, 8*DLOC] where slice i*DLOC:(i+1)*DLOC
    # is the [128, DLOC] tile for input-feature block i (host reshapes).
    wqT = nc.declare_dram_parameter("wqT", [128, 8 * DLOC], BF16, isOutput=False)
    wkT = nc.declare_dram_parameter("wkT", [128, 8 * DLOC], BF16, isOutput=False)
    wvT = nc.declare_dram_parameter("wvT", [128, 8 * DLOC], BF16, isOutput=False)
    woT = nc.declare_dram_parameter("woT", [128, 2 * DIM], BF16, isOutput=False)
    # bpack cols: 0-1 bq (scale-folded) fc halves, 2-3 bk fc halves
    bpack_ = nc.declare_dram_parameter("bpack", [128, 4], F32, isOutput=False)
    bv_ = nc.declare_dram_parameter("bv", [DLOC], F32, isOutput=False)
    # lrow: [lq1 | lk1 | lq2 | lk2]
    lrow_ = nc.declare_dram_parameter("lrow", [4 * HD], F32, isOutput=False)
    out_ = nc.declare_dram_parameter("out", [N, DIM], F32, isOutput=True)

    with tile.TileContext(nc) as tc:
        with tc.tile_pool(name="persist", bufs=1) as pp:
            # ---- constants ----
            identf = pp.tile([128, 128], F32, tag="identf")
            make_identity(nc, identf[:])
            ones_bf = pp.tile([128, 1], BF16, tag="ones")
            nc.vector.memset(ones_bf[:], 1.0)
            ones_f = pp.tile([128, 1], F32, tag="onesf")
            nc.vector.memset(ones_f[:], 1.0)

            # ---- DMA emission order is the DMA service order: feed the
            # first k/v projections ASAP (wk, xT chunk 0, wv), then smalls,
            # then the rest of x interleaved with remaining weights. ----
            xT = [pp.tile([128, N], BF16, tag=f"xT{i}", name=f"xT{i}") for i in range(8)]

            def emit_xt_chunk(tc4):
                for i in range(8):
                    nc.sync.dma_start_transpose(
                        xT[i][:, tc4 * 512:(tc4 + 1) * 512],
                        x_[tc4 * 512:(tc4 + 1) * 512, i * 128:(i + 1) * 128],
                    )

            wk_all = pp.tile([128, 8 * DLOC], BF16, tag="wk")
            nc.sync.dma_start(wk_all[:], wkT[:])
            bpack_t = pp.tile([128, 4], F32, tag="bpack")
            nc.sync.dma_start(bpack_t[:], bpack_[:])
            bq_t = [bpack_t[:, fc:fc + 1] for fc in range(2)]
            bk_t = [bpack_t[:, 2 + fc:3 + fc] for fc in range(2)]
            emit_xt_chunk(0)
            emit_xt_chunk(1)
            wv_all = pp.tile([128, 8 * DLOC], BF16, tag="wv")
            nc.sync.dma_start(wv_all[:], wvT[:])

            # ---- small DMAs (v bias, lambda) ----
            bv_row = pp.tile([1, DLOC], F32, tag="bvrow")
            nc.sync.dma_start(bv_row[:], bv_[:].rearrange("(one f) -> one f", one=1))
            bv_bc = pp.tile([128, DLOC], F32, tag="bvbc")
            nc.gpsimd.partition_broadcast(bv_bc[:], bv_row[:])

            lrow = pp.tile([1, 4 * HD], F32, tag="lrow")
            nc.sync.dma_start(lrow[:], lrow_[:].rearrange("(one f) -> one f", one=1))
            lprod = pp.tile([1, 2 * HD], F32, tag="lprod")
            nc.vector.tensor_mul(lprod[:, 0:HD], lrow[:, 0:HD], lrow[:, HD:2 * HD])
            nc.vector.tensor_mul(lprod[:, HD:2 * HD], lrow[:, 2 * HD:3 * HD], lrow[:, 3 * HD:4 * HD])
            lsum = pp.tile([1, 2], F32, tag="lsum")
            nc.vector.tensor_reduce(lsum[:, 0:1], lprod[:, 0:HD], mybir.AxisListType.X, AL.add)
            nc.vector.tensor_reduce(lsum[:, 1:2], lprod[:, HD:2 * HD], mybir.AxisListType.X, AL.add)
            lexp = pp.tile([1, 2], F32, tag="lexp")
            nc.scalar.activation(lexp[:], lsum[:], AF.Exp)
            lam_t = pp.tile([1, 1], F32, tag="lam")
            nc.vector.tensor_sub(lam_t[:], lexp[:, 0:1], lexp[:, 1:2])
            nc.vector.tensor_scalar_add(lam_t[:], lam_t[:], LAMBDA_INIT)
            lam_col = pp.tile([128, 1], F32, tag="lamc")
            nc.gpsimd.partition_broadcast(lam_col[:], lam_t[:])

            # ---- remaining bulk loads: chunks 2+3 merged per column tile
            # (halves the per-instruction HWDGE overhead, delivers chunk 3
            # earlier) ----
            for i in range(8):
                nc.sync.dma_start_transpose(
                    xT[i][:, 1024:2048], x_[1024:2048, i * 128:(i + 1) * 128])
            wq_all = pp.tile([128, 8 * DLOC], BF16, tag="wq")
            nc.sync.dma_start(wq_all[:], wqT[:])
            wo_all = pp.tile([128, 2 * DIM], BF16, tag="wo")
            nc.sync.dma_start(wo_all[:], woT[:])

            def wk_i(i):
                return wk_all[:, i * DLOC:(i + 1) * DLOC]

            def wv_i(i):
                return wv_all[:, i * DLOC:(i + 1) * DLOC]

            def wq_i(i):
                return wq_all[:, i * DLOC:(i + 1) * DLOC]

            def wo_p(p):
                return wo_all[:, p * DIM:(p + 1) * DIM]

            # ---- persistent activations ----
            qT_sb = [pp.tile([128, N], BF16, tag=f"qT{fc}", name=f"qT{fc}") for fc in range(2)]
            kT_sb = [pp.tile([128, N], BF16, tag=f"kT{fc}", name=f"kT{fc}") for fc in range(2)]
            v_sb = [pp.tile([128, DLOC], BF16, tag=f"v{tt}", name=f"v{tt}") for tt in range(NT)]
            oTn = [pp.tile([128, N], BF16, tag=f"oTn{p}", name=f"oTn{p}") for p in range(PPC)]

            # ================= PHASE A: k/v projections + qT chunk 0 ========
            with tc.tile_pool(name="pja", bufs=3, space="PSUM") as pj_ps:
                def emit_kproj(fc, tc4):
                    tsl = slice(tc4 * 512, (tc4 + 1) * 512)
                    ps = pj_ps.tile([128, 512], F32, tag="pj", name="kps")
                    for i in range(8):
                        nc.tensor.matmul(ps[:], wk_i(i)[:, fc * 128:(fc + 1) * 128],
                                         xT[i][:, tsl], start=(i == 0), stop=(i == 7))
                    nc.vector.tensor_scalar(kT_sb[fc][:, tsl], ps[:], 1.0, bk_t[fc],
                                            AL.mult, AL.add)

                def emit_vproj(tc4, tt, pool=None):
                    t = tc4 * 4 + tt
                    pool = pool if pool is not None else pj_ps
                    ps = pool.tile([128, DLOC], F32,
                                   tag="pj" if pool is pj_ps else "t", name="vps")
                    for i in range(8):
                        nc.tensor.matmul(ps[:], xT[i][:, t * 128:(t + 1) * 128],
                                         wv_i(i), start=(i == 0), stop=(i == 7))
                    nc.vector.scalar_tensor_tensor(v_sb[t][:], ps[:], 1.0, bv_bc[:],
                                                   AL.mult, AL.add)

                def emit_qproj(fc, tc4, pool):
                    tsl = slice(tc4 * 512, (tc4 + 1) * 512)
                    ps = pool.tile([128, 512], F32, tag="t", name="qps")
                    for i in range(8):
                        nc.tensor.matmul(ps[:], wq_i(i)[:, fc * 128:(fc + 1) * 128],
                                         xT[i][:, tsl], start=(i == 0), stop=(i == 7))
                    nc.vector.tensor_scalar(qT_sb[fc][:, tsl], ps[:], 1.0, bq_t[fc],
                                            AL.mult, AL.add)

                emit_kproj(0, 0)
                emit_kproj(1, 0)
                emit_kproj(0, 1)
                emit_kproj(1, 1)
                for tt in range(4):
                    emit_vproj(0, tt)
                emit_kproj(0, 2)
                emit_kproj(1, 2)
                for tt in range(4):
                    emit_vproj(1, tt)
                emit_kproj(0, 3)
                emit_kproj(1, 3)
                for tt in range(4):
                    emit_vproj(2, tt)
                for fc in range(2):
                    emit_qproj(fc, 0, pj_ps)

            # ========== PHASE B: attention + norm + output projection ======
            with (
                tc.tile_pool(name="s_ps", bufs=2, space="PSUM") as s_ps,
                tc.tile_pool(name="a_ps", bufs=2, space="PSUM") as a_ps,
                tc.tile_pool(name="d_ps", bufs=1, space="PSUM") as d_ps,
                tc.tile_pool(name="t_ps", bufs=1, space="PSUM") as t_ps,
                tc.tile_pool(name="e_sb", bufs=6) as e_pool,
                tc.tile_pool(name="cmb", bufs=8) as cmb_pool,
                tc.tile_pool(name="sqp", bufs=2) as sq_pool,
                tc.tile_pool(name="row", bufs=14) as row_pool,
                tc.tile_pool(name="bc", bufs=6) as bc_pool,
                tc.tile_pool(name="o_sb", bufs=4) as o_sb,
            ):
                # one persistent PSUM bank for all denominator accumulators;
                # (qc,p) parity picks column block 0:8 or 8:16.
                dacc_all = d_ps.tile([128, 16], F32, tag="dacc", name="dacc_all")

                def emit_scores(qc, p, g, A0, A1, dcols):
                    """Scores + exp for k-blocks 2g, 2g+1; returns pending
                    attn work (consumed one group later to keep PE fed)."""
                    qsl = slice(qc * 512, (qc + 1) * 512)
                    kb0 = 2 * g
                    s0 = s_ps.tile([128, 1024], F32, tag="s", name="s0")
                    s1 = s_ps.tile([128, 1024], F32, tag="s", name="s1")
                    for j in range(2):
                        ksl = slice((kb0 + j) * 128, (kb0 + j + 1) * 128)
                        nc.tensor.matmul(s0[:, j * 512:(j + 1) * 512],
                                         kT_sb[p][0:64, ksl],
                                         qT_sb[p][0:64, qsl],
                                         start=True, stop=True, skip_group_check=True)
                        nc.tensor.matmul(s1[:, j * 512:(j + 1) * 512],
                                         kT_sb[p][64:128, ksl],
                                         qT_sb[p][64:128, qsl],
                                         start=True, stop=True, skip_group_check=True)
                    e0 = e_pool.tile([128, 1024], BF16, tag="e", name="e0")
                    nc.scalar.activation(e0[:], s0[:], AF.Exp)
                    e1 = e_pool.tile([128, 1024], BF16, tag="e", name="e1")
                    nc.scalar.activation(e1[:], s1[:], AF.Exp)
                    return (p, g, e0, e1, A0, A1, dcols)

                def emit_attn(w):
                    """v-contraction + tiny denominator matmuls for a group."""
                    p, g, e0, e1, A0, A1, dcols = w
                    for j in range(2):
                        kb = 2 * g + j
                        st, sp = (kb == 0), (kb == KB - 1)
                        vt = v_sb[kb][:, p * 128:(p + 1) * 128]
                        jsl = slice(j * 512, (j + 1) * 512)
                        nc.tensor.matmul(A0[:], vt, e0[:, jsl], start=st, stop=sp)
                        nc.tensor.matmul(A1[:], vt, e1[:, jsl], start=st, stop=sp)
                        for s4 in range(4):
                            esl = slice(j * 512 + s4 * 128, j * 512 + (s4 + 1) * 128)
                            # PSUM start=True resets a whole granule (wipes
                            # neighboring columns): reset only on the FIRST
                            # matmul of the group-set; the rest accumulate
                            # onto the zeroed granule.
                            st_d = st and s4 == 0
                            nc.tensor.matmul(dacc_all[:, dcols + s4:dcols + s4 + 1],
                                             e0[:, esl], ones_bf[:],
                                             start=st_d, stop=sp, skip_group_check=True)
                            nc.tensor.matmul(dacc_all[:, dcols + 4 + s4:dcols + 5 + s4],
                                             e1[:, esl], ones_bf[:],
                                             start=False, stop=sp, skip_group_check=True)

                def boundary0(qc, p, A0, A1, dcols, copy=True):
                    """Right after kb15: free A banks via SBUF copies; start
                    the c = lam*d0/d1 chain on DVE."""
                    if copy:
                        A0c = cmb_pool.tile([128, 512], F32, tag="m", name="A0c")
                        nc.vector.tensor_copy(A0c[:], A0[:])
                        A1c = cmb_pool.tile([128, 512], F32, tag="m", name="A1c")
                        nc.vector.tensor_copy(A1c[:], A1[:])
                    else:
                        A0c, A1c = A0, A1
                    cinv = row_pool.tile([128, 4], F32, tag="row", name="cinv")
                    nc.vector.reciprocal(cinv[:], dacc_all[:, dcols + 4:dcols + 8])
                    cl = row_pool.tile([128, 4], F32, tag="row", name="cl")
                    nc.vector.tensor_mul(cl[:], cinv[:], dacc_all[:, dcols:dcols + 4])
                    nc.vector.tensor_scalar(cl[:], cl[:], lam_col[:, 0:1], None,
                                            AL.mult, AL.bypass)
                    # ou is d0-scaled, so the reference's +EPS inside the
                    # RMSNorm becomes +EPS*d0^2. Capture it now -- this dacc
                    # parity slot is re-accumulated two iterations later.
                    d0c = row_pool.tile([128, 4], F32, tag="row", name="d0c")
                    nc.vector.tensor_copy(d0c[:], dacc_all[:, dcols:dcols + 4])
                    epsd = row_pool.tile([128, 4], F32, tag="row", name="epsd")
                    nc.vector.tensor_mul(epsd[:], d0c[:], d0c[:])
                    nc.vector.tensor_scalar(epsd[:], epsd[:], EPS, None,
                                            AL.mult, AL.bypass)
                    return (qc, p, A0c, A1c, cl, epsd)

                def bcast_cols(vec4, name):
                    """[128,4] per-(qsub) column vector -> [128,512] broadcast.
                    Four M=1 PE transposes assemble a [1,512] row (partition 0
                    only -- partition_broadcast from nonzero bases is broken on
                    HW), then one broadcast."""
                    vT5 = t_ps.tile([1, 512], F32, tag="t", name=f"{name}T")
                    for s4 in range(4):
                        nc.tensor.matmul(vT5[0:1, s4 * 128:(s4 + 1) * 128],
                                         vec4[:, s4:s4 + 1], identf[:],
                                         is_transpose=True, start=True, stop=True,
                                         skip_group_check=True)
                    row = row_pool.tile([1, 512], F32, tag="rowr", name=f"{name}row")
                    nc.vector.tensor_copy(row[:], vT5[:])
                    bc = bc_pool.tile([128, 512], F32, tag="bc", name=f"{name}bc")
                    nc.gpsimd.partition_broadcast(bc[:], row[:])
                    return bc

                def boundary1(st1):
                    """Diff-combine: ou = A0 - c*A1 (c broadcast per qsub)."""
                    qc, p, A0c, A1c, cl, epsd = st1
                    bcc = bcast_cols(cl, "c")
                    m1 = cmb_pool.tile([128, 512], F32, tag="m", name="m1")
                    nc.vector.tensor_mul(m1[:], A1c[:], bcc[:])
                    ou = cmb_pool.tile([128, 512], F32, tag="m", name="ou")
                    nc.vector.tensor_sub(ou[:], A0c[:], m1[:])
                    return (qc, p, ou, epsd)

                def boundary2(st2, tail=False):
                    """RMSNorm (scale-invariant: ou is d0-scaled, harmless)."""
                    qc, p, ou, epsd = st2
                    qsl = slice(qc * 512, (qc + 1) * 512)
                    sq = sq_pool.tile([128, 512], BF16, tag="sq", name="sq")
                    if tail:
                        nc.scalar.activation(sq[:], ou[:], AF.Square)
                    else:
                        nc.vector.tensor_mul(sq[:], ou[:], ou[:])
                    rms = t_ps.tile([128, 4], F32, tag="t", name="rms")
                    for s4 in range(4):
                        nc.tensor.matmul(rms[:, s4:s4 + 1],
                                         sq[:, s4 * 128:(s4 + 1) * 128], ones_bf[:],
                                         start=True, stop=True, skip_group_check=True)
                    msn = row_pool.tile([128, 4], F32, tag="row", name="msn")
                    nc.vector.scalar_tensor_tensor(msn[:], rms[:], 1.0 / (2 * HD),
                                                   epsd[:], AL.mult, AL.add)
                    lnm = row_pool.tile([128, 4], F32, tag="row", name="lnm")
                    nc.scalar.activation(lnm[:], msn[:], AF.Ln)
                    rstd = row_pool.tile([128, 4], F32, tag="row", name="rstd")
                    nc.scalar.activation(rstd[:], lnm[:], AF.Exp, scale=-0.5)
                    bcr = bcast_cols(rstd, "r")
                    nc.vector.tensor_mul(oTn[p][:, qsl], ou[:], bcr[:])

                def emit_outproj_tile(qc, tt4, pool, split_dma=False):
                    tt = qc * 4 + tt4
                    tsl = slice(tt * 128, (tt + 1) * 128)
                    ot = o_sb.tile([128, DIM], F32, tag="ot", name="ot")
                    for oc in range(2):
                        osl = slice(oc * 512, (oc + 1) * 512)
                        ps = pool.tile([128, 512], F32, tag="t" if pool is t_ps else "A",
                                       name="ops")
                        for p in range(PPC):
                            nc.tensor.matmul(ps[:], oTn[p][:, tsl],
                                             wo_p(p)[:, osl],
                                             start=(p == 0), stop=(p == PPC - 1))
                        if split_dma:
                            nc.scalar.activation(ot[:, osl], ps[:], AF.Copy)
                        else:
                            nc.vector.tensor_copy(ot[:, osl], ps[:])
                        if split_dma:
                            nc.sync.dma_start(out_[tsl, osl], ot[:, osl])
                    if not split_dma:
                        nc.sync.dma_start(out_[tsl, :], ot[:])

                pend_attn = None   # attn work lagging one group behind scores
                prev_acc = None    # (qc, p, A0, A1, dcols) awaiting boundary0
                pend1 = None       # awaiting boundary1 (diff-combine)
                pend2 = None       # awaiting boundary2 (RMSNorm)
                parity = 0
                for qc in range(NC4):
                    for p in range(PPC):
                        A0 = a_ps.tile([128, 512], F32, tag="A", name="A0")
                        A1 = a_ps.tile([128, 512], F32, tag="A", name="A1")
                        dcols = parity * 8
                        for g in range(NG):
                            w = emit_scores(qc, p, g, A0, A1, dcols)
                            if pend_attn is not None:
                                emit_attn(pend_attn)
                            pend_attn = w
                            if qc == 0 and p == 0 and g < 4:
                                emit_vproj(3, g, t_ps)
                            if g == 0 and prev_acc is not None:
                                pend1 = boundary0(*prev_acc)
                                prev_acc = None
                            if g == 2 and pend1 is not None:
                                pend2 = boundary1(pend1)
                                pend1 = None
                            if g == 4 and pend2 is not None:
                                boundary2(pend2)
                                pend2 = None
                            if g == 5 and p == 0 and qc < NC4 - 1:
                                for fc in range(2):
                                    emit_qproj(fc, qc + 1, t_ps)
                            if p == 1 and qc > 0 and g in (2, 3, 4, 5):
                                emit_outproj_tile(qc - 1, g - 2, t_ps)
                        prev_acc = (qc, p, A0, A1, dcols)
                        parity ^= 1
                # tail: finish last (qc=3, p=1). The p=0 halves of the first
                # two out-projection tiles are issued into the freed score
                # banks while the DVE c-chain runs (PE would otherwise idle),
                # then completed with the p=1 halves after the final RMSNorm.
                emit_attn(pend_attn)
                st0 = boundary0(*prev_acc, copy=False)
                sA = s_ps.tile([128, 1024], F32, tag="s", name="opA")
                sB = s_ps.tile([128, 1024], F32, tag="s", name="opB")
                for tt4 in range(2):
                    tt = (NC4 - 1) * 4 + tt4
                    tsl = slice(tt * 128, (tt + 1) * 128)
                    hold = sA if tt4 == 0 else sB
                    for oc in range(2):
                        osl = slice(oc * 512, (oc + 1) * 512)
                        nc.tensor.matmul(hold[:, oc * 512:(oc + 1) * 512],
                                         oTn[0][:, tsl], wo_p(0)[:, osl],
                                         start=True, stop=False,
                                         skip_group_check=True)
                boundary2(boundary1(st0), tail=True)
                for tt4 in range(2):
                    tt = (NC4 - 1) * 4 + tt4
                    tsl = slice(tt * 128, (tt + 1) * 128)
                    hold = sA if tt4 == 0 else sB
                    ot = o_sb.tile([128, DIM], F32, tag="ot", name="ot")
                    for oc in range(2):
                        osl = slice(oc * 512, (oc + 1) * 512)
                        nc.tensor.matmul(hold[:, oc * 512:(oc + 1) * 512],
                                         oTn[1][:, tsl], wo_p(1)[:, osl],
                                         start=False, stop=True,
                                         skip_group_check=True)
                        nc.scalar.activation(ot[:, osl],
                                             hold[:, oc * 512:(oc + 1) * 512],
                                             AF.Copy)
                        nc.sync.dma_start(out_[tsl, osl], ot[:, osl])
                for tt4 in (2, 3):
                    emit_outproj_tile(NC4 - 1, tt4, a_ps if tt4 == 2 else t_ps,
                                      split_dma=True)

    nc.finalize()
    return nc


def kernel(x, Wq, bq, Wk, bk, Wv, bv, Wo, bo, norm_w, lq1, lk1, lq2, lk2):
    x = np.asarray(x, dtype=np.float32)
    Wq = np.asarray(Wq, dtype=np.float32)
    Wk = np.asarray(Wk, dtype=np.float32)
    Wv = np.asarray(Wv, dtype=np.float32)
    Wo = np.asarray(Wo, dtype=np.float32)
    nw = np.asarray(norm_w, np.float32)

    if "nc" not in _CACHE:
        _CACHE["nc"] = _build_nc()
    nc = _CACHE["nc"]

    in_maps = []
    for c in range(8):
        b, g = c // 4, c % 4
        sl = slice(g * DLOC, (g + 1) * DLOC)
        # fold q-scale into Wq/bq; fold norm_w * (1-lambda_init) into Wo.
        # pre-tile [DIM, DLOC] -> [128, 8*DLOC] (block i at cols i*DLOC:)
        def tile8(w):  # w: [1024, 256] -> [128, 2048]
            return np.ascontiguousarray(
                w.reshape(8, 128, DLOC).transpose(1, 0, 2).reshape(128, 8 * DLOC))

        wo_scale = (nw[np.arange(DLOC) % (2 * HD)] * (1.0 - LAMBDA_INIT)).astype(np.float32)
        woT_full = (Wo[:, sl] * wo_scale[None, :]).T  # [256, 1024]
        in_maps.append({
            "x": np.ascontiguousarray(x[b]).astype(BF),
            "wqT": tile8((Wq[sl, :] * SCALE).T).astype(BF),
            "wkT": tile8(Wk[sl, :].T).astype(BF),
            "wvT": tile8(Wv[sl, :].T).astype(BF),
            "woT": np.ascontiguousarray(
                woT_full.reshape(2, 128, DIM).transpose(1, 0, 2).reshape(128, 2 * DIM)).astype(BF),
            "bpack": np.ascontiguousarray(np.stack([
                np.asarray(bq, np.float32)[sl][0:128] * SCALE,
                np.asarray(bq, np.float32)[sl][128:256] * SCALE,
                np.asarray(bk, np.float32)[sl][0:128],
                np.asarray(bk, np.float32)[sl][128:256],
            ], axis=1)),
            "bv": np.ascontiguousarray(np.asarray(bv, np.float32)[sl]),
            "lrow": np.ascontiguousarray(np.concatenate([
                np.asarray(lq1, np.float32), np.asarray(lk1, np.float32),
                np.asarray(lq2, np.float32), np.asarray(lk2, np.float32)])),
        })

    res = None
    for attempt in range(3):
        try:
            res = run_bass_kernel_spmd(nc, in_maps, list(range(8))).results
            break
        except Exception:
            if attempt == 2:
                raise
            import time as _time

            _time.sleep(5 + 10 * attempt)
            try:
                import jax

                jax.clear_caches()
                jax.extend.backend.clear_backends()
            except Exception:
                pass
    bo_f = np.asarray(bo, np.float32)
    out = np.empty((B, N, DIM), np.float32)
    for b in range(B):
        acc = res[4 * b]["out"].astype(np.float32)
        for g in range(1, 4):
            acc = acc + res[4 * b + g]["out"]
        out[b] = acc + bo_f[None, :]
    return out


# revision 49
# speedup vs baseline: 1.0283x; 1.0059x over previous
"""DiffAttention TRN2 kernel v2: 8-core SPMD (batch x head-group sharding).

Sharding: core c -> batch b = c//4, head-group g = c%4 (4 raw heads = 2
effective pairs per core). Each core computes q/k/v projections for its
head slice, diff-attention, headwise RMSNorm, and a partial output
projection (its 256 input-feature slice of Wo). Host sums the 4 partials
per batch and adds bo.

v2 design notes (cost model: matmul time = out-free-size * cycles/row,
independent of K and M):
  - x shipped bf16 from host; xT produced by DMA XBAR transpose (no PE
    transposes, no PSUM->SBUF copies for xT).
  - all weights bf16 host-side; q-scale folded into Wq/bq; norm_w and
    (1-lambda_init) folded into Wo columns host-side.
  - softmax denominators via tiny matmuls d[128q,1] = e[128k,128q]^T @
    ones (1 cycle each) instead of M=1 ones-matmuls (512 cycles each).
    PSUM pitfall: a matmul with start=True resets a whole accumulation
    granule (wipes neighboring columns mid-accumulation) -- only the
    FIRST matmul of the interleaved group-set carries start=True.
  - diff-combine uses RMSNorm scale-invariance: ou = A0 - c*A1 with
    c = lam*d0/d1 (one per-q broadcast for the diff + one for rstd);
    the reference's +EPS becomes +EPS*d0^2 since ou is d0-scaled.
  - per-q row vectors are assembled via four M=1 PE transposes into a
    [1,512] PSUM row (partition_broadcast from nonzero partition bases
    is broken on HW), then one partition_broadcast.
  - exps widened to 2 PSUM banks ([128,1024]) to amortize Act access.
  - software pipelining: attention matmuls lag their scores/exp by one
    k-group; v chunk 3 / q-proj chunks 1-3 / the output projection are
    deferred into the attention loop to fill PE stall slots; boundary
    (combine + RMSNorm) work for iteration i runs inside iteration i+1.
  - PSUM budget (8 banks): scores 2x[128,1024]=4, A0/A1=2, dacc=1 (both
    parities share one bank), deferred-work/transposes=1.
"""

import sys

sys.path.insert(0, "/opt/trn_rl_repo")

import numpy as np
import ml_dtypes

import concourse.bass as bass  # noqa: F401
import concourse.mybir as mybir
import concourse.tile as tile
from concourse import bacc
from concourse.bass_utils import run_bass_kernel_spmd
from concourse.masks import make_identity

# Steer every activation to the one table set containing both Exp and Ln
# (no ACT table thrash). See v1 notes: hide used funcs from other sets so
# the chooser can't pick them.
_orig_gat = bacc.get_activation_tables
_PREF_SET = "natural_log_exp_and_others"
def _gat_pref(arch):
    tabs = _orig_gat(arch)
    if _PREF_SET not in tabs:
        return tabs
    used = {mybir.ActivationFunctionType.Exp, mybir.ActivationFunctionType.Ln,
            mybir.ActivationFunctionType.Identity, mybir.ActivationFunctionType.Copy,
            mybir.ActivationFunctionType.Square}
    out = {}
    for name, fns in tabs.items():
        if name == _PREF_SET:
            out[name] = fns
        else:
            out[name] = {f for f in fns if f not in used}
    return out
bacc.get_activation_tables = _gat_pref

F32 = mybir.dt.float32
F32R = mybir.dt.float32r
BF16 = mybir.dt.bfloat16
AL = mybir.AluOpType
AF = mybir.ActivationFunctionType
BF = ml_dtypes.bfloat16

B, N, DIM = 2, 2048, 1024
NUM_HEADS = 16
EFF = NUM_HEADS // 2
HD = DIM // NUM_HEADS          # 64
HPC = 4                        # raw heads per core
PPC = 2                        # head pairs per core
DLOC = HPC * HD                # 256 local feature dims
LAMBDA_INIT = 0.8
EPS = 1e-5
SCALE = HD ** -0.5

NT = N // 128                  # 16 token tiles of 128
NC4 = N // 512                 # 4 chunks of 512 tokens
KB = N // 128                  # 16 k-blocks
NG = KB // 2                   # 8 k-block pair groups

_CACHE = {}


def _build_nc():
    nc = bacc.Bacc()

    x_ = nc.declare_dram_parameter("x", [N, DIM], BF16, isOutput=False)
    # weights shipped pre-tiled: [128, 8*DLOC] where slice i*DLOC:(i+1)*DLOC
    # is the [128, DLOC] tile for input-feature block i (host reshapes).
    wqT = nc.declare_dram_parameter("wqT", [128, 8 * DLOC], BF16, isOutput=False)
    wkT = nc.declare_dram_parameter("wkT", [128, 8 * DLOC], BF16, isOutput=False)
    wvT = nc.declare_dram_parameter("wvT", [128, 8 * DLOC], BF16, isOutput=False)
    woT = nc.declare_dram_parameter("woT", [128, 2 * DIM], BF16, isOutput=False)
    # bpack cols: 0-1 bq (scale-folded) fc halves, 2-3 bk fc halves
    bpack_ = nc.declare_dram_parameter("bpack", [128, 4], F32, isOutput=False)
    bv_ = nc.declare_dram_parameter("bv", [DLOC], F32, isOutput=False)
    # lrow: [lq1 | lk1 | lq2 | lk2]
    lrow_ = nc.declare_dram_parameter("lrow", [4 * HD], F32, isOutput=False)
    out_ = nc.declare_dram_parameter("out", [N, DIM], F32, isOutput=True)

    with tile.TileContext(nc) as tc:
        with tc.tile_pool(name="persist", bufs=1) as pp:
            # ---- constants ----
            identf = pp.tile([128, 128], F32, tag="identf")
            make_identity(nc, identf[:])
            identb = pp.tile([128, 128], BF16, tag="identb")
            make_identity(nc, identb[:])
            ones_bf = pp.tile([128, 1], BF16, tag="ones")
            nc.vector.memset(ones_bf[:], 1.0)
            ones_f = pp.tile([128, 1], F32, tag="onesf")
            nc.vector.memset(ones_f[:], 1.0)

            # ---- DMA emission order is the DMA service order: feed the
            # first k/v projections ASAP (wk, xT chunk 0, wv), then smalls,
            # then the rest of x interleaved with remaining weights. ----
            xT = [pp.tile([128, N], BF16, tag=f"xT{i}", name=f"xT{i}") for i in range(8)]

            def emit_xt_chunk(tc4):
                for i in range(8):
                    nc.sync.dma_start_transpose(
                        xT[i][:, tc4 * 512:(tc4 + 1) * 512],
                        x_[tc4 * 512:(tc4 + 1) * 512, i * 128:(i + 1) * 128],
                    )

            wk_all = pp.tile([128, 8 * DLOC], BF16, tag="wk")
            nc.sync.dma_start(wk_all[:], wkT[:])
            bpack_t = pp.tile([128, 4], F32, tag="bpack")
            nc.sync.dma_start(bpack_t[:], bpack_[:])
            bq_t = [bpack_t[:, fc:fc + 1] for fc in range(2)]
            bk_t = [bpack_t[:, 2 + fc:3 + fc] for fc in range(2)]
            emit_xt_chunk(0)
            emit_xt_chunk(1)
            wv_all = pp.tile([128, 8 * DLOC], BF16, tag="wv")
            nc.sync.dma_start(wv_all[:], wvT[:])

            # ---- small DMAs (v bias, lambda) ----
            bv_row = pp.tile([1, DLOC], F32, tag="bvrow")
            nc.sync.dma_start(bv_row[:], bv_[:].rearrange("(one f) -> one f", one=1))
            bv_bc = pp.tile([128, DLOC], F32, tag="bvbc")
            nc.gpsimd.partition_broadcast(bv_bc[:], bv_row[:])

            lrow = pp.tile([1, 4 * HD], F32, tag="lrow")
            nc.sync.dma_start(lrow[:], lrow_[:].rearrange("(one f) -> one f", one=1))
            lprod = pp.tile([1, 2 * HD], F32, tag="lprod")
            nc.vector.tensor_mul(lprod[:, 0:HD], lrow[:, 0:HD], lrow[:, HD:2 * HD])
            nc.vector.tensor_mul(lprod[:, HD:2 * HD], lrow[:, 2 * HD:3 * HD], lrow[:, 3 * HD:4 * HD])
            lsum = pp.tile([1, 2], F32, tag="lsum")
            nc.vector.tensor_reduce(lsum[:, 0:1], lprod[:, 0:HD], mybir.AxisListType.X, AL.add)
            nc.vector.tensor_reduce(lsum[:, 1:2], lprod[:, HD:2 * HD], mybir.AxisListType.X, AL.add)
            lexp = pp.tile([1, 2], F32, tag="lexp")
            nc.scalar.activation(lexp[:], lsum[:], AF.Exp)
            lam_t = pp.tile([1, 1], F32, tag="lam")
            nc.vector.tensor_sub(lam_t[:], lexp[:, 0:1], lexp[:, 1:2])
            nc.vector.tensor_scalar_add(lam_t[:], lam_t[:], LAMBDA_INIT)
            lam_col = pp.tile([128, 1], F32, tag="lamc")
            nc.gpsimd.partition_broadcast(lam_col[:], lam_t[:])

            # ---- remaining bulk loads: chunks 2+3 merged per column tile
            # (halves the per-instruction HWDGE overhead, delivers chunk 3
            # earlier) ----
            for i in range(8):
                nc.sync.dma_start_transpose(
                    xT[i][:, 1024:2048], x_[1024:2048, i * 128:(i + 1) * 128])
            wq_all = pp.tile([128, 8 * DLOC], BF16, tag="wq")
            nc.sync.dma_start(wq_all[:], wqT[:])
            wo_all = pp.tile([128, 2 * DIM], BF16, tag="wo")
            nc.sync.dma_start(wo_all[:], woT[:])

            def wk_i(i):
                return wk_all[:, i * DLOC:(i + 1) * DLOC]

            def wv_i(i):
                return wv_all[:, i * DLOC:(i + 1) * DLOC]

            def wq_i(i):
                return wq_all[:, i * DLOC:(i + 1) * DLOC]

            def wo_p(p):
                return wo_all[:, p * DIM:(p + 1) * DIM]

            # ---- persistent activations ----
            qT_sb = [pp.tile([128, N], BF16, tag=f"qT{fc}", name=f"qT{fc}") for fc in range(2)]
            kT_sb = [pp.tile([128, N], BF16, tag=f"kT{fc}", name=f"kT{fc}") for fc in range(2)]
            v_sb = [pp.tile([128, DLOC], BF16, tag=f"v{tt}", name=f"v{tt}") for tt in range(NT)]
            oTn = [pp.tile([128, N], BF16, tag=f"oTn{p}", name=f"oTn{p}") for p in range(PPC)]

            # ================= PHASE A: k/v projections + qT chunk 0 ========
            with tc.tile_pool(name="pja", bufs=3, space="PSUM") as pj_ps:
                def emit_kproj(fc, tc4):
                    tsl = slice(tc4 * 512, (tc4 + 1) * 512)
                    ps = pj_ps.tile([128, 512], F32, tag="pj", name="kps")
                    for i in range(8):
                        nc.tensor.matmul(ps[:], wk_i(i)[:, fc * 128:(fc + 1) * 128],
                                         xT[i][:, tsl], start=(i == 0), stop=(i == 7))
                    nc.vector.tensor_scalar(kT_sb[fc][:, tsl], ps[:], 1.0, bk_t[fc],
                                            AL.mult, AL.add)

                def emit_vproj(tc4, tt, pool=None):
                    t = tc4 * 4 + tt
                    pool = pool if pool is not None else pj_ps
                    ps = pool.tile([128, DLOC], F32,
                                   tag="pj" if pool is pj_ps else "t", name="vps")
                    for i in range(8):
                        nc.tensor.matmul(ps[:], xT[i][:, t * 128:(t + 1) * 128],
                                         wv_i(i), start=(i == 0), stop=(i == 7))
                    nc.vector.scalar_tensor_tensor(v_sb[t][:], ps[:], 1.0, bv_bc[:],
                                                   AL.mult, AL.add)

                def emit_qproj(fc, tc4, pool):
                    tsl = slice(tc4 * 512, (tc4 + 1) * 512)
                    ps = pool.tile([128, 512], F32, tag="t", name="qps")
                    for i in range(8):
                        nc.tensor.matmul(ps[:], wq_i(i)[:, fc * 128:(fc + 1) * 128],
                                         xT[i][:, tsl], start=(i == 0), stop=(i == 7))
                    nc.vector.tensor_scalar(qT_sb[fc][:, tsl], ps[:], 1.0, bq_t[fc],
                                            AL.mult, AL.add)

                emit_kproj(0, 0)
                emit_kproj(1, 0)
                emit_kproj(0, 1)
                emit_kproj(1, 1)
                for tt in range(4):
                    emit_vproj(0, tt)
                emit_kproj(0, 2)
                emit_kproj(1, 2)
                for tt in range(4):
                    emit_vproj(1, tt)
                emit_kproj(0, 3)
                emit_kproj(1, 3)
                for tt in range(4):
                    emit_vproj(2, tt)
                for fc in range(2):
                    emit_qproj(fc, 0, pj_ps)

            # ========== PHASE B: attention + norm + output projection ======
            with (
                tc.tile_pool(name="s_ps", bufs=2, space="PSUM") as s_ps,
                tc.tile_pool(name="a_ps", bufs=2, space="PSUM") as a_ps,
                tc.tile_pool(name="d_ps", bufs=1, space="PSUM") as d_ps,
                tc.tile_pool(name="t_ps", bufs=1, space="PSUM") as t_ps,
                tc.tile_pool(name="e_sb", bufs=6) as e_pool,
                tc.tile_pool(name="cmb", bufs=8) as cmb_pool,
                tc.tile_pool(name="sqp", bufs=2) as sq_pool,
                tc.tile_pool(name="row", bufs=14) as row_pool,
                tc.tile_pool(name="bc", bufs=6) as bc_pool,
                tc.tile_pool(name="o_sb", bufs=4) as o_sb,
            ):
                # one persistent PSUM bank for all denominator accumulators;
                # (qc,p) parity picks column block 0:8 or 8:16.
                dacc_all = d_ps.tile([128, 16], F32, tag="dacc", name="dacc_all")

                def emit_scores(qc, p, g, A0, A1, dcols):
                    """Scores + exp for k-blocks 2g, 2g+1; returns pending
                    attn work (consumed one group later to keep PE fed)."""
                    qsl = slice(qc * 512, (qc + 1) * 512)
                    kb0 = 2 * g
                    s0 = s_ps.tile([128, 1024], F32, tag="s", name="s0")
                    s1 = s_ps.tile([128, 1024], F32, tag="s", name="s1")
                    for j in range(2):
                        ksl = slice((kb0 + j) * 128, (kb0 + j + 1) * 128)
                        nc.tensor.matmul(s0[:, j * 512:(j + 1) * 512],
                                         kT_sb[p][0:64, ksl],
                                         qT_sb[p][0:64, qsl],
                                         start=True, stop=True, skip_group_check=True)
                        nc.tensor.matmul(s1[:, j * 512:(j + 1) * 512],
                                         kT_sb[p][64:128, ksl],
                                         qT_sb[p][64:128, qsl],
                                         start=True, stop=True, skip_group_check=True)
                    e0 = e_pool.tile([128, 1024], BF16, tag="e", name="e0")
                    nc.scalar.activation(e0[:], s0[:], AF.Exp)
                    e1 = e_pool.tile([128, 1024], BF16, tag="e", name="e1")
                    nc.scalar.activation(e1[:], s1[:], AF.Exp)
                    return (p, g, e0, e1, A0, A1, dcols)

                def emit_attn(w):
                    """v-contraction + tiny denominator matmuls for a group."""
                    p, g, e0, e1, A0, A1, dcols = w
                    for j in range(2):
                        kb = 2 * g + j
                        st, sp = (kb == 0), (kb == KB - 1)
                        vt = v_sb[kb][:, p * 128:(p + 1) * 128]
                        jsl = slice(j * 512, (j + 1) * 512)
                        nc.tensor.matmul(A0[:], vt, e0[:, jsl], start=st, stop=sp)
                        nc.tensor.matmul(A1[:], vt, e1[:, jsl], start=st, stop=sp)
                        for s4 in range(4):
                            esl = slice(j * 512 + s4 * 128, j * 512 + (s4 + 1) * 128)
                            # PSUM start=True resets a whole granule (wipes
                            # neighboring columns): reset only on the FIRST
                            # matmul of the group-set; the rest accumulate
                            # onto the zeroed granule.
                            st_d = st and s4 == 0
                            nc.tensor.matmul(dacc_all[:, dcols + s4:dcols + s4 + 1],
                                             e0[:, esl], ones_bf[:],
                                             start=st_d, stop=sp, skip_group_check=True)
                            nc.tensor.matmul(dacc_all[:, dcols + 4 + s4:dcols + 5 + s4],
                                             e1[:, esl], ones_bf[:],
                                             start=False, stop=sp, skip_group_check=True)

                def boundary0(qc, p, A0, A1, dcols, copy=True):
                    """Right after kb15: free A banks via SBUF copies; start
                    the c = lam*d0/d1 chain on DVE."""
                    if copy:
                        A0c = cmb_pool.tile([128, 512], F32, tag="m", name="A0c")
                        nc.vector.tensor_copy(A0c[:], A0[:])
                        A1c = cmb_pool.tile([128, 512], F32, tag="m", name="A1c")
                        nc.vector.tensor_copy(A1c[:], A1[:])
                    else:
                        A0c, A1c = A0, A1
                    cinv = row_pool.tile([128, 4], F32, tag="row", name="cinv")
                    nc.vector.reciprocal(cinv[:], dacc_all[:, dcols + 4:dcols + 8])
                    cl = row_pool.tile([128, 4], F32, tag="row", name="cl")
                    nc.vector.tensor_mul(cl[:], cinv[:], dacc_all[:, dcols:dcols + 4])
                    nc.vector.tensor_scalar(cl[:], cl[:], lam_col[:, 0:1], None,
                                            AL.mult, AL.bypass)
                    # ou is d0-scaled, so the reference's +EPS inside the
                    # RMSNorm becomes +EPS*d0^2. Capture it now -- this dacc
                    # parity slot is re-accumulated two iterations later.
                    d0c = row_pool.tile([128, 4], F32, tag="row", name="d0c")
                    nc.vector.tensor_copy(d0c[:], dacc_all[:, dcols:dcols + 4])
                    epsd = row_pool.tile([128, 4], F32, tag="row", name="epsd")
                    nc.vector.tensor_mul(epsd[:], d0c[:], d0c[:])
                    nc.vector.tensor_scalar(epsd[:], epsd[:], EPS, None,
                                            AL.mult, AL.bypass)
                    return (qc, p, A0c, A1c, cl, epsd)

                def bcast_cols(vec4, name, dt=F32, ident=None):
                    """[128,4] per-(qsub) column vector -> [128,512] broadcast.
                    Four M=1 PE transposes assemble a [1,512] row (partition 0
                    only -- partition_broadcast from nonzero bases is broken on
                    HW), then one broadcast."""
                    ident = identf if ident is None else ident
                    vT5 = t_ps.tile([1, 512], dt, tag="t", name=f"{name}T")
                    for s4 in range(4):
                        nc.tensor.matmul(vT5[0:1, s4 * 128:(s4 + 1) * 128],
                                         vec4[:, s4:s4 + 1], ident[:],
                                         is_transpose=True, start=True, stop=True,
                                         skip_group_check=True)
                    row = row_pool.tile([1, 512], dt, tag="rowr", name=f"{name}row")
                    nc.vector.tensor_copy(row[:], vT5[:])
                    bc = bc_pool.tile([128, 512], dt, tag="bc", name=f"{name}bc")
                    nc.gpsimd.partition_broadcast(bc[:], row[:])
                    return bc

                def boundary1(st1):
                    """Diff-combine: ou = A0 - c*A1 (c broadcast per qsub)."""
                    qc, p, A0c, A1c, cl, epsd = st1
                    bcc = bcast_cols(cl, "c")
                    m1 = cmb_pool.tile([128, 512], F32, tag="m", name="m1")
                    nc.vector.tensor_mul(m1[:], A1c[:], bcc[:])
                    ou = cmb_pool.tile([128, 512], F32, tag="m", name="ou")
                    nc.vector.tensor_sub(ou[:], A0c[:], m1[:])
                    return (qc, p, ou, epsd)

                def boundary2(st2, tail=False):
                    """RMSNorm (scale-invariant: ou is d0-scaled, harmless)."""
                    qc, p, ou, epsd = st2
                    qsl = slice(qc * 512, (qc + 1) * 512)
                    sq = sq_pool.tile([128, 512], BF16, tag="sq", name="sq")
                    if tail:
                        nc.scalar.activation(sq[:], ou[:], AF.Square)
                    else:
                        nc.vector.tensor_mul(sq[:], ou[:], ou[:])
                    rms = t_ps.tile([128, 4], F32, tag="t", name="rms")
                    for s4 in range(4):
                        nc.tensor.matmul(rms[:, s4:s4 + 1],
                                         sq[:, s4 * 128:(s4 + 1) * 128], ones_bf[:],
                                         start=True, stop=True, skip_group_check=True)
                    msn = row_pool.tile([128, 4], F32, tag="row", name="msn")
                    nc.vector.scalar_tensor_tensor(msn[:], rms[:], 1.0 / (2 * HD),
                                                   epsd[:], AL.mult, AL.add)
                    lnm = row_pool.tile([128, 4], F32, tag="row", name="lnm")
                    nc.scalar.activation(lnm[:], msn[:], AF.Ln)
                    rstd = row_pool.tile([128, 4], BF16, tag="rowb", name="rstd")
                    nc.scalar.activation(rstd[:], lnm[:], AF.Exp, scale=-0.5)
                    bcr = bcast_cols(rstd, "r", BF16, identb)
                    nc.vector.tensor_mul(oTn[p][:, qsl], ou[:], bcr[:])

                def emit_outproj_tile(qc, tt4, pool, split_dma=False):
                    tt = qc * 4 + tt4
                    tsl = slice(tt * 128, (tt + 1) * 128)
                    ot = o_sb.tile([128, DIM], F32, tag="ot", name="ot")
                    for oc in range(2):
                        osl = slice(oc * 512, (oc + 1) * 512)
                        ps = pool.tile([128, 512], F32, tag="t" if pool is t_ps else "A",
                                       name="ops")
                        for p in range(PPC):
                            nc.tensor.matmul(ps[:], oTn[p][:, tsl],
                                             wo_p(p)[:, osl],
                                             start=(p == 0), stop=(p == PPC - 1))
                        if split_dma:
                            nc.scalar.activation(ot[:, osl], ps[:], AF.Copy)
                        else:
                            nc.vector.tensor_copy(ot[:, osl], ps[:])
                        if split_dma:
                            nc.sync.dma_start(out_[tsl, osl], ot[:, osl])
                    if not split_dma:
                        nc.sync.dma_start(out_[tsl, :], ot[:])

                pend_attn = None   # attn work lagging one group behind scores
                prev_acc = None    # (qc, p, A0, A1, dcols) awaiting boundary0
                pend1 = None       # awaiting boundary1 (diff-combine)
                pend2 = None       # awaiting boundary2 (RMSNorm)
                parity = 0
                for qc in range(NC4):
                    for p in range(PPC):
                        A0 = a_ps.tile([128, 512], F32, tag="A", name="A0")
                        A1 = a_ps.tile([128, 512], F32, tag="A", name="A1")
                        dcols = parity * 8
                        for g in range(NG):
                            w = emit_scores(qc, p, g, A0, A1, dcols)
                            if pend_attn is not None:
                                emit_attn(pend_attn)
                            pend_attn = w
                            if qc == 0 and p == 0 and g < 4:
                                emit_vproj(3, g, t_ps)
                            if g == 0 and prev_acc is not None:
                                pend1 = boundary0(*prev_acc)
                                prev_acc = None
                            if g == 2 and pend1 is not None:
                                pend2 = boundary1(pend1)
                                pend1 = None
                            if g == 4 and pend2 is not None:
                                boundary2(pend2)
                                pend2 = None
                            if g == 5 and p == 0 and qc < NC4 - 1:
                                for fc in range(2):
                                    emit_qproj(fc, qc + 1, t_ps)
                            if p == 1 and qc > 0 and g in (2, 3, 4, 5):
                                emit_outproj_tile(qc - 1, g - 2, t_ps)
                        prev_acc = (qc, p, A0, A1, dcols)
                        parity ^= 1
                # tail: finish last (qc=3, p=1). The p=0 halves of the first
                # two out-projection tiles are issued into the freed score
                # banks while the DVE c-chain runs (PE would otherwise idle),
                # then completed with the p=1 halves after the final RMSNorm.
                emit_attn(pend_attn)
                st0 = boundary0(*prev_acc, copy=False)
                sA = s_ps.tile([128, 1024], F32, tag="s", name="opA")
                sB = s_ps.tile([128, 1024], F32, tag="s", name="opB")
                for tt4 in range(2):
                    tt = (NC4 - 1) * 4 + tt4
                    tsl = slice(tt * 128, (tt + 1) * 128)
                    hold = sA if tt4 == 0 else sB
                    for oc in range(2):
                        osl = slice(oc * 512, (oc + 1) * 512)
                        nc.tensor.matmul(hold[:, oc * 512:(oc + 1) * 512],
                                         oTn[0][:, tsl], wo_p(0)[:, osl],
                                         start=True, stop=False,
                                         skip_group_check=True)
                st2t = boundary1(st0)
                # tile 2's p0 halves into the A banks just freed by ou
                aC = a_ps.tile([128, 512], F32, tag="A", name="opC")
                aD = a_ps.tile([128, 512], F32, tag="A", name="opD")
                tt2sl = slice(((NC4 - 1) * 4 + 2) * 128, ((NC4 - 1) * 4 + 3) * 128)
                nc.tensor.matmul(aC[:], oTn[0][:, tt2sl], wo_p(0)[:, 0:512],
                                 start=True, stop=False, skip_group_check=True)
                nc.tensor.matmul(aD[:], oTn[0][:, tt2sl], wo_p(0)[:, 512:1024],
                                 start=True, stop=False, skip_group_check=True)
                boundary2(st2t, tail=True)
                for tt4 in range(2):
                    tt = (NC4 - 1) * 4 + tt4
                    tsl = slice(tt * 128, (tt + 1) * 128)
                    hold = sA if tt4 == 0 else sB
                    ot = o_sb.tile([128, DIM], F32, tag="ot", name="ot")
                    for oc in range(2):
                        osl = slice(oc * 512, (oc + 1) * 512)
                        nc.tensor.matmul(hold[:, oc * 512:(oc + 1) * 512],
                                         oTn[1][:, tsl], wo_p(1)[:, osl],
                                         start=False, stop=True,
                                         skip_group_check=True)
                        nc.scalar.activation(ot[:, osl],
                                             hold[:, oc * 512:(oc + 1) * 512],
                                             AF.Copy)
                        nc.sync.dma_start(out_[tsl, osl], ot[:, osl])
                ot2 = o_sb.tile([128, DIM], F32, tag="ot", name="ot2")
                for oc, hold in ((0, aC), (1, aD)):
                    osl = slice(oc * 512, (oc + 1) * 512)
                    nc.tensor.matmul(hold[:], oTn[1][:, tt2sl], wo_p(1)[:, osl],
                                     start=False, stop=True, skip_group_check=True)
                    nc.scalar.activation(ot2[:, osl], hold[:], AF.Copy)
                    nc.sync.dma_start(out_[tt2sl, osl], ot2[:, osl])
                emit_outproj_tile(NC4 - 1, 3, t_ps, split_dma=True)

    nc.finalize()
    return nc


def kernel(x, Wq, bq, Wk, bk, Wv, bv, Wo, bo, norm_w, lq1, lk1, lq2, lk2):
    x = np.asarray(x, dtype=np.float32)
    Wq = np.asarray(Wq, dtype=np.float32)
    Wk = np.asarray(Wk, dtype=np.float32)
    Wv = np.asarray(Wv, dtype=np.float32)
    Wo = np.asarray(Wo, dtype=np.float32)
    nw = np.asarray(norm_w, np.float32)

    if "nc" not in _CACHE:
        _CACHE["nc"] = _build_nc()
    nc = _CACHE["nc"]

    in_maps = []
    for c in range(8):
        b, g = c // 4, c % 4
        sl = slice(g * DLOC, (g + 1) * DLOC)
        # fold q-scale into Wq/bq; fold norm_w * (1-lambda_init) into Wo.
        # pre-tile [DIM, DLOC] -> [128, 8*DLOC] (block i at cols i*DLOC:)
        def tile8(w):  # w: [1024, 256] -> [128, 2048]
            return np.ascontiguousarray(
                w.reshape(8, 128, DLOC).transpose(1, 0, 2).reshape(128, 8 * DLOC))

        wo_scale = (nw[np.arange(DLOC) % (2 * HD)] * (1.0 - LAMBDA_INIT)).astype(np.float32)
        woT_full = (Wo[:, sl] * wo_scale[None, :]).T  # [256, 1024]
        in_maps.append({
            "x": np.ascontiguousarray(x[b]).astype(BF),
            "wqT": tile8((Wq[sl, :] * SCALE).T).astype(BF),
            "wkT": tile8(Wk[sl, :].T).astype(BF),
            "wvT": tile8(Wv[sl, :].T).astype(BF),
            "woT": np.ascontiguousarray(
                woT_full.reshape(2, 128, DIM).transpose(1, 0, 2).reshape(128, 2 * DIM)).astype(BF),
            "bpack": np.ascontiguousarray(np.stack([
                np.asarray(bq, np.float32)[sl][0:128] * SCALE,
                np.asarray(bq, np.float32)[sl][128:256] * SCALE,
                np.asarray(bk, np.float32)[sl][0:128],
                np.asarray(bk, np.float32)[sl][128:256],
            ], axis=1)),
            "bv": np.ascontiguousarray(np.asarray(bv, np.float32)[sl]),
            "lrow": np.ascontiguousarray(np.concatenate([
                np.asarray(lq1, np.float32), np.asarray(lk1, np.float32),
                np.asarray(lq2, np.float32), np.asarray(lk2, np.float32)])),
        })

    res = None
    for attempt in range(3):
        try:
            res = run_bass_kernel_spmd(nc, in_maps, list(range(8))).results
            break
        except Exception:
            if attempt == 2:
                raise
            import time as _time

            _time.sleep(5 + 10 * attempt)
            try:
                import jax

                jax.clear_caches()
                jax.extend.backend.clear_backends()
            except Exception:
                pass
    bo_f = np.asarray(bo, np.float32)
    out = np.empty((B, N, DIM), np.float32)
    for b in range(B):
        acc = res[4 * b]["out"].astype(np.float32)
        for g in range(1, 4):
            acc = acc + res[4 * b + g]["out"]
        out[b] = acc + bo_f[None, :]
    return out


# revision 51
# speedup vs baseline: 1.0291x; 1.0008x over previous
"""DiffAttention TRN2 kernel v2: 8-core SPMD (batch x head-group sharding).

Sharding: core c -> batch b = c//4, head-group g = c%4 (4 raw heads = 2
effective pairs per core). Each core computes q/k/v projections for its
head slice, diff-attention, headwise RMSNorm, and a partial output
projection (its 256 input-feature slice of Wo). Host sums the 4 partials
per batch and adds bo.

v2 design notes (cost model: matmul time = out-free-size * cycles/row,
independent of K and M):
  - x shipped bf16 from host; xT produced by DMA XBAR transpose (no PE
    transposes, no PSUM->SBUF copies for xT).
  - all weights bf16 host-side; q-scale folded into Wq/bq; norm_w and
    (1-lambda_init) folded into Wo columns host-side.
  - softmax denominators via tiny matmuls d[128q,1] = e[128k,128q]^T @
    ones (1 cycle each) instead of M=1 ones-matmuls (512 cycles each).
    PSUM pitfall: a matmul with start=True resets a whole accumulation
    granule (wipes neighboring columns mid-accumulation) -- only the
    FIRST matmul of the interleaved group-set carries start=True.
  - diff-combine uses RMSNorm scale-invariance: ou = A0 - c*A1 with
    c = lam*d0/d1 (one per-q broadcast for the diff + one for rstd);
    the reference's +EPS becomes +EPS*d0^2 since ou is d0-scaled.
  - per-q row vectors are assembled via four M=1 PE transposes into a
    [1,512] PSUM row (partition_broadcast from nonzero partition bases
    is broken on HW), then one partition_broadcast.
  - exps widened to 2 PSUM banks ([128,1024]) to amortize Act access.
  - software pipelining: attention matmuls lag their scores/exp by one
    k-group; v chunk 3 / q-proj chunks 1-3 / the output projection are
    deferred into the attention loop to fill PE stall slots; boundary
    (combine + RMSNorm) work for iteration i runs inside iteration i+1.
  - PSUM budget (8 banks): scores 2x[128,1024]=4, A0/A1=2, dacc=1 (both
    parities share one bank), deferred-work/transposes=1.
"""

import sys

sys.path.insert(0, "/opt/trn_rl_repo")

import numpy as np
import ml_dtypes

import concourse.bass as bass  # noqa: F401
import concourse.mybir as mybir
import concourse.tile as tile
from concourse import bacc
from concourse.bass_utils import run_bass_kernel_spmd
from concourse.masks import make_identity

# Steer every activation to the one table set containing both Exp and Ln
# (no ACT table thrash). See v1 notes: hide used funcs from other sets so
# the chooser can't pick them.
_orig_gat = bacc.get_activation_tables
_PREF_SET = "natural_log_exp_and_others"
def _gat_pref(arch):
    tabs = _orig_gat(arch)
    if _PREF_SET not in tabs:
        return tabs
    used = {mybir.ActivationFunctionType.Exp, mybir.ActivationFunctionType.Ln,
            mybir.ActivationFunctionType.Identity, mybir.ActivationFunctionType.Copy,
            mybir.ActivationFunctionType.Square}
    out = {}
    for name, fns in tabs.items():
        if name == _PREF_SET:
            out[name] = fns
        else:
            out[name] = {f for f in fns if f not in used}
    return out
bacc.get_activation_tables = _gat_pref

F32 = mybir.dt.float32
F32R = mybir.dt.float32r
BF16 = mybir.dt.bfloat16
AL = mybir.AluOpType
AF = mybir.ActivationFunctionType
BF = ml_dtypes.bfloat16

B, N, DIM = 2, 2048, 1024
NUM_HEADS = 16
EFF = NUM_HEADS // 2
HD = DIM // NUM_HEADS          # 64
HPC = 4                        # raw heads per core
PPC = 2                        # head pairs per core
DLOC = HPC * HD                # 256 local feature dims
LAMBDA_INIT = 0.8
EPS = 1e-5
SCALE = HD ** -0.5

NT = N // 128                  # 16 token tiles of 128
NC4 = N // 512                 # 4 chunks of 512 tokens
KB = N // 128                  # 16 k-blocks
NG = KB // 2                   # 8 k-block pair groups

_CACHE = {}


def _build_nc():
    nc = bacc.Bacc()

    x_ = nc.declare_dram_parameter("x", [N, DIM], BF16, isOutput=False)
    # weights shipped pre-tiled: [128, 8*DLOC] where slice i*DLOC:(i+1)*DLOC
    # is the [128, DLOC] tile for input-feature block i (host reshapes).
    wqT = nc.declare_dram_parameter("wqT", [128, 8 * DLOC], BF16, isOutput=False)
    wkT = nc.declare_dram_parameter("wkT", [128, 8 * DLOC], BF16, isOutput=False)
    wvT = nc.declare_dram_parameter("wvT", [128, 8 * DLOC], BF16, isOutput=False)
    woT = nc.declare_dram_parameter("woT", [128, 2 * DIM], BF16, isOutput=False)
    # bpack cols: 0-1 bq (scale-folded) fc halves, 2-3 bk fc halves
    bpack_ = nc.declare_dram_parameter("bpack", [128, 4], F32, isOutput=False)
    bv_ = nc.declare_dram_parameter("bv", [DLOC], F32, isOutput=False)
    # lrow: [lq1 | lk1 | lq2 | lk2]
    lrow_ = nc.declare_dram_parameter("lrow", [4 * HD], F32, isOutput=False)
    out_ = nc.declare_dram_parameter("out", [N, DIM], F32, isOutput=True)

    with tile.TileContext(nc) as tc:
        with tc.tile_pool(name="persist", bufs=1) as pp:
            # ---- constants ----
            identf = pp.tile([128, 128], F32, tag="identf")
            make_identity(nc, identf[:])
            identb = pp.tile([128, 128], BF16, tag="identb")
            make_identity(nc, identb[:])
            identr = pp.tile([128, 128], F32R, tag="identr")
            nc.vector.tensor_copy(identr[:], identf[:])
            ones_bf = pp.tile([128, 1], BF16, tag="ones")
            nc.vector.memset(ones_bf[:], 1.0)
            ones_f = pp.tile([128, 1], F32, tag="onesf")
            nc.vector.memset(ones_f[:], 1.0)

            # ---- DMA emission order is the DMA service order: feed the
            # first k/v projections ASAP (wk, xT chunk 0, wv), then smalls,
            # then the rest of x interleaved with remaining weights. ----
            xT = [pp.tile([128, N], BF16, tag=f"xT{i}", name=f"xT{i}") for i in range(8)]

            def emit_xt_chunk(tc4):
                for i in range(8):
                    nc.sync.dma_start_transpose(
                        xT[i][:, tc4 * 512:(tc4 + 1) * 512],
                        x_[tc4 * 512:(tc4 + 1) * 512, i * 128:(i + 1) * 128],
                    )

            wk_all = pp.tile([128, 8 * DLOC], BF16, tag="wk")
            nc.sync.dma_start(wk_all[:], wkT[:])
            bpack_t = pp.tile([128, 4], F32, tag="bpack")
            nc.sync.dma_start(bpack_t[:], bpack_[:])
            bq_t = [bpack_t[:, fc:fc + 1] for fc in range(2)]
            bk_t = [bpack_t[:, 2 + fc:3 + fc] for fc in range(2)]
            emit_xt_chunk(0)
            emit_xt_chunk(1)
            wv_all = pp.tile([128, 8 * DLOC], BF16, tag="wv")
            nc.sync.dma_start(wv_all[:], wvT[:])

            # ---- small DMAs (v bias, lambda) ----
            bv_row = pp.tile([1, DLOC], F32, tag="bvrow")
            nc.sync.dma_start(bv_row[:], bv_[:].rearrange("(one f) -> one f", one=1))
            bv_bc = pp.tile([128, DLOC], F32, tag="bvbc")
            nc.gpsimd.partition_broadcast(bv_bc[:], bv_row[:])

            lrow = pp.tile([1, 4 * HD], F32, tag="lrow")
            nc.sync.dma_start(lrow[:], lrow_[:].rearrange("(one f) -> one f", one=1))
            lprod = pp.tile([1, 2 * HD], F32, tag="lprod")
            nc.vector.tensor_mul(lprod[:, 0:HD], lrow[:, 0:HD], lrow[:, HD:2 * HD])
            nc.vector.tensor_mul(lprod[:, HD:2 * HD], lrow[:, 2 * HD:3 * HD], lrow[:, 3 * HD:4 * HD])
            lsum = pp.tile([1, 2], F32, tag="lsum")
            nc.vector.tensor_reduce(lsum[:, 0:1], lprod[:, 0:HD], mybir.AxisListType.X, AL.add)
            nc.vector.tensor_reduce(lsum[:, 1:2], lprod[:, HD:2 * HD], mybir.AxisListType.X, AL.add)
            lexp = pp.tile([1, 2], F32, tag="lexp")
            nc.scalar.activation(lexp[:], lsum[:], AF.Exp)
            lam_t = pp.tile([1, 1], F32, tag="lam")
            nc.vector.tensor_sub(lam_t[:], lexp[:, 0:1], lexp[:, 1:2])
            nc.vector.tensor_scalar_add(lam_t[:], lam_t[:], LAMBDA_INIT)
            lam_col = pp.tile([128, 1], F32, tag="lamc")
            nc.gpsimd.partition_broadcast(lam_col[:], lam_t[:])

            # ---- remaining bulk loads: chunks 2+3 merged per column tile
            # (halves the per-instruction HWDGE overhead, delivers chunk 3
            # earlier) ----
            for i in range(8):
                nc.sync.dma_start_transpose(
                    xT[i][:, 1024:2048], x_[1024:2048, i * 128:(i + 1) * 128])
            wq_all = pp.tile([128, 8 * DLOC], BF16, tag="wq")
            nc.sync.dma_start(wq_all[:], wqT[:])
            wo_all = pp.tile([128, 2 * DIM], BF16, tag="wo")
            nc.sync.dma_start(wo_all[:], woT[:])

            def wk_i(i):
                return wk_all[:, i * DLOC:(i + 1) * DLOC]

            def wv_i(i):
                return wv_all[:, i * DLOC:(i + 1) * DLOC]

            def wq_i(i):
                return wq_all[:, i * DLOC:(i + 1) * DLOC]

            def wo_p(p):
                return wo_all[:, p * DIM:(p + 1) * DIM]

            # ---- persistent activations ----
            qT_sb = [pp.tile([128, N], BF16, tag=f"qT{fc}", name=f"qT{fc}") for fc in range(2)]
            kT_sb = [pp.tile([128, N], BF16, tag=f"kT{fc}", name=f"kT{fc}") for fc in range(2)]
            v_sb = [pp.tile([128, DLOC], BF16, tag=f"v{tt}", name=f"v{tt}") for tt in range(NT)]
            oTn = [pp.tile([128, N], BF16, tag=f"oTn{p}", name=f"oTn{p}") for p in range(PPC)]

            # ================= PHASE A: k/v projections + qT chunk 0 ========
            with tc.tile_pool(name="pja", bufs=3, space="PSUM") as pj_ps:
                def emit_kproj(fc, tc4):
                    tsl = slice(tc4 * 512, (tc4 + 1) * 512)
                    ps = pj_ps.tile([128, 512], F32, tag="pj", name="kps")
                    for i in range(8):
                        nc.tensor.matmul(ps[:], wk_i(i)[:, fc * 128:(fc + 1) * 128],
                                         xT[i][:, tsl], start=(i == 0), stop=(i == 7))
                    nc.vector.tensor_scalar(kT_sb[fc][:, tsl], ps[:], 1.0, bk_t[fc],
                                            AL.mult, AL.add)

                def emit_vproj(tc4, tt, pool=None):
                    t = tc4 * 4 + tt
                    pool = pool if pool is not None else pj_ps
                    ps = pool.tile([128, DLOC], F32,
                                   tag="pj" if pool is pj_ps else "t", name="vps")
                    for i in range(8):
                        nc.tensor.matmul(ps[:], xT[i][:, t * 128:(t + 1) * 128],
                                         wv_i(i), start=(i == 0), stop=(i == 7))
                    nc.vector.scalar_tensor_tensor(v_sb[t][:], ps[:], 1.0, bv_bc[:],
                                                   AL.mult, AL.add)

                def emit_qproj(fc, tc4, pool):
                    tsl = slice(tc4 * 512, (tc4 + 1) * 512)
                    ps = pool.tile([128, 512], F32, tag="t", name="qps")
                    for i in range(8):
                        nc.tensor.matmul(ps[:], wq_i(i)[:, fc * 128:(fc + 1) * 128],
                                         xT[i][:, tsl], start=(i == 0), stop=(i == 7))
                    nc.vector.tensor_scalar(qT_sb[fc][:, tsl], ps[:], 1.0, bq_t[fc],
                                            AL.mult, AL.add)

                emit_kproj(0, 0)
                emit_kproj(1, 0)
                emit_kproj(0, 1)
                emit_kproj(1, 1)
                for tt in range(4):
                    emit_vproj(0, tt)
                emit_kproj(0, 2)
                emit_kproj(1, 2)
                for tt in range(4):
                    emit_vproj(1, tt)
                emit_kproj(0, 3)
                emit_kproj(1, 3)
                for tt in range(4):
                    emit_vproj(2, tt)
                for fc in range(2):
                    emit_qproj(fc, 0, pj_ps)

            # ========== PHASE B: attention + norm + output projection ======
            with (
                tc.tile_pool(name="s_ps", bufs=2, space="PSUM") as s_ps,
                tc.tile_pool(name="a_ps", bufs=2, space="PSUM") as a_ps,
                tc.tile_pool(name="d_ps", bufs=1, space="PSUM") as d_ps,
                tc.tile_pool(name="t_ps", bufs=1, space="PSUM") as t_ps,
                tc.tile_pool(name="e_sb", bufs=6) as e_pool,
                tc.tile_pool(name="cmb", bufs=8) as cmb_pool,
                tc.tile_pool(name="sqp", bufs=2) as sq_pool,
                tc.tile_pool(name="row", bufs=14) as row_pool,
                tc.tile_pool(name="bc", bufs=6) as bc_pool,
                tc.tile_pool(name="o_sb", bufs=4) as o_sb,
            ):
                # one persistent PSUM bank for all denominator accumulators;
                # (qc,p) parity picks column block 0:8 or 8:16.
                dacc_all = d_ps.tile([128, 16], F32, tag="dacc", name="dacc_all")

                def emit_scores(qc, p, g, A0, A1, dcols):
                    """Scores + exp for k-blocks 2g, 2g+1; returns pending
                    attn work (consumed one group later to keep PE fed)."""
                    qsl = slice(qc * 512, (qc + 1) * 512)
                    kb0 = 2 * g
                    s0 = s_ps.tile([128, 1024], F32, tag="s", name="s0")
                    s1 = s_ps.tile([128, 1024], F32, tag="s", name="s1")
                    for j in range(2):
                        ksl = slice((kb0 + j) * 128, (kb0 + j + 1) * 128)
                        nc.tensor.matmul(s0[:, j * 512:(j + 1) * 512],
                                         kT_sb[p][0:64, ksl],
                                         qT_sb[p][0:64, qsl],
                                         start=True, stop=True, skip_group_check=True)
                        nc.tensor.matmul(s1[:, j * 512:(j + 1) * 512],
                                         kT_sb[p][64:128, ksl],
                                         qT_sb[p][64:128, qsl],
                                         start=True, stop=True, skip_group_check=True)
                    e0 = e_pool.tile([128, 1024], BF16, tag="e", name="e0")
                    nc.scalar.activation(e0[:], s0[:], AF.Exp)
                    e1 = e_pool.tile([128, 1024], BF16, tag="e", name="e1")
                    nc.scalar.activation(e1[:], s1[:], AF.Exp)
                    return (p, g, e0, e1, A0, A1, dcols)

                def emit_attn(w):
                    """v-contraction + tiny denominator matmuls for a group."""
                    p, g, e0, e1, A0, A1, dcols = w
                    for j in range(2):
                        kb = 2 * g + j
                        st, sp = (kb == 0), (kb == KB - 1)
                        vt = v_sb[kb][:, p * 128:(p + 1) * 128]
                        jsl = slice(j * 512, (j + 1) * 512)
                        nc.tensor.matmul(A0[:], vt, e0[:, jsl], start=st, stop=sp)
                        nc.tensor.matmul(A1[:], vt, e1[:, jsl], start=st, stop=sp)
                        for s4 in range(4):
                            esl = slice(j * 512 + s4 * 128, j * 512 + (s4 + 1) * 128)
                            # PSUM start=True resets a whole granule (wipes
                            # neighboring columns): reset only on the FIRST
                            # matmul of the group-set; the rest accumulate
                            # onto the zeroed granule.
                            st_d = st and s4 == 0
                            nc.tensor.matmul(dacc_all[:, dcols + s4:dcols + s4 + 1],
                                             e0[:, esl], ones_bf[:],
                                             start=st_d, stop=sp, skip_group_check=True)
                            nc.tensor.matmul(dacc_all[:, dcols + 4 + s4:dcols + 5 + s4],
                                             e1[:, esl], ones_bf[:],
                                             start=False, stop=sp, skip_group_check=True)

                def boundary0(qc, p, A0, A1, dcols, copy=True):
                    """Right after kb15: free A banks via SBUF copies; start
                    the c = lam*d0/d1 chain on DVE."""
                    if copy:
                        A0c = cmb_pool.tile([128, 512], F32, tag="m", name="A0c")
                        nc.vector.tensor_copy(A0c[:], A0[:])
                        A1c = cmb_pool.tile([128, 512], F32, tag="m", name="A1c")
                        nc.vector.tensor_copy(A1c[:], A1[:])
                    else:
                        A0c, A1c = A0, A1
                    cinv = row_pool.tile([128, 4], F32, tag="row", name="cinv")
                    nc.vector.reciprocal(cinv[:], dacc_all[:, dcols + 4:dcols + 8])
                    cl = row_pool.tile([128, 4], F32R, tag="rowc", name="cl")
                    nc.vector.tensor_mul(cl[:], cinv[:], dacc_all[:, dcols:dcols + 4])
                    nc.vector.tensor_scalar(cl[:], cl[:], lam_col[:, 0:1], None,
                                            AL.mult, AL.bypass)
                    # ou is d0-scaled, so the reference's +EPS inside the
                    # RMSNorm becomes +EPS*d0^2. Capture it now -- this dacc
                    # parity slot is re-accumulated two iterations later.
                    d0c = row_pool.tile([128, 4], F32, tag="row", name="d0c")
                    nc.vector.tensor_copy(d0c[:], dacc_all[:, dcols:dcols + 4])
                    epsd = row_pool.tile([128, 4], F32, tag="row", name="epsd")
                    nc.vector.tensor_mul(epsd[:], d0c[:], d0c[:])
                    nc.vector.tensor_scalar(epsd[:], epsd[:], EPS, None,
                                            AL.mult, AL.bypass)
                    return (qc, p, A0c, A1c, cl, epsd)

                def bcast_cols(vec4, name, dt=F32, ident=None):
                    """[128,4] per-(qsub) column vector -> [128,512] broadcast.
                    Four M=1 PE transposes assemble a [1,512] row (partition 0
                    only -- partition_broadcast from nonzero bases is broken on
                    HW), then one broadcast."""
                    ident = identf if ident is None else ident
                    vT5 = t_ps.tile([1, 512], dt, tag="t", name=f"{name}T")
                    for s4 in range(4):
                        nc.tensor.matmul(vT5[0:1, s4 * 128:(s4 + 1) * 128],
                                         vec4[:, s4:s4 + 1], ident[:],
                                         is_transpose=True, start=True, stop=True,
                                         skip_group_check=True)
                    row = row_pool.tile([1, 512], dt, tag="rowr", name=f"{name}row")
                    nc.vector.tensor_copy(row[:], vT5[:])
                    bc = bc_pool.tile([128, 512], dt, tag="bc", name=f"{name}bc")
                    nc.gpsimd.partition_broadcast(bc[:], row[:])
                    return bc

                def boundary1(st1):
                    """Diff-combine: ou = A0 - c*A1 (c broadcast per qsub)."""
                    qc, p, A0c, A1c, cl, epsd = st1
                    # f32r transpose (1.5 c/row vs 2 for f32); rounding error
                    # ~1.6e-4 on the diff coefficient is negligible.
                    bcc = bcast_cols(cl, "c", F32R, identr)
                    m1 = cmb_pool.tile([128, 512], F32, tag="m", name="m1")
                    nc.vector.tensor_mul(m1[:], A1c[:], bcc[:])
                    ou = cmb_pool.tile([128, 512], F32, tag="m", name="ou")
                    nc.vector.tensor_sub(ou[:], A0c[:], m1[:])
                    return (qc, p, ou, epsd)

                def boundary2(st2, tail=False):
                    """RMSNorm (scale-invariant: ou is d0-scaled, harmless)."""
                    qc, p, ou, epsd = st2
                    qsl = slice(qc * 512, (qc + 1) * 512)
                    sq = sq_pool.tile([128, 512], BF16, tag="sq", name="sq")
                    if tail:
                        nc.scalar.activation(sq[:], ou[:], AF.Square)
                    else:
                        nc.vector.tensor_mul(sq[:], ou[:], ou[:])
                    rms = t_ps.tile([128, 4], F32, tag="t", name="rms")
                    for s4 in range(4):
                        nc.tensor.matmul(rms[:, s4:s4 + 1],
                                         sq[:, s4 * 128:(s4 + 1) * 128], ones_bf[:],
                                         start=True, stop=True, skip_group_check=True)
                    msn = row_pool.tile([128, 4], F32, tag="row", name="msn")
                    nc.vector.scalar_tensor_tensor(msn[:], rms[:], 1.0 / (2 * HD),
                                                   epsd[:], AL.mult, AL.add)
                    lnm = row_pool.tile([128, 4], F32, tag="row", name="lnm")
                    nc.scalar.activation(lnm[:], msn[:], AF.Ln)
                    rstd = row_pool.tile([128, 4], BF16, tag="rowb", name="rstd")
                    nc.scalar.activation(rstd[:], lnm[:], AF.Exp, scale=-0.5)
                    bcr = bcast_cols(rstd, "r", BF16, identb)
                    nc.vector.tensor_mul(oTn[p][:, qsl], ou[:], bcr[:])

                def emit_outproj_tile(qc, tt4, pool, split_dma=False):
                    tt = qc * 4 + tt4
                    tsl = slice(tt * 128, (tt + 1) * 128)
                    ot = o_sb.tile([128, DIM], F32, tag="ot", name="ot")
                    for oc in range(2):
                        osl = slice(oc * 512, (oc + 1) * 512)
                        ps = pool.tile([128, 512], F32, tag="t" if pool is t_ps else "A",
                                       name="ops")
                        for p in range(PPC):
                            nc.tensor.matmul(ps[:], oTn[p][:, tsl],
                                             wo_p(p)[:, osl],
                                             start=(p == 0), stop=(p == PPC - 1))
                        if split_dma:
                            nc.scalar.activation(ot[:, osl], ps[:], AF.Copy)
                        else:
                            nc.vector.tensor_copy(ot[:, osl], ps[:])
                        if split_dma:
                            nc.sync.dma_start(out_[tsl, osl], ot[:, osl])
                    if not split_dma:
                        nc.sync.dma_start(out_[tsl, :], ot[:])

                pend_attn = None   # attn work lagging one group behind scores
                prev_acc = None    # (qc, p, A0, A1, dcols) awaiting boundary0
                pend1 = None       # awaiting boundary1 (diff-combine)
                pend2 = None       # awaiting boundary2 (RMSNorm)
                parity = 0
                for qc in range(NC4):
                    for p in range(PPC):
                        A0 = a_ps.tile([128, 512], F32, tag="A", name="A0")
                        A1 = a_ps.tile([128, 512], F32, tag="A", name="A1")
                        dcols = parity * 8
                        for g in range(NG):
                            w = emit_scores(qc, p, g, A0, A1, dcols)
                            if pend_attn is not None:
                                emit_attn(pend_attn)
                            pend_attn = w
                            if qc == 0 and p == 0 and g < 4:
                                emit_vproj(3, g, t_ps)
                            if g == 0 and prev_acc is not None:
                                pend1 = boundary0(*prev_acc)
                                prev_acc = None
                            if g == 2 and pend1 is not None:
                                pend2 = boundary1(pend1)
                                pend1 = None
                            if g == 4 and pend2 is not None:
                                boundary2(pend2)
                                pend2 = None
                            if g == 5 and p == 0 and qc < NC4 - 1:
                                for fc in range(2):
                                    emit_qproj(fc, qc + 1, t_ps)
                            if p == 1 and qc > 0 and g in (2, 3, 4, 5):
                                emit_outproj_tile(qc - 1, g - 2, t_ps)
                        prev_acc = (qc, p, A0, A1, dcols)
                        parity ^= 1
                # tail: finish last (qc=3, p=1). The p=0 halves of the first
                # two out-projection tiles are issued into the freed score
                # banks while the DVE c-chain runs (PE would otherwise idle),
                # then completed with the p=1 halves after the final RMSNorm.
                emit_attn(pend_attn)
                st0 = boundary0(*prev_acc, copy=False)
                sA = s_ps.tile([128, 1024], F32, tag="s", name="opA")
                sB = s_ps.tile([128, 1024], F32, tag="s", name="opB")
                for tt4 in range(2):
                    tt = (NC4 - 1) * 4 + tt4
                    tsl = slice(tt * 128, (tt + 1) * 128)
                    hold = sA if tt4 == 0 else sB
                    for oc in range(2):
                        osl = slice(oc * 512, (oc + 1) * 512)
                        nc.tensor.matmul(hold[:, oc * 512:(oc + 1) * 512],
                                         oTn[0][:, tsl], wo_p(0)[:, osl],
                                         start=True, stop=False,
                                         skip_group_check=True)
                st2t = boundary1(st0)
                # tile 2's p0 halves into the A banks just freed by ou
                aC = a_ps.tile([128, 512], F32, tag="A", name="opC")
                aD = a_ps.tile([128, 512], F32, tag="A", name="opD")
                tt2sl = slice(((NC4 - 1) * 4 + 2) * 128, ((NC4 - 1) * 4 + 3) * 128)
                nc.tensor.matmul(aC[:], oTn[0][:, tt2sl], wo_p(0)[:, 0:512],
                                 start=True, stop=False, skip_group_check=True)
                nc.tensor.matmul(aD[:], oTn[0][:, tt2sl], wo_p(0)[:, 512:1024],
                                 start=True, stop=False, skip_group_check=True)
                boundary2(st2t, tail=True)
                for tt4 in range(2):
                    tt = (NC4 - 1) * 4 + tt4
                    tsl = slice(tt * 128, (tt + 1) * 128)
                    hold = sA if tt4 == 0 else sB
                    ot = o_sb.tile([128, DIM], F32, tag="ot", name="ot")
                    for oc in range(2):
                        osl = slice(oc * 512, (oc + 1) * 512)
                        nc.tensor.matmul(hold[:, oc * 512:(oc + 1) * 512],
                                         oTn[1][:, tsl], wo_p(1)[:, osl],
                                         start=False, stop=True,
                                         skip_group_check=True)
                        nc.scalar.activation(ot[:, osl],
                                             hold[:, oc * 512:(oc + 1) * 512],
                                             AF.Copy)
                        nc.sync.dma_start(out_[tsl, osl], ot[:, osl])
                ot2 = o_sb.tile([128, DIM], F32, tag="ot", name="ot2")
                for oc, hold in ((0, aC), (1, aD)):
                    osl = slice(oc * 512, (oc + 1) * 512)
                    nc.tensor.matmul(hold[:], oTn[1][:, tt2sl], wo_p(1)[:, osl],
                                     start=False, stop=True, skip_group_check=True)
                    nc.scalar.activation(ot2[:, osl], hold[:], AF.Copy)
                    nc.sync.dma_start(out_[tt2sl, osl], ot2[:, osl])
                emit_outproj_tile(NC4 - 1, 3, t_ps, split_dma=True)

    nc.finalize()
    return nc


def kernel(x, Wq, bq, Wk, bk, Wv, bv, Wo, bo, norm_w, lq1, lk1, lq2, lk2):
    x = np.asarray(x, dtype=np.float32)
    Wq = np.asarray(Wq, dtype=np.float32)
    Wk = np.asarray(Wk, dtype=np.float32)
    Wv = np.asarray(Wv, dtype=np.float32)
    Wo = np.asarray(Wo, dtype=np.float32)
    nw = np.asarray(norm_w, np.float32)

    if "nc" not in _CACHE:
        _CACHE["nc"] = _build_nc()
    nc = _CACHE["nc"]

    in_maps = []
    for c in range(8):
        b, g = c // 4, c % 4
        sl = slice(g * DLOC, (g + 1) * DLOC)
        # fold q-scale into Wq/bq; fold norm_w * (1-lambda_init) into Wo.
        # pre-tile [DIM, DLOC] -> [128, 8*DLOC] (block i at cols i*DLOC:)
        def tile8(w):  # w: [1024, 256] -> [128, 2048]
            return np.ascontiguousarray(
                w.reshape(8, 128, DLOC).transpose(1, 0, 2).reshape(128, 8 * DLOC))

        wo_scale = (nw[np.arange(DLOC) % (2 * HD)] * (1.0 - LAMBDA_INIT)).astype(np.float32)
        woT_full = (Wo[:, sl] * wo_scale[None, :]).T  # [256, 1024]
        in_maps.append({
            "x": np.ascontiguousarray(x[b]).astype(BF),
            "wqT": tile8((Wq[sl, :] * SCALE).T).astype(BF),
            "wkT": tile8(Wk[sl, :].T).astype(BF),
            "wvT": tile8(Wv[sl, :].T).astype(BF),
            "woT": np.ascontiguousarray(
                woT_full.reshape(2, 128, DIM).transpose(1, 0, 2).reshape(128, 2 * DIM)).astype(BF),
            "bpack": np.ascontiguousarray(np.stack([
                np.asarray(bq, np.float32)[sl][0:128] * SCALE,
                np.asarray(bq, np.float32)[sl][128:256] * SCALE,
                np.asarray(bk, np.float32)[sl][0:128],
                np.asarray(bk, np.float32)[sl][128:256],
            ], axis=1)),
            "bv": np.ascontiguousarray(np.asarray(bv, np.float32)[sl]),
            "lrow": np.ascontiguousarray(np.concatenate([
                np.asarray(lq1, np.float32), np.asarray(lk1, np.float32),
                np.asarray(lq2, np.float32), np.asarray(lk2, np.float32)])),
        })

    res = None
    for attempt in range(3):
        try:
            res = run_bass_kernel_spmd(nc, in_maps, list(range(8))).results
            break
        except Exception:
            if attempt == 2:
                raise
            import time as _time

            _time.sleep(5 + 10 * attempt)
            try:
                import jax

                jax.clear_caches()
                jax.extend.backend.clear_backends()
            except Exception:
                pass
    bo_f = np.asarray(bo, np.float32)
    out = np.empty((B, N, DIM), np.float32)
    for b in range(B):
        acc = res[4 * b]["out"].astype(np.float32)
        for g in range(1, 4):
            acc = acc + res[4 * b + g]["out"]
        out[b] = acc + bo_f[None, :]
    return out


# revision 52
# speedup vs baseline: 1.0490x; 1.0193x over previous
"""DiffAttention TRN2 kernel v2: 8-core SPMD (batch x head-group sharding).

Sharding: core c -> batch b = c//4, head-group g = c%4 (4 raw heads = 2
effective pairs per core). Each core computes q/k/v projections for its
head slice, diff-attention, headwise RMSNorm, and a partial output
projection (its 256 input-feature slice of Wo). Host sums the 4 partials
per batch and adds bo.

v2 design notes (cost model: matmul time = out-free-size * cycles/row,
independent of K and M):
  - x shipped bf16 from host; xT produced by DMA XBAR transpose (no PE
    transposes, no PSUM->SBUF copies for xT).
  - all weights bf16 host-side; q-scale folded into Wq/bq; norm_w and
    (1-lambda_init) folded into Wo columns host-side.
  - softmax denominators via tiny matmuls d[128q,1] = e[128k,128q]^T @
    ones (1 cycle each) instead of M=1 ones-matmuls (512 cycles each).
    PSUM pitfall: a matmul with start=True resets a whole accumulation
    granule (wipes neighboring columns mid-accumulation) -- only the
    FIRST matmul of the interleaved group-set carries start=True.
  - diff-combine uses RMSNorm scale-invariance: ou = A0 - c*A1 with
    c = lam*d0/d1 (one per-q broadcast for the diff + one for rstd);
    the reference's +EPS becomes +EPS*d0^2 since ou is d0-scaled.
  - per-q row vectors are assembled via four M=1 PE transposes into a
    [1,512] PSUM row (partition_broadcast from nonzero partition bases
    is broken on HW), then one partition_broadcast.
  - exps widened to 2 PSUM banks ([128,1024]) to amortize Act access.
  - software pipelining: attention matmuls lag their scores/exp by one
    k-group; v chunk 3 / q-proj chunks 1-3 / the output projection are
    deferred into the attention loop to fill PE stall slots; boundary
    (combine + RMSNorm) work for iteration i runs inside iteration i+1.
  - PSUM budget (8 banks): scores 2x[128,1024]=4, A0/A1=2, dacc=1 (both
    parities share one bank), deferred-work/transposes=1.
"""

import sys

sys.path.insert(0, "/opt/trn_rl_repo")

import numpy as np
import ml_dtypes

import concourse.bass as bass  # noqa: F401
import concourse.mybir as mybir
import concourse.tile as tile
from concourse import bacc
from concourse.bass_utils import run_bass_kernel_spmd
from concourse.masks import make_identity

# Steer every activation to the one table set containing both Exp and Ln
# (no ACT table thrash). See v1 notes: hide used funcs from other sets so
# the chooser can't pick them.
_orig_gat = bacc.get_activation_tables
_PREF_SET = "natural_log_exp_and_others"
def _gat_pref(arch):
    tabs = _orig_gat(arch)
    if _PREF_SET not in tabs:
        return tabs
    used = {mybir.ActivationFunctionType.Exp, mybir.ActivationFunctionType.Ln,
            mybir.ActivationFunctionType.Identity, mybir.ActivationFunctionType.Copy,
            mybir.ActivationFunctionType.Square}
    out = {}
    for name, fns in tabs.items():
        if name == _PREF_SET:
            out[name] = fns
        else:
            out[name] = {f for f in fns if f not in used}
    return out
bacc.get_activation_tables = _gat_pref

F32 = mybir.dt.float32
F32R = mybir.dt.float32r
BF16 = mybir.dt.bfloat16
AL = mybir.AluOpType
AF = mybir.ActivationFunctionType
BF = ml_dtypes.bfloat16

B, N, DIM = 2, 2048, 1024
NUM_HEADS = 16
EFF = NUM_HEADS // 2
HD = DIM // NUM_HEADS          # 64
HPC = 4                        # raw heads per core
PPC = 2                        # head pairs per core
DLOC = HPC * HD                # 256 local feature dims
LAMBDA_INIT = 0.8
EPS = 1e-5
SCALE = HD ** -0.5

NT = N // 128                  # 16 token tiles of 128
NC4 = N // 512                 # 4 chunks of 512 tokens
KB = N // 128                  # 16 k-blocks
NG = KB // 2                   # 8 k-block pair groups

_CACHE = {}


def _build_nc():
    nc = bacc.Bacc()

    x_ = nc.declare_dram_parameter("x", [N, DIM], BF16, isOutput=False)
    # weights shipped pre-tiled: [128, 8*DLOC] where slice i*DLOC:(i+1)*DLOC
    # is the [128, DLOC] tile for input-feature block i (host reshapes).
    wqT = nc.declare_dram_parameter("wqT", [128, 8 * DLOC], BF16, isOutput=False)
    wkT = nc.declare_dram_parameter("wkT", [128, 8 * DLOC], BF16, isOutput=False)
    wvT = nc.declare_dram_parameter("wvT", [128, 8 * DLOC], BF16, isOutput=False)
    woT = nc.declare_dram_parameter("woT", [128, 2 * DIM], BF16, isOutput=False)
    # bpack cols: 0-1 bq (scale-folded) fc halves, 2-3 bk fc halves
    bpack_ = nc.declare_dram_parameter("bpack", [128, 4], F32, isOutput=False)
    bv_ = nc.declare_dram_parameter("bv", [DLOC], F32, isOutput=False)
    # lrow: [lq1 | lk1 | lq2 | lk2]
    lrow_ = nc.declare_dram_parameter("lrow", [4 * HD], F32, isOutput=False)
    out_ = nc.declare_dram_parameter("out", [N, DIM], F32, isOutput=True)

    with tile.TileContext(nc) as tc:
        with tc.tile_pool(name="persist", bufs=1) as pp:
            # ---- constants ----
            identf = pp.tile([128, 128], F32, tag="identf")
            make_identity(nc, identf[:])
            identb = pp.tile([128, 128], BF16, tag="identb")
            make_identity(nc, identb[:])
            identr = pp.tile([128, 128], F32R, tag="identr")
            nc.vector.tensor_copy(identr[:], identf[:])
            ones_bf = pp.tile([128, 1], BF16, tag="ones")
            nc.vector.memset(ones_bf[:], 1.0)
            ones_f = pp.tile([128, 1], F32, tag="onesf")
            nc.vector.memset(ones_f[:], 1.0)

            # ---- DMA emission order is the DMA service order: feed the
            # first k/v projections ASAP (wk, xT chunk 0, wv), then smalls,
            # then the rest of x interleaved with remaining weights. ----
            xT = [pp.tile([128, N], BF16, tag=f"xT{i}", name=f"xT{i}") for i in range(8)]

            def emit_xt_chunk(tc4):
                for i in range(8):
                    nc.sync.dma_start_transpose(
                        xT[i][:, tc4 * 512:(tc4 + 1) * 512],
                        x_[tc4 * 512:(tc4 + 1) * 512, i * 128:(i + 1) * 128],
                    )

            wk_all = pp.tile([128, 8 * DLOC], BF16, tag="wk")
            nc.sync.dma_start(wk_all[:], wkT[:])
            bpack_t = pp.tile([128, 4], F32, tag="bpack")
            nc.sync.dma_start(bpack_t[:], bpack_[:])
            bq_t = [bpack_t[:, fc:fc + 1] for fc in range(2)]
            bk_t = [bpack_t[:, 2 + fc:3 + fc] for fc in range(2)]
            emit_xt_chunk(0)
            emit_xt_chunk(1)
            wv_all = pp.tile([128, 8 * DLOC], BF16, tag="wv")
            nc.sync.dma_start(wv_all[:], wvT[:])

            # ---- small DMAs (v bias, lambda) ----
            bv_row = pp.tile([1, DLOC], F32, tag="bvrow")
            nc.sync.dma_start(bv_row[:], bv_[:].rearrange("(one f) -> one f", one=1))
            bv_bc = pp.tile([128, DLOC], F32, tag="bvbc")
            nc.gpsimd.partition_broadcast(bv_bc[:], bv_row[:])

            lrow = pp.tile([1, 4 * HD], F32, tag="lrow")
            nc.sync.dma_start(lrow[:], lrow_[:].rearrange("(one f) -> one f", one=1))
            lprod = pp.tile([1, 2 * HD], F32, tag="lprod")
            nc.vector.tensor_mul(lprod[:, 0:HD], lrow[:, 0:HD], lrow[:, HD:2 * HD])
            nc.vector.tensor_mul(lprod[:, HD:2 * HD], lrow[:, 2 * HD:3 * HD], lrow[:, 3 * HD:4 * HD])
            lsum = pp.tile([1, 2], F32, tag="lsum")
            nc.vector.tensor_reduce(lsum[:, 0:1], lprod[:, 0:HD], mybir.AxisListType.X, AL.add)
            nc.vector.tensor_reduce(lsum[:, 1:2], lprod[:, HD:2 * HD], mybir.AxisListType.X, AL.add)
            lexp = pp.tile([1, 2], F32, tag="lexp")
            nc.scalar.activation(lexp[:], lsum[:], AF.Exp)
            lam_t = pp.tile([1, 1], F32, tag="lam")
            nc.vector.tensor_sub(lam_t[:], lexp[:, 0:1], lexp[:, 1:2])
            nc.vector.tensor_scalar_add(lam_t[:], lam_t[:], LAMBDA_INIT)
            lam_col = pp.tile([128, 1], F32, tag="lamc")
            nc.gpsimd.partition_broadcast(lam_col[:], lam_t[:])

            # ---- remaining bulk loads: chunks 2+3 merged per column tile
            # (halves the per-instruction HWDGE overhead, delivers chunk 3
            # earlier) ----
            for i in range(8):
                nc.sync.dma_start_transpose(
                    xT[i][:, 1024:2048], x_[1024:2048, i * 128:(i + 1) * 128])
            wq_all = pp.tile([128, 8 * DLOC], BF16, tag="wq")
            nc.sync.dma_start(wq_all[:], wqT[:])
            wo_all = pp.tile([128, 2 * DIM], BF16, tag="wo")
            nc.sync.dma_start(wo_all[:], woT[:])

            def wk_i(i):
                return wk_all[:, i * DLOC:(i + 1) * DLOC]

            def wv_i(i):
                return wv_all[:, i * DLOC:(i + 1) * DLOC]

            def wq_i(i):
                return wq_all[:, i * DLOC:(i + 1) * DLOC]

            def wo_p(p):
                return wo_all[:, p * DIM:(p + 1) * DIM]

            # ---- persistent activations ----
            qT_sb = [pp.tile([128, N], BF16, tag=f"qT{fc}", name=f"qT{fc}") for fc in range(2)]
            kT_sb = [pp.tile([128, N], BF16, tag=f"kT{fc}", name=f"kT{fc}") for fc in range(2)]
            v_sb = [pp.tile([128, DLOC], BF16, tag=f"v{tt}", name=f"v{tt}") for tt in range(NT)]
            oTn = [pp.tile([128, N], BF16, tag=f"oTn{p}", name=f"oTn{p}") for p in range(PPC)]

            # ================= PHASE A: k/v projections + qT chunk 0 ========
            with tc.tile_pool(name="pja", bufs=3, space="PSUM") as pj_ps:
                def emit_kproj(fc, tc4):
                    tsl = slice(tc4 * 512, (tc4 + 1) * 512)
                    ps = pj_ps.tile([128, 512], F32, tag="pj", name="kps")
                    for i in range(8):
                        nc.tensor.matmul(ps[:], wk_i(i)[:, fc * 128:(fc + 1) * 128],
                                         xT[i][:, tsl], start=(i == 0), stop=(i == 7))
                    nc.vector.tensor_scalar(kT_sb[fc][:, tsl], ps[:], 1.0, bk_t[fc],
                                            AL.mult, AL.add)

                def emit_vproj(tc4, tt, pool=None):
                    t = tc4 * 4 + tt
                    pool = pool if pool is not None else pj_ps
                    ps = pool.tile([128, DLOC], F32,
                                   tag="pj" if pool is pj_ps else "t", name="vps")
                    for i in range(8):
                        nc.tensor.matmul(ps[:], xT[i][:, t * 128:(t + 1) * 128],
                                         wv_i(i), start=(i == 0), stop=(i == 7))
                    nc.vector.scalar_tensor_tensor(v_sb[t][:], ps[:], 1.0, bv_bc[:],
                                                   AL.mult, AL.add)

                def emit_qproj(fc, tc4, pool):
                    tsl = slice(tc4 * 512, (tc4 + 1) * 512)
                    ps = pool.tile([128, 512], F32, tag="t", name="qps")
                    for i in range(8):
                        nc.tensor.matmul(ps[:], wq_i(i)[:, fc * 128:(fc + 1) * 128],
                                         xT[i][:, tsl], start=(i == 0), stop=(i == 7))
                    nc.vector.tensor_scalar(qT_sb[fc][:, tsl], ps[:], 1.0, bq_t[fc],
                                            AL.mult, AL.add)

                emit_kproj(0, 0)
                emit_kproj(1, 0)
                emit_kproj(0, 1)
                emit_kproj(1, 1)
                for tt in range(4):
                    emit_vproj(0, tt)
                emit_kproj(0, 2)
                emit_kproj(1, 2)
                for tt in range(4):
                    emit_vproj(1, tt)
                emit_kproj(0, 3)
                emit_kproj(1, 3)
                for tt in range(4):
                    emit_vproj(2, tt)
                for fc in range(2):
                    emit_qproj(fc, 0, pj_ps)

            # ========== PHASE B: attention + norm + output projection ======
            with (
                tc.tile_pool(name="s_ps", bufs=2, space="PSUM") as s_ps,
                tc.tile_pool(name="a_ps", bufs=2, space="PSUM") as a_ps,
                tc.tile_pool(name="d_ps", bufs=1, space="PSUM") as d_ps,
                tc.tile_pool(name="t_ps", bufs=1, space="PSUM") as t_ps,
                tc.tile_pool(name="e_sb", bufs=6) as e_pool,
                tc.tile_pool(name="cmb", bufs=8) as cmb_pool,
                tc.tile_pool(name="sqp", bufs=2) as sq_pool,
                tc.tile_pool(name="row", bufs=14) as row_pool,
                tc.tile_pool(name="bc", bufs=6) as bc_pool,
                tc.tile_pool(name="o_sb", bufs=4) as o_sb,
            ):
                # one persistent PSUM bank for all denominator accumulators;
                # (qc,p) parity picks column block 0:8 or 8:16.
                dacc_all = d_ps.tile([128, 16], F32, tag="dacc", name="dacc_all")

                def emit_scores(qc, p, g, A0, A1, dcols):
                    """Scores + exp for k-blocks 2g, 2g+1; returns pending
                    attn work (consumed one group later to keep PE fed)."""
                    qsl = slice(qc * 512, (qc + 1) * 512)
                    kb0 = 2 * g
                    s0 = s_ps.tile([128, 1024], F32, tag="s", name="s0")
                    s1 = s_ps.tile([128, 1024], F32, tag="s", name="s1")
                    for j in range(2):
                        ksl = slice((kb0 + j) * 128, (kb0 + j + 1) * 128)
                        nc.tensor.matmul(s0[:, j * 512:(j + 1) * 512],
                                         kT_sb[p][0:64, ksl],
                                         qT_sb[p][0:64, qsl],
                                         start=True, stop=True, skip_group_check=True)
                        nc.tensor.matmul(s1[:, j * 512:(j + 1) * 512],
                                         kT_sb[p][64:128, ksl],
                                         qT_sb[p][64:128, qsl],
                                         start=True, stop=True, skip_group_check=True)
                    e0 = e_pool.tile([128, 1024], BF16, tag="e", name="e0")
                    nc.scalar.activation(e0[:], s0[:], AF.Exp)
                    e1 = e_pool.tile([128, 1024], BF16, tag="e", name="e1")
                    nc.scalar.activation(e1[:], s1[:], AF.Exp)
                    return (p, g, e0, e1, A0, A1, dcols)

                def emit_attn(w):
                    """v-contraction + tiny denominator matmuls for a group."""
                    p, g, e0, e1, A0, A1, dcols = w
                    for j in range(2):
                        kb = 2 * g + j
                        st, sp = (kb == 0), (kb == KB - 1)
                        vt = v_sb[kb][:, p * 128:(p + 1) * 128]
                        jsl = slice(j * 512, (j + 1) * 512)
                        nc.tensor.matmul(A0[:], vt, e0[:, jsl], start=st, stop=sp)
                        nc.tensor.matmul(A1[:], vt, e1[:, jsl], start=st, stop=sp)
                        for s4 in range(4):
                            esl = slice(j * 512 + s4 * 128, j * 512 + (s4 + 1) * 128)
                            # PSUM start=True resets a whole granule (wipes
                            # neighboring columns): reset only on the FIRST
                            # matmul of the group-set; the rest accumulate
                            # onto the zeroed granule.
                            st_d = st and s4 == 0
                            nc.tensor.matmul(dacc_all[:, dcols + s4:dcols + s4 + 1],
                                             e0[:, esl], ones_bf[:],
                                             start=st_d, stop=sp, skip_group_check=True)
                            nc.tensor.matmul(dacc_all[:, dcols + 4 + s4:dcols + 5 + s4],
                                             e1[:, esl], ones_bf[:],
                                             start=False, stop=sp, skip_group_check=True)

                def boundary0(qc, p, A0, A1, dcols, copy=True):
                    """Right after kb15: free A banks via SBUF copies; start
                    the c = lam*d0/d1 chain on DVE."""
                    if copy:
                        A0c = cmb_pool.tile([128, 512], F32, tag="m", name="A0c")
                        nc.vector.tensor_copy(A0c[:], A0[:])
                        A1c = cmb_pool.tile([128, 512], F32, tag="m", name="A1c")
                        nc.vector.tensor_copy(A1c[:], A1[:])
                    else:
                        A0c, A1c = A0, A1
                    cinv = row_pool.tile([128, 4], F32, tag="row", name="cinv")
                    nc.vector.reciprocal(cinv[:], dacc_all[:, dcols + 4:dcols + 8])
                    cl = row_pool.tile([128, 4], F32R, tag="rowc", name="cl")
                    nc.vector.tensor_mul(cl[:], cinv[:], dacc_all[:, dcols:dcols + 4])
                    nc.vector.tensor_scalar(cl[:], cl[:], lam_col[:, 0:1], None,
                                            AL.mult, AL.bypass)
                    # ou is d0-scaled, so the reference's +EPS inside the
                    # RMSNorm becomes +EPS*d0^2. Capture it now -- this dacc
                    # parity slot is re-accumulated two iterations later.
                    d0c = row_pool.tile([128, 4], F32, tag="row", name="d0c")
                    nc.vector.tensor_copy(d0c[:], dacc_all[:, dcols:dcols + 4])
                    epsd = row_pool.tile([128, 4], F32, tag="row", name="epsd")
                    nc.vector.tensor_mul(epsd[:], d0c[:], d0c[:])
                    nc.vector.tensor_scalar(epsd[:], epsd[:], EPS, None,
                                            AL.mult, AL.bypass)
                    return (qc, p, A0c, A1c, cl, epsd)

                def bcast_cols(vec4, name, dt=F32, ident=None):
                    """[128,4] per-(qsub) column vector -> [128,512] broadcast.
                    Four M=1 PE transposes assemble a [1,512] row (partition 0
                    only -- partition_broadcast from nonzero bases is broken on
                    HW), then one broadcast."""
                    ident = identf if ident is None else ident
                    vT5 = t_ps.tile([1, 512], dt, tag="t", name=f"{name}T")
                    for s4 in range(4):
                        nc.tensor.matmul(vT5[0:1, s4 * 128:(s4 + 1) * 128],
                                         vec4[:, s4:s4 + 1], ident[:],
                                         is_transpose=True, start=True, stop=True,
                                         skip_group_check=True)
                    row = row_pool.tile([1, 512], dt, tag="rowr", name=f"{name}row")
                    nc.vector.tensor_copy(row[:], vT5[:])
                    bc = bc_pool.tile([128, 512], dt, tag="bc", name=f"{name}bc")
                    nc.gpsimd.partition_broadcast(bc[:], row[:])
                    return bc

                def boundary1(st1):
                    """Diff-combine: ou = A0 - c*A1 (c broadcast per qsub)."""
                    qc, p, A0c, A1c, cl, epsd = st1
                    # f32r transpose (1.5 c/row vs 2 for f32); rounding error
                    # ~1.6e-4 on the diff coefficient is negligible.
                    bcc = bcast_cols(cl, "c", F32R, identr)
                    m1 = cmb_pool.tile([128, 512], F32, tag="m", name="m1")
                    nc.vector.tensor_mul(m1[:], A1c[:], bcc[:])
                    ou = cmb_pool.tile([128, 512], F32, tag="m", name="ou")
                    nc.vector.tensor_sub(ou[:], A0c[:], m1[:])
                    return (qc, p, ou, epsd)

                def boundary2(st2, tail=False):
                    """RMSNorm (scale-invariant: ou is d0-scaled, harmless)."""
                    qc, p, ou, epsd = st2
                    qsl = slice(qc * 512, (qc + 1) * 512)
                    sq = sq_pool.tile([128, 512], BF16, tag="sq", name="sq")
                    if tail:
                        nc.scalar.activation(sq[:], ou[:], AF.Square)
                    else:
                        nc.vector.tensor_mul(sq[:], ou[:], ou[:])
                    rms = t_ps.tile([128, 4], F32, tag="t", name="rms")
                    for s4 in range(4):
                        nc.tensor.matmul(rms[:, s4:s4 + 1],
                                         sq[:, s4 * 128:(s4 + 1) * 128], ones_bf[:],
                                         start=True, stop=True, skip_group_check=True)
                    msn = row_pool.tile([128, 4], F32, tag="row", name="msn")
                    nc.vector.scalar_tensor_tensor(msn[:], rms[:], 1.0 / (2 * HD),
                                                   epsd[:], AL.mult, AL.add)
                    lnm = row_pool.tile([128, 4], F32, tag="row", name="lnm")
                    nc.scalar.activation(lnm[:], msn[:], AF.Ln)
                    rstd = row_pool.tile([128, 4], BF16, tag="rowb", name="rstd")
                    nc.scalar.activation(rstd[:], lnm[:], AF.Exp, scale=-0.5)
                    bcr = bcast_cols(rstd, "r", BF16, identb)
                    nc.vector.tensor_mul(oTn[p][:, qsl], ou[:], bcr[:])

                def emit_outproj_tile(qc, tt4, pool, split_dma=False):
                    tt = qc * 4 + tt4
                    tsl = slice(tt * 128, (tt + 1) * 128)
                    ot = o_sb.tile([128, DIM], F32, tag="ot", name="ot")
                    for oc in range(2):
                        osl = slice(oc * 512, (oc + 1) * 512)
                        ps = pool.tile([128, 512], F32, tag="t" if pool is t_ps else "A",
                                       name="ops")
                        for p in range(PPC):
                            nc.tensor.matmul(ps[:], oTn[p][:, tsl],
                                             wo_p(p)[:, osl],
                                             start=(p == 0), stop=(p == PPC - 1))
                        if split_dma:
                            nc.scalar.activation(ot[:, osl], ps[:], AF.Copy)
                        else:
                            nc.vector.tensor_copy(ot[:, osl], ps[:])
                        if split_dma:
                            nc.sync.dma_start(out_[tsl, osl], ot[:, osl])
                    if not split_dma:
                        nc.sync.dma_start(out_[tsl, :], ot[:])

                pend_attn = None   # attn work lagging one group behind scores
                prev_acc = None    # (qc, p, A0, A1, dcols) awaiting boundary0
                pend1 = None       # awaiting boundary1 (diff-combine)
                pend2 = None       # awaiting boundary2 (RMSNorm)
                parity = 0
                for qc in range(NC4):
                    for p in range(PPC):
                        A0 = a_ps.tile([128, 512], F32, tag="A", name="A0")
                        A1 = a_ps.tile([128, 512], F32, tag="A", name="A1")
                        dcols = parity * 8
                        for g in range(NG):
                            w = emit_scores(qc, p, g, A0, A1, dcols)
                            if pend_attn is not None:
                                emit_attn(pend_attn)
                            pend_attn = w
                            if qc == 0 and p == 0 and g < 4:
                                emit_vproj(3, g, t_ps)
                            if g == 0 and prev_acc is not None:
                                pend1 = boundary0(*prev_acc)
                                prev_acc = None
                            if g == 3 and pend1 is not None:
                                pend2 = boundary1(pend1)
                                pend1 = None
                            if g == 6 and pend2 is not None:
                                boundary2(pend2)
                                pend2 = None
                            if g == 5 and p == 0 and qc < NC4 - 1:
                                for fc in range(2):
                                    emit_qproj(fc, qc + 1, t_ps)
                            if p == 1 and qc > 0 and g in (2, 3, 4, 5):
                                emit_outproj_tile(qc - 1, g - 2, t_ps)
                        prev_acc = (qc, p, A0, A1, dcols)
                        parity ^= 1
                # tail: finish last (qc=3, p=1). The p=0 halves of the first
                # two out-projection tiles are issued into the freed score
                # banks while the DVE c-chain runs (PE would otherwise idle),
                # then completed with the p=1 halves after the final RMSNorm.
                emit_attn(pend_attn)
                st0 = boundary0(*prev_acc, copy=False)
                sA = s_ps.tile([128, 1024], F32, tag="s", name="opA")
                sB = s_ps.tile([128, 1024], F32, tag="s", name="opB")
                for tt4 in range(2):
                    tt = (NC4 - 1) * 4 + tt4
                    tsl = slice(tt * 128, (tt + 1) * 128)
                    hold = sA if tt4 == 0 else sB
                    for oc in range(2):
                        osl = slice(oc * 512, (oc + 1) * 512)
                        nc.tensor.matmul(hold[:, oc * 512:(oc + 1) * 512],
                                         oTn[0][:, tsl], wo_p(0)[:, osl],
                                         start=True, stop=False,
                                         skip_group_check=True)
                st2t = boundary1(st0)
                # tile 2's p0 halves into the A banks just freed by ou
                aC = a_ps.tile([128, 512], F32, tag="A", name="opC")
                aD = a_ps.tile([128, 512], F32, tag="A", name="opD")
                tt2sl = slice(((NC4 - 1) * 4 + 2) * 128, ((NC4 - 1) * 4 + 3) * 128)
                nc.tensor.matmul(aC[:], oTn[0][:, tt2sl], wo_p(0)[:, 0:512],
                                 start=True, stop=False, skip_group_check=True)
                nc.tensor.matmul(aD[:], oTn[0][:, tt2sl], wo_p(0)[:, 512:1024],
                                 start=True, stop=False, skip_group_check=True)
                boundary2(st2t, tail=True)
                for tt4 in range(2):
                    tt = (NC4 - 1) * 4 + tt4
                    tsl = slice(tt * 128, (tt + 1) * 128)
                    hold = sA if tt4 == 0 else sB
                    ot = o_sb.tile([128, DIM], F32, tag="ot", name="ot")
                    for oc in range(2):
                        osl = slice(oc * 512, (oc + 1) * 512)
                        nc.tensor.matmul(hold[:, oc * 512:(oc + 1) * 512],
                                         oTn[1][:, tsl], wo_p(1)[:, osl],
                                         start=False, stop=True,
                                         skip_group_check=True)
                        nc.scalar.activation(ot[:, osl],
                                             hold[:, oc * 512:(oc + 1) * 512],
                                             AF.Copy)
                        nc.sync.dma_start(out_[tsl, osl], ot[:, osl])
                ot2 = o_sb.tile([128, DIM], F32, tag="ot", name="ot2")
                for oc, hold in ((0, aC), (1, aD)):
                    osl = slice(oc * 512, (oc + 1) * 512)
                    nc.tensor.matmul(hold[:], oTn[1][:, tt2sl], wo_p(1)[:, osl],
                                     start=False, stop=True, skip_group_check=True)
                    nc.scalar.activation(ot2[:, osl], hold[:], AF.Copy)
                    nc.sync.dma_start(out_[tt2sl, osl], ot2[:, osl])
                emit_outproj_tile(NC4 - 1, 3, t_ps, split_dma=True)

    nc.finalize()
    return nc


def kernel(x, Wq, bq, Wk, bk, Wv, bv, Wo, bo, norm_w, lq1, lk1, lq2, lk2):
    x = np.asarray(x, dtype=np.float32)
    Wq = np.asarray(Wq, dtype=np.float32)
    Wk = np.asarray(Wk, dtype=np.float32)
    Wv = np.asarray(Wv, dtype=np.float32)
    Wo = np.asarray(Wo, dtype=np.float32)
    nw = np.asarray(norm_w, np.float32)

    if "nc" not in _CACHE:
        _CACHE["nc"] = _build_nc()
    nc = _CACHE["nc"]

    in_maps = []
    for c in range(8):
        b, g = c // 4, c % 4
        sl = slice(g * DLOC, (g + 1) * DLOC)
        # fold q-scale into Wq/bq; fold norm_w * (1-lambda_init) into Wo.
        # pre-tile [DIM, DLOC] -> [128, 8*DLOC] (block i at cols i*DLOC:)
        def tile8(w):  # w: [1024, 256] -> [128, 2048]
            return np.ascontiguousarray(
                w.reshape(8, 128, DLOC).transpose(1, 0, 2).reshape(128, 8 * DLOC))

        wo_scale = (nw[np.arange(DLOC) % (2 * HD)] * (1.0 - LAMBDA_INIT)).astype(np.float32)
        woT_full = (Wo[:, sl] * wo_scale[None, :]).T  # [256, 1024]
        in_maps.append({
            "x": np.ascontiguousarray(x[b]).astype(BF),
            "wqT": tile8((Wq[sl, :] * SCALE).T).astype(BF),
            "wkT": tile8(Wk[sl, :].T).astype(BF),
            "wvT": tile8(Wv[sl, :].T).astype(BF),
            "woT": np.ascontiguousarray(
                woT_full.reshape(2, 128, DIM).transpose(1, 0, 2).reshape(128, 2 * DIM)).astype(BF),
            "bpack": np.ascontiguousarray(np.stack([
                np.asarray(bq, np.float32)[sl][0:128] * SCALE,
                np.asarray(bq, np.float32)[sl][128:256] * SCALE,
                np.asarray(bk, np.float32)[sl][0:128],
                np.asarray(bk, np.float32)[sl][128:256],
            ], axis=1)),
            "bv": np.ascontiguousarray(np.asarray(bv, np.float32)[sl]),
            "lrow": np.ascontiguousarray(np.concatenate([
                np.asarray(lq1, np.float32), np.asarray(lk1, np.float32),
                np.asarray(lq2, np.float32), np.asarray(lk2, np.float32)])),
        })

    res = None
    for attempt in range(3):
        try:
            res = run_bass_kernel_spmd(nc, in_maps, list(range(8))).results
            break
        except Exception:
            if attempt == 2:
                raise
            import time as _time

            _time.sleep(5 + 10 * attempt)
            try:
                import jax

                jax.clear_caches()
                jax.extend.backend.clear_backends()
            except Exception:
                pass
    bo_f = np.asarray(bo, np.float32)
    out = np.empty((B, N, DIM), np.float32)
    for b in range(B):
        acc = res[4 * b]["out"].astype(np.float32)
        for g in range(1, 4):
            acc = acc + res[4 * b + g]["out"]
        out[b] = acc + bo_f[None, :]
    return out


# revision 58
# speedup vs baseline: 1.0561x; 1.0068x over previous
"""DiffAttention TRN2 kernel v2: 8-core SPMD (batch x head-group sharding).

Sharding: core c -> batch b = c//4, head-group g = c%4 (4 raw heads = 2
effective pairs per core). Each core computes q/k/v projections for its
head slice, diff-attention, headwise RMSNorm, and a partial output
projection (its 256 input-feature slice of Wo). Host sums the 4 partials
per batch and adds bo.

v2 design notes (cost model: matmul time = out-free-size * cycles/row,
independent of K and M):
  - x shipped bf16 from host; xT produced by DMA XBAR transpose (no PE
    transposes, no PSUM->SBUF copies for xT).
  - all weights bf16 host-side; q-scale folded into Wq/bq; norm_w and
    (1-lambda_init) folded into Wo columns host-side.
  - softmax denominators via tiny matmuls d[128q,1] = e[128k,128q]^T @
    ones (1 cycle each) instead of M=1 ones-matmuls (512 cycles each).
    PSUM pitfall: a matmul with start=True resets a whole accumulation
    granule (wipes neighboring columns mid-accumulation) -- only the
    FIRST matmul of the interleaved group-set carries start=True.
  - diff-combine uses RMSNorm scale-invariance: ou = A0 - c*A1 with
    c = lam*d0/d1 (one per-q broadcast for the diff + one for rstd);
    the reference's +EPS becomes +EPS*d0^2 since ou is d0-scaled.
  - per-q row vectors are assembled via four M=1 PE transposes into a
    [1,512] PSUM row (partition_broadcast from nonzero partition bases
    is broken on HW), then one partition_broadcast.
  - exps widened to 2 PSUM banks ([128,1024]) to amortize Act access.
  - software pipelining: attention matmuls lag their scores/exp by one
    k-group; v chunk 3 / q-proj chunks 1-3 / the output projection are
    deferred into the attention loop to fill PE stall slots; boundary
    (combine + RMSNorm) work for iteration i runs inside iteration i+1.
  - PSUM budget (8 banks): scores 2x[128,1024]=4, A0/A1=2, dacc=1 (both
    parities share one bank), deferred-work/transposes=1.
"""

import sys

sys.path.insert(0, "/opt/trn_rl_repo")

import numpy as np
import ml_dtypes

import concourse.bass as bass  # noqa: F401
import concourse.mybir as mybir
import concourse.tile as tile
from concourse import bacc
from concourse.bass_utils import run_bass_kernel_spmd
from concourse.masks import make_identity

# Steer every activation to the one table set containing both Exp and Ln
# (no ACT table thrash). See v1 notes: hide used funcs from other sets so
# the chooser can't pick them.
_orig_gat = bacc.get_activation_tables
_PREF_SET = "natural_log_exp_and_others"
def _gat_pref(arch):
    tabs = _orig_gat(arch)
    if _PREF_SET not in tabs:
        return tabs
    used = {mybir.ActivationFunctionType.Exp, mybir.ActivationFunctionType.Ln,
            mybir.ActivationFunctionType.Identity, mybir.ActivationFunctionType.Copy,
            mybir.ActivationFunctionType.Square}
    out = {}
    for name, fns in tabs.items():
        if name == _PREF_SET:
            out[name] = fns
        else:
            out[name] = {f for f in fns if f not in used}
    return out
bacc.get_activation_tables = _gat_pref

F32 = mybir.dt.float32
F32R = mybir.dt.float32r
BF16 = mybir.dt.bfloat16
AL = mybir.AluOpType
AF = mybir.ActivationFunctionType
BF = ml_dtypes.bfloat16

B, N, DIM = 2, 2048, 1024
NUM_HEADS = 16
EFF = NUM_HEADS // 2
HD = DIM // NUM_HEADS          # 64
HPC = 4                        # raw heads per core
PPC = 2                        # head pairs per core
DLOC = HPC * HD                # 256 local feature dims
LAMBDA_INIT = 0.8
EPS = 1e-5
SCALE = HD ** -0.5

NT = N // 128                  # 16 token tiles of 128
NC4 = N // 512                 # 4 chunks of 512 tokens
KB = N // 128                  # 16 k-blocks
NG = KB // 2                   # 8 k-block pair groups

_CACHE = {}


def _build_nc():
    nc = bacc.Bacc()

    x_ = nc.declare_dram_parameter("x", [N, DIM], BF16, isOutput=False)
    # weights shipped pre-tiled: [128, 8*DLOC] where slice i*DLOC:(i+1)*DLOC
    # is the [128, DLOC] tile for input-feature block i (host reshapes).
    wqT = nc.declare_dram_parameter("wqT", [128, 8 * DLOC], BF16, isOutput=False)
    wkT = nc.declare_dram_parameter("wkT", [128, 8 * DLOC], BF16, isOutput=False)
    wvT = nc.declare_dram_parameter("wvT", [128, 8 * DLOC], BF16, isOutput=False)
    woT = nc.declare_dram_parameter("woT", [128, 2 * DIM], BF16, isOutput=False)
    # bpack cols: 0-1 bq (scale-folded) fc halves, 2-3 bk fc halves
    bpack_ = nc.declare_dram_parameter("bpack", [128, 4], F32, isOutput=False)
    bv_ = nc.declare_dram_parameter("bv", [DLOC], F32, isOutput=False)
    # lrow: [lq1 | lk1 | lq2 | lk2]
    lrow_ = nc.declare_dram_parameter("lrow", [4 * HD], F32, isOutput=False)
    out_ = nc.declare_dram_parameter("out", [N, DIM], F32, isOutput=True)

    with tile.TileContext(nc) as tc:
        with tc.tile_pool(name="persist", bufs=1) as pp:
            # ---- constants ----
            identf = pp.tile([128, 128], F32, tag="identf")
            make_identity(nc, identf[:])
            identb = pp.tile([128, 128], BF16, tag="identb")
            make_identity(nc, identb[:])
            identr = pp.tile([128, 128], F32R, tag="identr")
            nc.vector.tensor_copy(identr[:], identf[:])
            ones_bf = pp.tile([128, 1], BF16, tag="ones")
            nc.vector.memset(ones_bf[:], 1.0)
            ones_f = pp.tile([128, 1], F32, tag="onesf")
            nc.vector.memset(ones_f[:], 1.0)

            # ---- DMA emission order is the DMA service order: feed the
            # first k/v projections ASAP (wk, xT chunk 0, wv), then smalls,
            # then the rest of x interleaved with remaining weights. ----
            xT = [pp.tile([128, N], BF16, tag=f"xT{i}", name=f"xT{i}") for i in range(8)]

            def emit_xt_chunk(tc4):
                for i in range(8):
                    nc.sync.dma_start_transpose(
                        xT[i][:, tc4 * 512:(tc4 + 1) * 512],
                        x_[tc4 * 512:(tc4 + 1) * 512, i * 128:(i + 1) * 128],
                    )

            wk_all = pp.tile([128, 8 * DLOC], BF16, tag="wk")
            nc.sync.dma_start(wk_all[:], wkT[:])
            bpack_t = pp.tile([128, 4], F32, tag="bpack")
            nc.sync.dma_start(bpack_t[:], bpack_[:])
            bq_t = [bpack_t[:, fc:fc + 1] for fc in range(2)]
            bk_t = [bpack_t[:, 2 + fc:3 + fc] for fc in range(2)]
            emit_xt_chunk(0)
            emit_xt_chunk(1)
            wv_all = pp.tile([128, 8 * DLOC], BF16, tag="wv")
            nc.sync.dma_start(wv_all[:], wvT[:])

            # ---- small DMAs (v bias, lambda) ----
            bv_row = pp.tile([1, DLOC], F32, tag="bvrow")
            nc.sync.dma_start(bv_row[:], bv_[:].rearrange("(one f) -> one f", one=1))
            bv_bc = pp.tile([128, DLOC], F32, tag="bvbc")
            nc.gpsimd.partition_broadcast(bv_bc[:], bv_row[:])

            lrow = pp.tile([1, 4 * HD], F32, tag="lrow")
            nc.sync.dma_start(lrow[:], lrow_[:].rearrange("(one f) -> one f", one=1))
            lprod = pp.tile([1, 2 * HD], F32, tag="lprod")
            nc.vector.tensor_mul(lprod[:, 0:HD], lrow[:, 0:HD], lrow[:, HD:2 * HD])
            nc.vector.tensor_mul(lprod[:, HD:2 * HD], lrow[:, 2 * HD:3 * HD], lrow[:, 3 * HD:4 * HD])
            lsum = pp.tile([1, 2], F32, tag="lsum")
            nc.vector.tensor_reduce(lsum[:, 0:1], lprod[:, 0:HD], mybir.AxisListType.X, AL.add)
            nc.vector.tensor_reduce(lsum[:, 1:2], lprod[:, HD:2 * HD], mybir.AxisListType.X, AL.add)
            lexp = pp.tile([1, 2], F32, tag="lexp")
            nc.scalar.activation(lexp[:], lsum[:], AF.Exp)
            lam_t = pp.tile([1, 1], F32, tag="lam")
            nc.vector.tensor_sub(lam_t[:], lexp[:, 0:1], lexp[:, 1:2])
            nc.vector.tensor_scalar_add(lam_t[:], lam_t[:], LAMBDA_INIT)
            lam_col = pp.tile([128, 1], F32, tag="lamc")
            nc.gpsimd.partition_broadcast(lam_col[:], lam_t[:])

            # ---- remaining bulk loads: chunks 2+3 merged per column tile
            # (halves the per-instruction HWDGE overhead, delivers chunk 3
            # earlier) ----
            for i in range(8):
                nc.sync.dma_start_transpose(
                    xT[i][:, 1024:2048], x_[1024:2048, i * 128:(i + 1) * 128])
            wq_all = pp.tile([128, 8 * DLOC], BF16, tag="wq")
            nc.sync.dma_start(wq_all[:], wqT[:])
            wo_all = pp.tile([128, 2 * DIM], BF16, tag="wo")
            nc.sync.dma_start(wo_all[:], woT[:])

            def wk_i(i):
                return wk_all[:, i * DLOC:(i + 1) * DLOC]

            def wv_i(i):
                return wv_all[:, i * DLOC:(i + 1) * DLOC]

            def wq_i(i):
                return wq_all[:, i * DLOC:(i + 1) * DLOC]

            def wo_p(p):
                return wo_all[:, p * DIM:(p + 1) * DIM]

            # ---- persistent activations ----
            qT_sb = [pp.tile([128, N], BF16, tag=f"qT{fc}", name=f"qT{fc}") for fc in range(2)]
            kT_sb = [pp.tile([128, N], BF16, tag=f"kT{fc}", name=f"kT{fc}") for fc in range(2)]
            v_sb = [pp.tile([128, DLOC], BF16, tag=f"v{tt}", name=f"v{tt}") for tt in range(NT)]
            oTn = [pp.tile([128, N], BF16, tag=f"oTn{p}", name=f"oTn{p}") for p in range(PPC)]

            # ================= PHASE A: k/v projections + qT chunk 0 ========
            with tc.tile_pool(name="pja", bufs=3, space="PSUM") as pj_ps:
                def emit_kproj(fc, tc4):
                    tsl = slice(tc4 * 512, (tc4 + 1) * 512)
                    ps = pj_ps.tile([128, 512], F32, tag="pj", name="kps")
                    for i in range(8):
                        nc.tensor.matmul(ps[:], wk_i(i)[:, fc * 128:(fc + 1) * 128],
                                         xT[i][:, tsl], start=(i == 0), stop=(i == 7))
                    nc.vector.tensor_scalar(kT_sb[fc][:, tsl], ps[:], 1.0, bk_t[fc],
                                            AL.mult, AL.add)

                def emit_vproj(tc4, tt, pool=None):
                    t = tc4 * 4 + tt
                    pool = pool if pool is not None else pj_ps
                    ps = pool.tile([128, DLOC], F32,
                                   tag="pj" if pool is pj_ps else "t", name="vps")
                    for i in range(8):
                        nc.tensor.matmul(ps[:], xT[i][:, t * 128:(t + 1) * 128],
                                         wv_i(i), start=(i == 0), stop=(i == 7))
                    nc.vector.scalar_tensor_tensor(v_sb[t][:], ps[:], 1.0, bv_bc[:],
                                                   AL.mult, AL.add)

                def emit_qproj(fc, tc4, pool):
                    tsl = slice(tc4 * 512, (tc4 + 1) * 512)
                    ps = pool.tile([128, 512], F32, tag="t", name="qps")
                    for i in range(8):
                        nc.tensor.matmul(ps[:], wq_i(i)[:, fc * 128:(fc + 1) * 128],
                                         xT[i][:, tsl], start=(i == 0), stop=(i == 7))
                    nc.vector.tensor_scalar(qT_sb[fc][:, tsl], ps[:], 1.0, bq_t[fc],
                                            AL.mult, AL.add)

                emit_kproj(0, 0)
                emit_kproj(1, 0)
                emit_kproj(0, 1)
                emit_kproj(1, 1)
                for tt in range(4):
                    emit_vproj(0, tt)
                emit_kproj(0, 2)
                emit_kproj(1, 2)
                for tt in range(4):
                    emit_vproj(1, tt)
                emit_kproj(0, 3)
                emit_kproj(1, 3)
                for tt in range(4):
                    emit_vproj(2, tt)
                for fc in range(2):
                    emit_qproj(fc, 0, pj_ps)

            # ========== PHASE B: attention + norm + output projection ======
            with (
                tc.tile_pool(name="s_ps", bufs=2, space="PSUM") as s_ps,
                tc.tile_pool(name="a_ps", bufs=2, space="PSUM") as a_ps,
                tc.tile_pool(name="d_ps", bufs=1, space="PSUM") as d_ps,
                tc.tile_pool(name="t_ps", bufs=1, space="PSUM") as t_ps,
                tc.tile_pool(name="e_sb", bufs=6) as e_pool,
                tc.tile_pool(name="cmb", bufs=8) as cmb_pool,
                tc.tile_pool(name="sqp", bufs=2) as sq_pool,
                tc.tile_pool(name="row", bufs=14) as row_pool,
                tc.tile_pool(name="bc", bufs=6) as bc_pool,
                tc.tile_pool(name="o_sb", bufs=4) as o_sb,
            ):
                # one persistent PSUM bank for all denominator accumulators;
                # (qc,p) parity picks column block 0:8 or 8:16.
                dacc_all = d_ps.tile([128, 16], F32, tag="dacc", name="dacc_all")

                def emit_scores(qc, p, g, A0, A1, dcols):
                    """Scores + exp for k-blocks 2g, 2g+1; returns pending
                    attn work (consumed one group later to keep PE fed)."""
                    qsl = slice(qc * 512, (qc + 1) * 512)
                    kb0 = 2 * g
                    s0 = s_ps.tile([128, 1024], F32, tag="s", name="s0")
                    s1 = s_ps.tile([128, 1024], F32, tag="s", name="s1")
                    for j in range(2):
                        ksl = slice((kb0 + j) * 128, (kb0 + j + 1) * 128)
                        nc.tensor.matmul(s0[:, j * 512:(j + 1) * 512],
                                         kT_sb[p][0:64, ksl],
                                         qT_sb[p][0:64, qsl],
                                         start=True, stop=True, skip_group_check=True)
                    # exp for head 0 issued before head 1's scores: it starts
                    # ~430ns earlier and frees s0's banks sooner
                    e0 = e_pool.tile([128, 1024], BF16, tag="e", name="e0")
                    nc.scalar.activation(e0[:], s0[:], AF.Exp)
                    for j in range(2):
                        ksl = slice((kb0 + j) * 128, (kb0 + j + 1) * 128)
                        nc.tensor.matmul(s1[:, j * 512:(j + 1) * 512],
                                         kT_sb[p][64:128, ksl],
                                         qT_sb[p][64:128, qsl],
                                         start=True, stop=True, skip_group_check=True)
                    e1 = e_pool.tile([128, 1024], BF16, tag="e", name="e1")
                    nc.scalar.activation(e1[:], s1[:], AF.Exp)
                    return (p, g, e0, e1, A0, A1, dcols)

                def emit_attn(w):
                    """v-contraction + tiny denominator matmuls for a group."""
                    p, g, e0, e1, A0, A1, dcols = w
                    for j in range(2):
                        kb = 2 * g + j
                        st, sp = (kb == 0), (kb == KB - 1)
                        vt = v_sb[kb][:, p * 128:(p + 1) * 128]
                        jsl = slice(j * 512, (j + 1) * 512)
                        nc.tensor.matmul(A0[:], vt, e0[:, jsl], start=st, stop=sp)
                        nc.tensor.matmul(A1[:], vt, e1[:, jsl], start=st, stop=sp)
                        for s4 in range(4):
                            esl = slice(j * 512 + s4 * 128, j * 512 + (s4 + 1) * 128)
                            # PSUM start=True resets a whole granule (wipes
                            # neighboring columns): reset only on the FIRST
                            # matmul of the group-set; the rest accumulate
                            # onto the zeroed granule.
                            st_d = st and s4 == 0
                            nc.tensor.matmul(dacc_all[:, dcols + s4:dcols + s4 + 1],
                                             e0[:, esl], ones_bf[:],
                                             start=st_d, stop=sp, skip_group_check=True)
                            nc.tensor.matmul(dacc_all[:, dcols + 4 + s4:dcols + 5 + s4],
                                             e1[:, esl], ones_bf[:],
                                             start=False, stop=sp, skip_group_check=True)

                def boundary0(qc, p, A0, A1, dcols, copy=True):
                    """Right after kb15: free A banks via SBUF copies; start
                    the c = lam*d0/d1 chain on DVE."""
                    if copy:
                        A0c = cmb_pool.tile([128, 512], F32, tag="m", name="A0c")
                        nc.vector.tensor_copy(A0c[:], A0[:])
                        A1c = cmb_pool.tile([128, 512], F32, tag="m", name="A1c")
                        nc.vector.tensor_copy(A1c[:], A1[:])
                    else:
                        A0c, A1c = A0, A1
                    cinv = row_pool.tile([128, 4], F32, tag="row", name="cinv")
                    nc.vector.reciprocal(cinv[:], dacc_all[:, dcols + 4:dcols + 8])
                    cl = row_pool.tile([128, 4], F32R, tag="rowc", name="cl")
                    nc.vector.tensor_mul(cl[:], cinv[:], dacc_all[:, dcols:dcols + 4])
                    nc.vector.tensor_scalar(cl[:], cl[:], lam_col[:, 0:1], None,
                                            AL.mult, AL.bypass)
                    # ou is d0-scaled, so the reference's +EPS inside the
                    # RMSNorm becomes +EPS*d0^2. Capture it now -- this dacc
                    # parity slot is re-accumulated two iterations later.
                    d0c = row_pool.tile([128, 4], F32, tag="row", name="d0c")
                    nc.vector.tensor_copy(d0c[:], dacc_all[:, dcols:dcols + 4])
                    epsd = row_pool.tile([128, 4], F32, tag="row", name="epsd")
                    nc.vector.tensor_mul(epsd[:], d0c[:], d0c[:])
                    nc.vector.tensor_scalar(epsd[:], epsd[:], EPS, None,
                                            AL.mult, AL.bypass)
                    return (qc, p, A0c, A1c, cl, epsd)

                def bcast_cols(vec4, name, dt=F32, ident=None):
                    """[128,4] per-(qsub) column vector -> [128,512] broadcast.
                    Four M=1 PE transposes assemble a [1,512] row (partition 0
                    only -- partition_broadcast from nonzero bases is broken on
                    HW), then one broadcast."""
                    ident = identf if ident is None else ident
                    vT5 = t_ps.tile([1, 512], dt, tag="t", name=f"{name}T")
                    for s4 in range(4):
                        nc.tensor.matmul(vT5[0:1, s4 * 128:(s4 + 1) * 128],
                                         vec4[:, s4:s4 + 1], ident[:],
                                         is_transpose=True, start=True, stop=True,
                                         skip_group_check=True)
                    row = row_pool.tile([1, 512], dt, tag="rowr", name=f"{name}row")
                    nc.vector.tensor_copy(row[:], vT5[:])
                    bc = bc_pool.tile([128, 512], dt, tag="bc", name=f"{name}bc")
                    nc.gpsimd.partition_broadcast(bc[:], row[:])
                    return bc

                def boundary1(st1):
                    """Diff-combine: ou = A0 - c*A1 (c broadcast per qsub)."""
                    qc, p, A0c, A1c, cl, epsd = st1
                    # f32r transpose (1.5 c/row vs 2 for f32); rounding error
                    # ~1.6e-4 on the diff coefficient is negligible.
                    bcc = bcast_cols(cl, "c", F32R, identr)
                    m1 = cmb_pool.tile([128, 512], F32, tag="m", name="m1")
                    nc.vector.tensor_mul(m1[:], A1c[:], bcc[:])
                    ou = cmb_pool.tile([128, 512], F32, tag="m", name="ou")
                    nc.vector.tensor_sub(ou[:], A0c[:], m1[:])
                    return (qc, p, ou, epsd)

                def boundary2(st2, tail=False):
                    """RMSNorm (scale-invariant: ou is d0-scaled, harmless)."""
                    qc, p, ou, epsd = st2
                    qsl = slice(qc * 512, (qc + 1) * 512)
                    sq = sq_pool.tile([128, 512], BF16, tag="sq", name="sq")
                    if tail:
                        nc.scalar.activation(sq[:], ou[:], AF.Square)
                    else:
                        nc.vector.tensor_mul(sq[:], ou[:], ou[:])
                    rms = t_ps.tile([128, 4], F32, tag="t", name="rms")
                    for s4 in range(4):
                        nc.tensor.matmul(rms[:, s4:s4 + 1],
                                         sq[:, s4 * 128:(s4 + 1) * 128], ones_bf[:],
                                         start=True, stop=True, skip_group_check=True)
                    msn = row_pool.tile([128, 4], F32, tag="row", name="msn")
                    nc.vector.scalar_tensor_tensor(msn[:], rms[:], 1.0 / (2 * HD),
                                                   epsd[:], AL.mult, AL.add)
                    lnm = row_pool.tile([128, 4], F32, tag="row", name="lnm")
                    nc.scalar.activation(lnm[:], msn[:], AF.Ln)
                    rstd = row_pool.tile([128, 4], BF16, tag="rowb", name="rstd")
                    nc.scalar.activation(rstd[:], lnm[:], AF.Exp, scale=-0.5)
                    bcr = bcast_cols(rstd, "r", BF16, identb)
                    nc.vector.tensor_mul(oTn[p][:, qsl], ou[:], bcr[:])

                def emit_outproj_tile(qc, tt4, pool, split_dma=False):
                    tt = qc * 4 + tt4
                    tsl = slice(tt * 128, (tt + 1) * 128)
                    ot = o_sb.tile([128, DIM], F32, tag="ot", name="ot")
                    for oc in range(2):
                        osl = slice(oc * 512, (oc + 1) * 512)
                        ps = pool.tile([128, 512], F32, tag="t" if pool is t_ps else "A",
                                       name="ops")
                        for p in range(PPC):
                            nc.tensor.matmul(ps[:], oTn[p][:, tsl],
                                             wo_p(p)[:, osl],
                                             start=(p == 0), stop=(p == PPC - 1))
                        if split_dma:
                            nc.scalar.activation(ot[:, osl], ps[:], AF.Copy)
                        else:
                            nc.vector.tensor_copy(ot[:, osl], ps[:])
                        if split_dma:
                            nc.sync.dma_start(out_[tsl, osl], ot[:, osl])
                    if not split_dma:
                        nc.sync.dma_start(out_[tsl, :], ot[:])

                pend_attn = None   # attn work lagging one group behind scores
                prev_acc = None    # (qc, p, A0, A1, dcols) awaiting boundary0
                pend1 = None       # awaiting boundary1 (diff-combine)
                pend2 = None       # awaiting boundary2 (RMSNorm)
                parity = 0
                for qc in range(NC4):
                    for p in range(PPC):
                        A0 = a_ps.tile([128, 512], F32, tag="A", name="A0")
                        A1 = a_ps.tile([128, 512], F32, tag="A", name="A1")
                        dcols = parity * 8
                        for g in range(NG):
                            w = emit_scores(qc, p, g, A0, A1, dcols)
                            if pend_attn is not None:
                                emit_attn(pend_attn)
                            pend_attn = w
                            if qc == 0 and p == 0 and g < 4:
                                emit_vproj(3, g, t_ps)
                            if g == 0 and prev_acc is not None:
                                pend1 = boundary0(*prev_acc)
                                prev_acc = None
                            if g == 3 and pend1 is not None:
                                pend2 = boundary1(pend1)
                                pend1 = None
                            if g == 6 and pend2 is not None:
                                boundary2(pend2)
                                pend2 = None
                            if g == 5 and p == 0 and qc < NC4 - 1:
                                for fc in range(2):
                                    emit_qproj(fc, qc + 1, t_ps)
                            if p == 1 and qc > 0 and g in (2, 3, 4, 5):
                                emit_outproj_tile(qc - 1, g - 2, t_ps)
                        prev_acc = (qc, p, A0, A1, dcols)
                        parity ^= 1
                # tail: finish last (qc=3, p=1). The p=0 halves of the first
                # two out-projection tiles are issued into the freed score
                # banks while the DVE c-chain runs (PE would otherwise idle),
                # then completed with the p=1 halves after the final RMSNorm.
                emit_attn(pend_attn)
                st0 = boundary0(*prev_acc, copy=False)
                sA = s_ps.tile([128, 1024], F32, tag="s", name="opA")
                sB = s_ps.tile([128, 1024], F32, tag="s", name="opB")
                for tt4 in range(2):
                    tt = (NC4 - 1) * 4 + tt4
                    tsl = slice(tt * 128, (tt + 1) * 128)
                    hold = sA if tt4 == 0 else sB
                    for oc in range(2):
                        osl = slice(oc * 512, (oc + 1) * 512)
                        nc.tensor.matmul(hold[:, oc * 512:(oc + 1) * 512],
                                         oTn[0][:, tsl], wo_p(0)[:, osl],
                                         start=True, stop=False,
                                         skip_group_check=True)
                st2t = boundary1(st0)
                # tile 2's p0 halves into the A banks just freed by ou
                aC = a_ps.tile([128, 512], F32, tag="A", name="opC")
                aD = a_ps.tile([128, 512], F32, tag="A", name="opD")
                tt2sl = slice(((NC4 - 1) * 4 + 2) * 128, ((NC4 - 1) * 4 + 3) * 128)
                nc.tensor.matmul(aC[:], oTn[0][:, tt2sl], wo_p(0)[:, 0:512],
                                 start=True, stop=False, skip_group_check=True)
                nc.tensor.matmul(aD[:], oTn[0][:, tt2sl], wo_p(0)[:, 512:1024],
                                 start=True, stop=False, skip_group_check=True)
                boundary2(st2t, tail=True)
                for tt4 in range(2):
                    tt = (NC4 - 1) * 4 + tt4
                    tsl = slice(tt * 128, (tt + 1) * 128)
                    hold = sA if tt4 == 0 else sB
                    ot = o_sb.tile([128, DIM], F32, tag="ot", name="ot")
                    for oc in range(2):
                        osl = slice(oc * 512, (oc + 1) * 512)
                        nc.tensor.matmul(hold[:, oc * 512:(oc + 1) * 512],
                                         oTn[1][:, tsl], wo_p(1)[:, osl],
                                         start=False, stop=True,
                                         skip_group_check=True)
                        nc.scalar.activation(ot[:, osl],
                                             hold[:, oc * 512:(oc + 1) * 512],
                                             AF.Copy)
                        nc.sync.dma_start(out_[tsl, osl], ot[:, osl])
                ot2 = o_sb.tile([128, DIM], F32, tag="ot", name="ot2")
                for oc, hold in ((0, aC), (1, aD)):
                    osl = slice(oc * 512, (oc + 1) * 512)
                    nc.tensor.matmul(hold[:], oTn[1][:, tt2sl], wo_p(1)[:, osl],
                                     start=False, stop=True, skip_group_check=True)
                    nc.scalar.activation(ot2[:, osl], hold[:], AF.Copy)
                    nc.sync.dma_start(out_[tt2sl, osl], ot2[:, osl])
                emit_outproj_tile(NC4 - 1, 3, t_ps, split_dma=True)

    nc.finalize()
    return nc


def kernel(x, Wq, bq, Wk, bk, Wv, bv, Wo, bo, norm_w, lq1, lk1, lq2, lk2):
    x = np.asarray(x, dtype=np.float32)
    Wq = np.asarray(Wq, dtype=np.float32)
    Wk = np.asarray(Wk, dtype=np.float32)
    Wv = np.asarray(Wv, dtype=np.float32)
    Wo = np.asarray(Wo, dtype=np.float32)
    nw = np.asarray(norm_w, np.float32)

    if "nc" not in _CACHE:
        _CACHE["nc"] = _build_nc()
    nc = _CACHE["nc"]

    in_maps = []
    for c in range(8):
        b, g = c // 4, c % 4
        sl = slice(g * DLOC, (g + 1) * DLOC)
        # fold q-scale into Wq/bq; fold norm_w * (1-lambda_init) into Wo.
        # pre-tile [DIM, DLOC] -> [128, 8*DLOC] (block i at cols i*DLOC:)
        def tile8(w):  # w: [1024, 256] -> [128, 2048]
            return np.ascontiguousarray(
                w.reshape(8, 128, DLOC).transpose(1, 0, 2).reshape(128, 8 * DLOC))

        wo_scale = (nw[np.arange(DLOC) % (2 * HD)] * (1.0 - LAMBDA_INIT)).astype(np.float32)
        woT_full = (Wo[:, sl] * wo_scale[None, :]).T  # [256, 1024]
        in_maps.append({
            "x": np.ascontiguousarray(x[b]).astype(BF),
            "wqT": tile8((Wq[sl, :] * SCALE).T).astype(BF),
            "wkT": tile8(Wk[sl, :].T).astype(BF),
            "wvT": tile8(Wv[sl, :].T).astype(BF),
            "woT": np.ascontiguousarray(
                woT_full.reshape(2, 128, DIM).transpose(1, 0, 2).reshape(128, 2 * DIM)).astype(BF),
            "bpack": np.ascontiguousarray(np.stack([
                np.asarray(bq, np.float32)[sl][0:128] * SCALE,
                np.asarray(bq, np.float32)[sl][128:256] * SCALE,
                np.asarray(bk, np.float32)[sl][0:128],
                np.asarray(bk, np.float32)[sl][128:256],
            ], axis=1)),
            "bv": np.ascontiguousarray(np.asarray(bv, np.float32)[sl]),
            "lrow": np.ascontiguousarray(np.concatenate([
                np.asarray(lq1, np.float32), np.asarray(lk1, np.float32),
                np.asarray(lq2, np.float32), np.asarray(lk2, np.float32)])),
        })

    res = None
    for attempt in range(3):
        try:
            res = run_bass_kernel_spmd(nc, in_maps, list(range(8))).results
            break
        except Exception:
            if attempt == 2:
                raise
            import time as _time

            _time.sleep(5 + 10 * attempt)
            try:
                import jax

                jax.clear_caches()
                jax.extend.backend.clear_backends()
            except Exception:
                pass
    bo_f = np.asarray(bo, np.float32)
    out = np.empty((B, N, DIM), np.float32)
    for b in range(B):
        acc = res[4 * b]["out"].astype(np.float32)
        for g in range(1, 4):
            acc = acc + res[4 * b + g]["out"]
        out[b] = acc + bo_f[None, :]
    return out
